# revision 1
# baseline (speedup 1.0000x reference)
"""Trainium2 Bass kernel for nn_DetectionBEVLoss.

Takes FULL inputs (B=8,...), shards batch across 8 NeuronCores (one batch
element per core), computes per-core partial sums of the 6 loss terms plus the
positive count on-device, and finishes the tiny reduction on host.

The rotated-IoU is computed with a Liang-Barsky-style convex clip (Green's
theorem over the boundary of the intersection): for each edge of quad A, the
parameter interval [t0,t1] inside quad B is found from the 4 half-plane
constraints; the segment contributes cross(P(t0), P(t1)) to the boundary
integral. Summing over A-edges-in-B and B-edges-in-A gives 2*area(A^B).
No sorting, no arctan2, no gathers - pure elementwise ops.
"""
import numpy as np

P = 128
S = 512          # free-dim elements per channel slot (65536 px per core)
NPX = P * S
EPS = 1e-7
BIG = 1e30

_CACHE = {}


def _ensure_ntff_hook():
    import sys, types
    if "antenv.axon_hooks" in sys.modules:
        return
    try:
        import trn_agent_boot.trn_boot as tb
        hook = tb._ntff_profile_via_ctypes('/opt/axon/libaxon_pjrt.so')
        mod = types.ModuleType("antenv.axon_hooks")
        mod.get_axon_ntff_profile_hook = lambda: hook
        sys.modules["antenv.axon_hooks"] = mod
    except Exception:
        pass


def _build(debug=False, max_ops=10**9, lvl=99):
    import concourse.bacc as bacc
    import concourse.tile as tile
    import concourse.mybir as mybir
    import concourse.bass as bass

    F32 = mybir.dt.float32
    I32 = mybir.dt.int32
    U8 = mybir.dt.uint8
    Alu = mybir.AluOpType
    Act = mybir.ActivationFunctionType
    AX_X = mybir.AxisListType.X

    nc = bacc.Bacc("TRN2", target_bir_lowering=False, debug=False, num_devices=8)

    for v in [float(np.pi / 2)]:
        t = nc.alloc_sbuf_tensor(f"const-f32-{v}", [P, 1], F32)
        nc.gpsimd.memset(t.ap(), v)
        nc.const_aps.aps[(F32, v)] = t.ap()
    nc.all_engine_barrier()

    d_cls = nc.dram_tensor("cls", [10, NPX], F32, kind="ExternalInput")
    d_rp = nc.dram_tensor("regp", [9, NPX], F32, kind="ExternalInput")
    d_rt = nc.dram_tensor("regt", [9, NPX], F32, kind="ExternalInput")
    d_ioup = nc.dram_tensor("ioup", [P, S], F32, kind="ExternalInput")
    d_iout = nc.dram_tensor("iout", [P, S], F32, kind="ExternalInput")
    d_ct = nc.dram_tensor("ct", [P, S], I32, kind="ExternalInput")
    d_w = nc.dram_tensor("w", [P, S], F32, kind="ExternalInput")
    d_out = nc.dram_tensor("out", [P, 8], F32, kind="ExternalOutput")

    _opc = [0]

    class _Gated:
        _OPS = {"tensor_tensor", "tensor_scalar", "scalar_tensor_tensor",
                "tensor_copy", "copy_predicated", "tensor_reduce",
                "reciprocal_approx_fast", "activation", "memzero", "memset"}

        def __init__(self, eng):
            self._eng = eng

        def __getattr__(self, name):
            f = getattr(self._eng, name)
            if name in self._OPS:
                def g(*a, **k):
                    _opc[0] += 1
                    if _opc[0] > max_ops:
                        return None
                    return f(*a, **k)
                return g
            return f

    V = _Gated(nc.vector)
    A_ = _Gated(nc.scalar)

    dbg_outs = []

    def sl(t, i, k=1):
        return t[:, i * S:(i + k) * S]

    def dump(name, t):
        if not debug:
            return
        shp = [t.shape[0], int(np.prod(t.shape[1:]))]
        dt_ = t.dtype
        d = nc.dram_tensor(f"dbg_{name}", shp, dt_, kind="ExternalOutput")
        nc.sync.dma_start(out=d[:, :], in_=t)
        dbg_outs.append(name)

    def bc(t, i, k):
        b = t[:, i * S:(i + 1) * S]
        return bass.AP(tensor=b.tensor, offset=b.offset, ap=[b.ap[0], [0, k], [1, S]])

    with tile.TileContext(nc) as tc:
      with tc.tile_pool(name="persist", bufs=1) as pp:
        OUT = pp.tile([P, 8], F32, name="OUT")
        ACCS = pp.tile([P, 8], F32, name="ACCS")
        W = pp.tile([P, S], F32, name="W")
        CTF = pp.tile([P, S], F32, name="CTF")
        AXt = pp.tile([P, 4 * S], F32, name="AXt")
        AYt = pp.tile([P, 4 * S], F32, name="AYt")
        BXt = pp.tile([P, 4 * S], F32, name="BXt")
        BYt = pp.tile([P, 4 * S], F32, name="BYt")
        EAX = pp.tile([P, 4 * S], F32, name="EAX")
        EAY = pp.tile([P, 4 * S], F32, name="EAY")
        EBX = pp.tile([P, 4 * S], F32, name="EBX")
        EBY = pp.tile([P, 4 * S], F32, name="EBY")
        Vv = pp.tile([P, S], F32, name="Vv")
        D2C2 = pp.tile([P, S], F32, name="D2C2")
        AAB = pp.tile([P, S], F32, name="AAB")
        SAB = pp.tile([P, S], F32, name="SAB")

        nc.sync.dma_start(out=W, in_=d_w[:, :])
        V.tensor_reduce(ACCS[:, 6:7], W, AX_X, Alu.add)
        dump("W", W)
        dump("npos", ACCS[:, 6:7])

        # ============ phase 1: reg tensors resident ============
        with tc.tile_pool(name="io1", bufs=1) as pio:
            RP = pio.tile([P, 9 * S], F32, name="RP")
            RT = pio.tile([P, 9 * S], F32, name="RT")
            IOUP = pio.tile([P, S], F32, name="IOUP")
            IOUT = pio.tile([P, S], F32, name="IOUT")
            CTI = pio.tile([P, S], I32, name="CTI")
            nc.sync.dma_start(out=RP.rearrange("p (c f) -> p c f", c=9),
                              in_=d_rp[:, :].rearrange("c (p f) -> p c f", p=P))
            nc.sync.dma_start(out=RT.rearrange("p (c f) -> p c f", c=9),
                              in_=d_rt[:, :].rearrange("c (p f) -> p c f", p=P))
            nc.sync.dma_start(out=IOUP, in_=d_ioup[:, :])
            nc.sync.dma_start(out=IOUT, in_=d_iout[:, :])
            nc.sync.dma_start(out=CTI, in_=d_ct[:, :])
            V.tensor_copy(CTF, CTI)

            with tc.tile_pool(name="geo", bufs=1) as pg:
                def scr(nm, tg, dt=F32):
                    return pg.tile([P, S], dt, name=nm, tag=tg)

                # ---- corners (Sin set: trig_and_small) ----
                for (qt, R, CX, CY) in ((("a", RP, AXt, AYt), ("b", RT, BXt, BYt)) if lvl >= 1 else ()):
                    cx, cy = sl(R, 0), sl(R, 1)
                    l, w_ = sl(R, 3), sl(R, 4)
                    yaw = sl(R, 6)
                    co = scr(f"co_{qt}", "s0"); si = scr(f"si_{qt}", "s1")
                    A_.activation(co, yaw, Act.Sin, bias=float(np.pi / 2))
                    A_.activation(si, yaw, Act.Sin)
                    u = scr(f"u_{qt}", "s2"); v2 = scr(f"v_{qt}", "s3")
                    pq = scr(f"p_{qt}", "s4"); qq = scr(f"q_{qt}", "s5")
                    V.scalar_tensor_tensor(u, l, 0.5, co, Alu.mult, Alu.mult)
                    V.scalar_tensor_tensor(v2, w_, 0.5, si, Alu.mult, Alu.mult)
                    V.scalar_tensor_tensor(pq, l, 0.5, si, Alu.mult, Alu.mult)
                    V.scalar_tensor_tensor(qq, w_, 0.5, co, Alu.mult, Alu.mult)
                    As_ = scr(f"As_{qt}", "s6"); Ad = scr(f"Ad_{qt}", "s7")
                    Ps_ = scr(f"Ps_{qt}", "s8"); Pd = scr(f"Pd_{qt}", "s9")
                    V.tensor_tensor(As_, u, v2, Alu.add)
                    V.tensor_tensor(Ad, u, v2, Alu.subtract)
                    V.tensor_tensor(Ps_, pq, qq, Alu.add)
                    V.tensor_tensor(Pd, pq, qq, Alu.subtract)
                    V.tensor_tensor(sl(CX, 0), cx, As_, Alu.add)
                    V.tensor_tensor(sl(CX, 1), cx, Ad, Alu.subtract)
                    V.tensor_tensor(sl(CX, 2), cx, As_, Alu.subtract)
                    V.tensor_tensor(sl(CX, 3), cx, Ad, Alu.add)
                    V.tensor_tensor(sl(CY, 0), cy, Pd, Alu.add)
                    V.tensor_tensor(sl(CY, 1), cy, Ps_, Alu.subtract)
                    V.tensor_tensor(sl(CY, 2), cy, Pd, Alu.subtract)
                    V.tensor_tensor(sl(CY, 3), cy, Ps_, Alu.add)
                for (E, C) in (((EAX, AXt), (EAY, AYt), (EBX, BXt), (EBY, BYt)) if lvl >= 1 else ()):
                    V.tensor_tensor(E[:, 0:3 * S], C[:, S:4 * S], C[:, 0:3 * S], Alu.subtract)
                    V.tensor_tensor(sl(E, 3), sl(C, 0), sl(C, 3), Alu.subtract)

                # ---- v term (Arctan, still trig set) ----
                ATP = pg.tile([P, S], F32, name="ATP")
                ATT = pg.tile([P, S], F32, name="ATT")
                for (qt, R, AT) in ((("a", RP, ATP), ("b", RT, ATT)) if lvl >= 2 else ()):
                    l, w_ = sl(R, 3), sl(R, 4)
                    lm = scr(f"lm_{qt}", "s0"); wm = scr(f"wm_{qt}", "s1")
                    V.tensor_scalar(lm, l, EPS, None, Alu.max)
                    V.tensor_scalar(wm, w_, 1e-30, None, Alu.max)
                    rl = scr(f"rl_{qt}", "s2"); rw = scr(f"rw_{qt}", "s3")
                    V.reciprocal_approx_fast(rl, lm)
                    V.reciprocal_approx_fast(rw, wm)
                    x1 = scr(f"x1_{qt}", "s4"); x2 = scr(f"x2_{qt}", "s5")
                    V.tensor_tensor(x1, w_, rl, Alu.mult)
                    V.tensor_tensor(x2, lm, rw, Alu.mult)
                    mn = scr(f"mn_{qt}", "s6")
                    V.tensor_tensor(mn, x1, x2, Alu.min)
                    aa = scr(f"aa_{qt}", "s7")
                    A_.activation(aa, mn, Act.Arctan)
                    tt = scr(f"tt_{qt}", "s8")
                    V.tensor_scalar(tt, aa, -2.0, float(np.pi / 2), Alu.mult, Alu.add)
                    m8 = scr(f"m8_{qt}", "s10", U8)
                    V.tensor_scalar(m8, x1, 1.0, None, Alu.is_gt)
                    mf = scr(f"mf_{qt}", "s9")
                    V.tensor_copy(mf, m8)
                    V.tensor_tensor(tt, tt, mf, Alu.mult)
                    V.tensor_tensor(AT, aa, tt, Alu.add)
                for nm, t in ((("AXt", AXt), ("AYt", AYt), ("BXt", BXt), ("BYt", BYt),
                              ("EAX", EAX), ("EAY", EAY), ("EBX", EBX), ("EBY", EBY)) if lvl >= 1 else ()):
                    dump(nm, t)
                if lvl >= 2:
                    dump("ATP", ATP); dump("ATT", ATT)
                if lvl >= 2:
                    DV = scr("DV", "s0")
                    V.tensor_tensor(DV, ATP, ATT, Alu.subtract)
                    A_.activation(Vv, DV, Act.Square, scale=float(2.0 / np.pi))
                    dump("Vv", Vv)

                # ---- d2 / c2 ----
                if lvl >= 3:
                  DX = scr("DX", "s1"); DY = scr("DY", "s2")
                  V.tensor_tensor(DX, sl(RP, 0), sl(RT, 0), Alu.subtract)
                  V.tensor_tensor(DY, sl(RP, 1), sl(RT, 1), Alu.subtract)
                  SQX = scr("SQX", "s3"); SQY = scr("SQY", "s4")
                  A_.activation(SQX, DX, Act.Square)
                  A_.activation(SQY, DY, Act.Square)
                  D2 = scr("D2", "s5")
                  V.tensor_tensor(D2, SQX, SQY, Alu.add)
                  FOLD2 = pg.tile([P, 2 * S], F32, name="FOLD2")
                  EXT = pg.tile([P, 4 * S], F32, name="EXT")
                  for (idx, C1t, C2t, op) in ((0, AXt, BXt, Alu.max), (1, AXt, BXt, Alu.min),
                                              (2, AYt, BYt, Alu.max), (3, AYt, BYt, Alu.min)):
                      V.tensor_tensor(FOLD2, C1t[:, 0:2 * S], C1t[:, 2 * S:4 * S], op)
                      V.tensor_tensor(sl(FOLD2, 0), sl(FOLD2, 0), sl(FOLD2, 1), op)
                      V.tensor_tensor(sl(FOLD2, 1), sl(C2t, 0), sl(C2t, 1), op)
                      V.tensor_tensor(sl(FOLD2, 1), sl(FOLD2, 1), sl(C2t, 2), op)
                      V.tensor_tensor(sl(FOLD2, 1), sl(FOLD2, 1), sl(C2t, 3), op)
                      V.tensor_tensor(sl(EXT, idx), sl(FOLD2, 0), sl(FOLD2, 1), op)
                  BW = scr("BW", "s6"); BH = scr("BH", "s7")
                  V.tensor_tensor(BW, sl(EXT, 0), sl(EXT, 1), Alu.subtract)
                  V.tensor_tensor(BH, sl(EXT, 2), sl(EXT, 3), Alu.subtract)
                  SQW = scr("SQW", "s8"); SQH = scr("SQH", "s9")
                  A_.activation(SQW, BW, Act.Square)
                  A_.activation(SQH, BH, Act.Square)
                  C2v = scr("C2v", "s1")
                  V.tensor_tensor(C2v, SQW, SQH, Alu.add)
                  V.tensor_scalar(C2v, C2v, EPS, None, Alu.max)
                  RC2 = scr("RC2", "s2")
                  V.reciprocal_approx_fast(RC2, C2v)
                  V.tensor_tensor(D2C2, D2, RC2, Alu.mult)
                  dump("D2C2", D2C2)

                  # ---- areas ----
                  ARA = scr("ARA", "s3"); ARB = scr("ARB", "s4")
                  V.tensor_tensor(ARA, sl(RP, 3), sl(RP, 4), Alu.mult)
                  V.tensor_tensor(ARB, sl(RT, 3), sl(RT, 4), Alu.mult)
                  V.tensor_tensor(AAB, ARA, ARB, Alu.add)
                  dump("AAB", AAB)

                # ---- smooth L1 (z=2, h=5, vel=7,8) ----
                if lvl >= 4:
                  vacc7 = pg.tile([P, 1], F32, name="vacc7")
                  vacc8 = pg.tile([P, 1], F32, name="vacc8")
                  for (cidx, col) in ((2, 2), (5, 3), (7, 4), (8, 5)):
                      D = scr(f"D_{cidx}", "s5")
                      V.tensor_tensor(D, sl(RP, cidx), sl(RT, cidx), Alu.subtract)
                      AD = scr(f"AD_{cidx}", "s6")
                      A_.activation(AD, D, Act.Abs)
                      Q = scr(f"Q_{cidx}", "s7")
                      A_.activation(Q, AD, Act.Square, scale=float(np.sqrt(0.5)))
                      L = scr(f"L_{cidx}", "s8")
                      A_.activation(L, AD, Act.Copy, bias=-0.5)
                      M8s = scr(f"M8_{cidx}", "s10", U8)
                      V.tensor_scalar(M8s, AD, 1.0, None, Alu.is_lt)
                      V.copy_predicated(L, M8s, Q)
                      V.tensor_tensor(L, L, W, Alu.mult)
                      if cidx == 7:
                          V.tensor_reduce(vacc7, L, AX_X, Alu.add)
                      elif cidx == 8:
                          V.tensor_reduce(vacc8, L, AX_X, Alu.add)
                      else:
                          V.tensor_reduce(ACCS[:, col:col + 1], L, AX_X, Alu.add)
                  V.tensor_tensor(ACCS[:, 4:5], vacc7, vacc8, Alu.add)

                # ---- BCE (switches to natural_log_exp set) ----
                if lvl >= 5:
                  AXb = scr("AXb", "s0")
                  A_.activation(AXb, IOUP, Act.Abs)
                  EB = scr("EB", "s1")
                  A_.activation(EB, AXb, Act.Exp, scale=-1.0)
                  L1P = scr("L1P", "s2")
                  A_.activation(L1P, EB, Act.Ln, bias=1.0)
                  RL = scr("RL", "s3")
                  V.tensor_scalar(RL, IOUP, 0.0, None, Alu.max)
                  XY = scr("XY", "s4")
                  V.tensor_tensor(XY, IOUP, IOUT, Alu.mult)
                  V.tensor_tensor(RL, RL, XY, Alu.subtract)
                  V.tensor_tensor(RL, RL, L1P, Alu.add)
                  V.tensor_tensor(RL, RL, W, Alu.mult)
                  V.tensor_reduce(ACCS[:, 5:6], RL, AX_X, Alu.add)
                  dump("accs_p1", ACCS)

        # ============ phase 2: focal loss (CLS resident) ============
        if lvl >= 6:
          with tc.tile_pool(name="io2", bufs=1) as pf:
              CLS = pf.tile([P, 10 * S], F32, name="CLS")
              nc.sync.dma_start(out=CLS.rearrange("p (c f) -> p c f", c=10),
                                in_=d_cls[:, :].rearrange("c (p f) -> p c f", p=P))
              SC = pf.tile([P, 5 * S], F32, name="SC")
              V.tensor_tensor(SC, CLS[:, 0:5 * S], CLS[:, 5 * S:10 * S], Alu.max)
              V.tensor_tensor(SC[:, 0:2 * S], SC[:, 0:2 * S], SC[:, 2 * S:4 * S], Alu.max)
              V.tensor_tensor(sl(SC, 0), sl(SC, 0), sl(SC, 1), Alu.max)
              M = pf.tile([P, S], F32, name="M")
              V.tensor_tensor(M, sl(SC, 0), sl(SC, 4), Alu.max)
              XS = pf.tile([P, 10 * S], F32, name="XS")
              V.tensor_tensor(XS, CLS, bc(M, 0, 10), Alu.subtract)
              E = pf.tile([P, 10 * S], F32, name="E")
              A_.activation(E, XS, Act.Exp)
              V.tensor_tensor(SC, E[:, 0:5 * S], E[:, 5 * S:10 * S], Alu.add)
              V.tensor_tensor(SC[:, 0:2 * S], SC[:, 0:2 * S], SC[:, 2 * S:4 * S], Alu.add)
              V.tensor_tensor(sl(SC, 0), sl(SC, 0), sl(SC, 1), Alu.add)
              Ssum = pf.tile([P, S], F32, name="Ssum")
              V.tensor_tensor(Ssum, sl(SC, 0), sl(SC, 4), Alu.add)
              ET = pf.tile([P, S], F32, name="ET")
              V.tensor_copy(ET, sl(E, 0))
              for c in range(1, 10):
                  MC = pf.tile([P, S], U8, name=f"MC_{c}", tag="MC")
                  V.tensor_scalar(MC, CTF, float(c), None, Alu.is_equal)
                  V.copy_predicated(ET, MC, sl(E, c))
              RS = pf.tile([P, S], F32, name="RS")
              V.reciprocal_approx_fast(RS, Ssum)
              PT = pf.tile([P, S], F32, name="PT")
              V.tensor_tensor(PT, ET, RS, Alu.mult)
              V.tensor_scalar(PT, PT, EPS, 1.0 - EPS, Alu.max, Alu.min)
              LG = pf.tile([P, S], F32, name="LG")
              A_.activation(LG, PT, Act.Ln)
              OMP = pf.tile([P, S], F32, name="OMP")
              V.tensor_scalar(OMP, PT, -1.0, 1.0, Alu.mult, Alu.add)
              MPOS = pf.tile([P, S], U8, name="MPOS")
              V.tensor_scalar(MPOS, CTF, 0.0, None, Alu.is_gt)
              MPF = pf.tile([P, S], F32, name="MPF")
              V.tensor_copy(MPF, MPOS)
              ALPH = pf.tile([P, S], F32, name="ALPH")
              V.tensor_scalar(ALPH, MPF, -0.5, 0.75, Alu.mult, Alu.add)
              FL = pf.tile([P, S], F32, name="FL")
              V.tensor_tensor(FL, OMP, OMP, Alu.mult)
              V.tensor_tensor(FL, FL, LG, Alu.mult)
              V.scalar_tensor_tensor(FL, FL, -1.0, ALPH, Alu.mult, Alu.mult)
              V.tensor_reduce(ACCS[:, 0:1], FL, AX_X, Alu.add)
              dump("Mx", M)
              dump("Ssum", Ssum)
              dump("ET", ET)
              dump("PT", PT)
              dump("FL", FL)
              dump("accs_p2", ACCS)

        # ============ phase 3: clip passes ============
        with tc.tile_pool(name="persist2", bufs=1) as pp2:
            _passes = []
            if lvl >= 7:
                _passes.append((0, AXt, AYt, EAX, EAY, BXt, BYt, EBX, EBY))
            if lvl >= 8:
                _passes.append((1, BXt, BYt, EBX, EBY, AXt, AYt, EAX, EAY))
            for (pi, SXt, SYt, SEX, SEY, PXt, PYt, PEX, PEY) in _passes:
                with tc.tile_pool(name=f"clip{pi}", bufs=1) as pc:
                    Cj = pc.tile([P, 4 * S], F32, name=f"Cj_{pi}")
                    Tc1 = pc.tile([P, 4 * S], F32, name=f"Tc1_{pi}", tag="T2j")
                    V.tensor_tensor(Tc1, PEY, PXt, Alu.mult)
                    V.tensor_tensor(Cj, PEX, PYt, Alu.mult)
                    V.tensor_tensor(Cj, Tc1, Cj, Alu.subtract)
                    LO = pc.tile([P, 4 * S], F32, name=f"LO_{pi}")
                    HI = pc.tile([P, 4 * S], F32, name=f"HI_{pi}")
                    for j in range(4):
                        Fj = pc.tile([P, 4 * S], F32, name=f"F_{pi}_{j}", tag="Fj")
                        T2_ = pc.tile([P, 4 * S], F32, name=f"T2_{pi}_{j}", tag="T2j")
                        V.tensor_tensor(Fj, bc(PEY, j, 4), SXt, Alu.mult)
                        V.tensor_tensor(T2_, bc(PEX, j, 4), SYt, Alu.mult)
                        V.tensor_tensor(Fj, Fj, T2_, Alu.subtract)
                        V.tensor_tensor(Fj, Fj, bc(Cj, j, 4), Alu.subtract)
                        C1 = pc.tile([P, 4 * S], F32, name=f"C1_{pi}_{j}", tag="C1j")
                        V.tensor_tensor(C1[:, 0:3 * S], Fj[:, S:4 * S], Fj[:, 0:3 * S], Alu.subtract)
                        V.tensor_tensor(sl(C1, 3), sl(Fj, 0), sl(Fj, 3), Alu.subtract)
                        REC = pc.tile([P, 4 * S], F32, name=f"REC_{pi}_{j}", tag="RECj")
                        V.reciprocal_approx_fast(REC, C1)
                        U = pc.tile([P, 4 * S], F32, name=f"U_{pi}_{j}", tag="Uj")
                        V.scalar_tensor_tensor(U, Fj, -1.0, REC, Alu.mult, Alu.mult)
                        SG = pc.tile([P, 4 * S], F32, name=f"SG_{pi}_{j}", tag="SGj")
                        A_.activation(SG, C1, Act.Sign)
                        if j == 0:
                            V.scalar_tensor_tensor(LO, SG, BIG, U, Alu.mult, Alu.min)
                            V.scalar_tensor_tensor(HI, SG, BIG, U, Alu.mult, Alu.max)
                        else:
                            LOj = pc.tile([P, 4 * S], F32, name=f"LOj_{pi}_{j}", tag="LOj")
                            HIj = pc.tile([P, 4 * S], F32, name=f"HIj_{pi}_{j}", tag="HIj")
                            V.scalar_tensor_tensor(LOj, SG, BIG, U, Alu.mult, Alu.min)
                            V.scalar_tensor_tensor(HIj, SG, BIG, U, Alu.mult, Alu.max)
                            V.tensor_tensor(LO, LO, LOj, Alu.max)
                            V.tensor_tensor(HI, HI, HIj, Alu.min)
                    T0 = pc.tile([P, 4 * S], F32, name=f"T0_{pi}")
                    T1v = pc.tile([P, 4 * S], F32, name=f"T1v_{pi}")
                    V.tensor_scalar(T0, LO, 0.0, 1.0, Alu.max, Alu.min)
                    V.tensor_scalar(T1v, HI, 1.0, 0.0, Alu.min, Alu.max)
                    P0X = pc.tile([P, 4 * S], F32, name=f"P0X_{pi}", tag="Fj")
                    P0Y = pc.tile([P, 4 * S], F32, name=f"P0Y_{pi}", tag="C1j")
                    P1X = pc.tile([P, 4 * S], F32, name=f"P1X_{pi}", tag="RECj")
                    P1Y = pc.tile([P, 4 * S], F32, name=f"P1Y_{pi}", tag="Uj")
                    V.tensor_tensor(P0X, T0, SEX, Alu.mult)
                    V.tensor_tensor(P0X, P0X, SXt, Alu.add)
                    V.tensor_tensor(P0Y, T0, SEY, Alu.mult)
                    V.tensor_tensor(P0Y, P0Y, SYt, Alu.add)
                    V.tensor_tensor(P1X, T1v, SEX, Alu.mult)
                    V.tensor_tensor(P1X, P1X, SXt, Alu.add)
                    V.tensor_tensor(P1Y, T1v, SEY, Alu.mult)
                    V.tensor_tensor(P1Y, P1Y, SYt, Alu.add)
                    CR = pc.tile([P, 4 * S], F32, name=f"CR_{pi}", tag="T2j")
                    TM = pc.tile([P, 4 * S], F32, name=f"TM_{pi}", tag="SGj")
                    V.tensor_tensor(CR, P0X, P1Y, Alu.mult)
                    V.tensor_tensor(TM, P0Y, P1X, Alu.mult)
                    V.tensor_tensor(CR, CR, TM, Alu.subtract)
                    MK = pc.tile([P, 4 * S], U8, name=f"MK_{pi}")
                    V.tensor_tensor(MK, T1v, T0, Alu.is_gt)
                    CONTR = pc.tile([P, 4 * S], F32, name=f"CONTR_{pi}", tag="LOj")
                    A_.memzero(CONTR)
                    V.copy_predicated(CONTR, MK, CR)
                    FL2 = pc.tile([P, 2 * S], F32, name=f"FL2_{pi}", tag="HIj")
                    V.tensor_tensor(FL2, CONTR[:, 0:2 * S], CONTR[:, 2 * S:4 * S], Alu.add)
                    dump(f"T0_{pi}", T0)
                    dump(f"T1v_{pi}", T1v)
                    dump(f"CONTR_{pi}", CONTR)
                    if pi == 0:
                        V.tensor_tensor(SAB, sl(FL2, 0), sl(FL2, 1), Alu.add)
                    else:
                        SP2 = pc.tile([P, S], F32, name="SP2", tag="MKx")
                        V.tensor_tensor(SP2, sl(FL2, 0), sl(FL2, 1), Alu.add)
                        V.tensor_tensor(SAB, SAB, SP2, Alu.add)

            # ---- iou + bev assembly ----
            if lvl >= 8:
              INTER = pp2.tile([P, S], F32, name="INTER")
              A_.activation(INTER, SAB, Act.Abs, scale=0.5)
              UN = pp2.tile([P, S], F32, name="UN")
              V.tensor_tensor(UN, AAB, INTER, Alu.subtract)
              V.tensor_scalar(UN, UN, 1e-7, None, Alu.max)
              URC = pp2.tile([P, S], F32, name="URC")
              V.reciprocal_approx_fast(URC, UN)
              IOU = pp2.tile([P, S], F32, name="IOU")
              V.tensor_tensor(IOU, INTER, URC, Alu.mult)
              DEN = pp2.tile([P, S], F32, name="DEN")
              V.tensor_scalar(DEN, Vv, float(1.0 + EPS), None, Alu.add)
              V.tensor_tensor(DEN, DEN, IOU, Alu.subtract)
              DRC = pp2.tile([P, S], F32, name="DRC")
              V.reciprocal_approx_fast(DRC, DEN)
              ALC = pp2.tile([P, S], F32, name="ALC")
              V.tensor_tensor(ALC, Vv, DRC, Alu.mult)
              LB = pp2.tile([P, S], F32, name="LB")
              V.tensor_scalar(LB, IOU, -1.0, 1.0, Alu.mult, Alu.add)
              V.tensor_tensor(LB, LB, D2C2, Alu.add)
              V.tensor_tensor(ALC, ALC, Vv, Alu.mult)
              V.tensor_tensor(LB, LB, ALC, Alu.add)
              V.tensor_tensor(LB, LB, W, Alu.mult)
              V.tensor_reduce(ACCS[:, 1:2], LB, AX_X, Alu.add)
              dump("SAB", SAB)
              dump("INTER", INTER)
              dump("IOU", IOU)
              dump("LBW", LB)

            dump("W_end", W)
            dump("CTF_end", CTF)
            A_.memzero(ACCS[:, 7:8])
            V.tensor_copy(OUT, ACCS)
            nc.sync.dma_start(out=d_out[:, :], in_=OUT)

    nc.compile()
    nc._dbg_opcount = _opc[0]
    return nc


def _get_nc():
    if "nc" not in _CACHE:
        _ensure_ntff_hook()
        _CACHE["nc"] = _build()
    return _CACHE["nc"]


def kernel(**inputs):
    from concourse.bass_utils import run_bass_kernel_spmd

    nc = _get_nc()
    cls_pred = np.asarray(inputs["cls_pred"], dtype=np.float32)
    reg_pred = np.asarray(inputs["reg_pred"], dtype=np.float32)
    iou_pred = np.asarray(inputs["iou_pred"], dtype=np.float32)
    cls_targets = np.asarray(inputs["cls_targets"], dtype=np.int32)
    reg_targets = np.asarray(inputs["reg_targets"], dtype=np.float32)
    reg_weights = np.asarray(inputs["reg_weights"], dtype=np.float32)
    iou_targets = np.asarray(inputs["iou_targets"], dtype=np.float32)

    B = cls_pred.shape[0]
    in_maps = []
    for b in range(B):
        in_maps.append({
            "cls": np.ascontiguousarray(cls_pred[b].reshape(10, NPX)),
            "regp": np.ascontiguousarray(reg_pred[b].reshape(9, NPX)),
            "regt": np.ascontiguousarray(reg_targets[b].reshape(9, NPX)),
            "ioup": np.ascontiguousarray(iou_pred[b].reshape(P, S)),
            "iout": np.ascontiguousarray(iou_targets[b].reshape(P, S)),
            "ct": np.ascontiguousarray(cls_targets[b].reshape(P, S)),
            "w": np.ascontiguousarray(reg_weights[b].reshape(P, S)),
        })
    res = run_bass_kernel_spmd(nc, in_maps, core_ids=list(range(8)))
    _CACHE["last_result"] = res
    sums = np.zeros(8, np.float64)
    for r in res.results:
        sums += r["out"].astype(np.float64).sum(axis=0)
    num_pos = max(sums[6], 1.0)
    out = np.array([sums[0], sums[1], sums[2], sums[3], sums[4], sums[5]],
                   np.float64) / num_pos
    return out.astype(np.float32)



# revision 6
# speedup vs baseline: 1.9284x; 1.9284x over previous
"""Trainium2 Bass kernel for nn_DetectionBEVLoss (bf16 pipeline).

Takes FULL inputs (B=8,...), shards batch across 8 NeuronCores (one batch
element per core), computes per-core partial sums of the 6 loss terms plus the
positive count on-device, and finishes the tiny reduction on host.

Key optimizations over the f32 baseline:
 - host casts inputs to bf16 and pre-arranges [P, C*S] layouts (half the DMA,
   no on-device transposes/casts; bf16 doubles DVE throughput via 2x packing)
 - rotated rects are parallelograms: opposite edges are +/-E, so each clip
   pass needs only 2 shared cross-product tensors G (not 4), and each edge
   pair forms a slab whose inside-interval is [min(U0,U2), max(U0,U2)] --
   no sign/BIG bookkeeping
 - Green's theorem: the boundary contribution of a clipped segment is
   (t1-t0)*cross(S_k, S_{k+1}); no intersection points are materialized
 - single-source ops (sin, arctan, exp, ln, softplus, square, abs, relu,
   reciprocal) run on the otherwise-idle Scalar/Act engine; reciprocal
   degeneracy is handled with a 1e-30 bias folded into the Act op
 - alpha_c denominator computed as relu(1-iou)+v to survive bf16 rounding
"""
import numpy as np

P = 128
S = 512          # free-dim elements per channel slot (65536 px per core)
NPX = P * S
EPS = 1e-7

_CACHE = {}


def _ensure_ntff_hook():
    import sys, types
    if "antenv.axon_hooks" in sys.modules:
        return
    try:
        import trn_agent_boot.trn_boot as tb
        hook = tb._ntff_profile_via_ctypes('/opt/axon/libaxon_pjrt.so')
        mod = types.ModuleType("antenv.axon_hooks")
        mod.get_axon_ntff_profile_hook = lambda: hook
        sys.modules["antenv.axon_hooks"] = mod
    except Exception:
        pass


def _build(debug=False, lvl=99):
    import concourse.bacc as bacc
    import concourse.tile as tile
    import concourse.mybir as mybir
    import concourse.bass as bass

    F32 = mybir.dt.float32
    BF = mybir.dt.bfloat16
    U8 = mybir.dt.uint8
    Alu = mybir.AluOpType
    Act = mybir.ActivationFunctionType
    AX_X = mybir.AxisListType.X
    PI2 = float(np.pi / 2)

    nc = bacc.Bacc("TRN2", target_bir_lowering=False, debug=False, num_devices=8)

    for v in [PI2, 1e-30, 1.0]:
        t = nc.alloc_sbuf_tensor(f"const-f32-{v}", [P, 1], F32)
        nc.gpsimd.memset(t.ap(), v)
        nc.const_aps.aps[(F32, v)] = t.ap()
    nc.all_engine_barrier()

    d_cls = nc.dram_tensor("cls", [P, 10 * S], BF, kind="ExternalInput")
    d_geop = nc.dram_tensor("geop", [P, 5 * S], BF, kind="ExternalInput")
    d_geot = nc.dram_tensor("geot", [P, 5 * S], BF, kind="ExternalInput")
    d_zbp = nc.dram_tensor("zbp", [P, 4 * S], BF, kind="ExternalInput")
    d_zbt = nc.dram_tensor("zbt", [P, 4 * S], BF, kind="ExternalInput")
    d_ioup = nc.dram_tensor("ioup", [P, S], BF, kind="ExternalInput")
    d_iout = nc.dram_tensor("iout", [P, S], BF, kind="ExternalInput")
    d_ctf = nc.dram_tensor("ctf", [P, S], BF, kind="ExternalInput")
    d_w = nc.dram_tensor("w", [P, S], BF, kind="ExternalInput")
    d_out = nc.dram_tensor("out", [P, 8], F32, kind="ExternalOutput")

    V = nc.vector
    A = nc.scalar
    G = nc.gpsimd

    dbg_outs = []

    def dump(name, t):
        if not debug:
            return
        shp = [t.shape[0], int(np.prod(t.shape[1:]))]
        d = nc.dram_tensor(f"dbg_{name}", shp, t.dtype, kind="ExternalOutput")
        nc.sync.dma_start(out=d[:, :], in_=t)
        dbg_outs.append(name)

    def bc(t, i, k):
        # broadcast S-slice i of tile t over k slots
        b_ = t[:, i * S:(i + 1) * S]
        return bass.AP(tensor=b_.tensor, offset=b_.offset,
                       ap=[b_.ap[0], [0, k], [1, S]])

    def sl(t, i, k=1):
        return t[:, i * S:(i + k) * S]

    with tile.TileContext(nc) as tc:
      with tc.tile_pool(name="persist", bufs=1) as pp:
        ACCS = pp.tile([P, 8], F32, name="ACCS")
        ZACC = pp.tile([P, 4], F32, name="ZACC")
        W = pp.tile([P, S], BF, name="W")
        CTF = pp.tile([P, S], BF, name="CTF")
        IOUP = pp.tile([P, S], BF, name="IOUP")
        IOUT = pp.tile([P, S], BF, name="IOUT")
        GEOP = pp.tile([P, 5 * S], BF, name="GEOP")
        GEOT = pp.tile([P, 5 * S], BF, name="GEOT")
        CORX = pp.tile([P, 8 * S], BF, name="CORX")   # [AX(4S) | BX(4S)]
        CORY = pp.tile([P, 8 * S], BF, name="CORY")
        CA = pp.tile([P, 4 * S], BF, name="CA")
        CB = pp.tile([P, 4 * S], BF, name="CB")
        UVT = pp.tile([P, 8 * S], BF, name="UVT")     # uxA vxA uyA vyA uxB vxB uyB vyB
        SAB = pp.tile([P, S], BF, name="SAB")
        Vv = pp.tile([P, S], BF, name="Vv")
        D2C2 = pp.tile([P, S], BF, name="D2C2")

        AX4 = CORX[:, 0:4 * S]; BX4 = CORX[:, 4 * S:8 * S]
        AY4 = CORY[:, 0:4 * S]; BY4 = CORY[:, 4 * S:8 * S]

        nc.sync.dma_start(out=W, in_=d_w[:, :])
        nc.sync.dma_start(out=CTF, in_=d_ctf[:, :])
        nc.sync.dma_start(out=IOUP, in_=d_ioup[:, :])
        nc.sync.dma_start(out=IOUT, in_=d_iout[:, :])
        nc.sync.dma_start(out=GEOP, in_=d_geop[:, :])
        nc.sync.dma_start(out=GEOT, in_=d_geot[:, :])

        V.tensor_reduce(ACCS[:, 6:7], W, AX_X, Alu.add)

        # ============ smooth-L1 block (z,h,vel channels) ============
        if lvl >= 1:
          with tc.tile_pool(name="sl1", bufs=1) as ps:
            ZBP = ps.tile([P, 4 * S], BF, name="ZBP")
            ZBT = ps.tile([P, 4 * S], BF, name="ZBT")
            nc.sync.dma_start(out=ZBP, in_=d_zbp[:, :])
            nc.sync.dma_start(out=ZBT, in_=d_zbt[:, :])
            D = ps.tile([P, 4 * S], BF, name="D")
            AD = ps.tile([P, 4 * S], BF, name="AD")
            M = ps.tile([P, 4 * S], BF, name="M")
            MD = ps.tile([P, 4 * S], BF, name="MD")
            M2H = ps.tile([P, 4 * S], BF, name="M2H")
            SL1 = ps.tile([P, 4 * S], BF, name="SL1")
            V.tensor_tensor(D, ZBP, ZBT, Alu.subtract)
            A.activation(AD, D, Act.Abs)
            V.tensor_scalar(M, AD, 1.0, None, Alu.min)
            V.tensor_tensor(MD, M, AD, Alu.mult)
            A.activation(M2H, M, Act.Square, scale=float(np.sqrt(0.5)))
            V.tensor_tensor(SL1, MD, M2H, Alu.subtract)
            V.tensor_tensor(SL1, SL1, bc(W, 0, 4), Alu.mult)
            V.tensor_reduce(ZACC, SL1.rearrange("p (c f) -> p c f", c=4),
                            AX_X, Alu.add)
            V.tensor_copy(ACCS[:, 2:3], ZACC[:, 0:1])
            V.tensor_copy(ACCS[:, 3:4], ZACC[:, 1:2])
            V.tensor_tensor(ACCS[:, 4:5], ZACC[:, 2:3], ZACC[:, 3:4], Alu.add)
            dump("SL1", SL1)

        # ============ BCE (iou head) ============
        if lvl >= 2:
          with tc.tile_pool(name="bce", bufs=1) as pb:
            AXb = pb.tile([P, S], BF, name="AXb")
            SP = pb.tile([P, S], BF, name="SP")
            RL = pb.tile([P, S], BF, name="RL")
            XY = pb.tile([P, S], BF, name="XY")
            A.activation(AXb, IOUP, Act.Abs)
            EB = pb.tile([P, S], BF, name="EB")
            A.activation(EB, AXb, Act.Exp, scale=-1.0)
            A.activation(SP, EB, Act.Ln, bias=1.0)
            A.activation(RL, IOUP, Act.Relu)
            V.tensor_tensor(XY, IOUP, IOUT, Alu.mult)
            V.tensor_tensor(RL, RL, XY, Alu.subtract)
            V.tensor_tensor(RL, RL, SP, Alu.add)
            V.tensor_tensor(RL, RL, W, Alu.mult)
            V.tensor_reduce(ACCS[:, 5:6], RL, AX_X, Alu.add)
            dump("BCE", RL)

        # ============ corners + uv smalls ============
        # GEO layout: [cx, cy, l, w, yaw] at slots 0..4
        if lvl >= 3:
          with tc.tile_pool(name="corn", bufs=1) as pc:
            for qi, (GEO, CX, CY, uvoff) in enumerate(
                    ((GEOP, AX4, AY4, 0), (GEOT, BX4, BY4, 4))):
                cx, cy = sl(GEO, 0), sl(GEO, 1)
                l_, w_ = sl(GEO, 2), sl(GEO, 3)
                yaw = sl(GEO, 4)
                ux = sl(UVT, uvoff + 0); vx = sl(UVT, uvoff + 1)
                uy = sl(UVT, uvoff + 2); vy = sl(UVT, uvoff + 3)
                co = pc.tile([P, S], BF, name=f"co{qi}", tag="co")
                si = pc.tile([P, S], BF, name=f"si{qi}", tag="si")
                A.activation(co, yaw, Act.Sin, bias=PI2)
                A.activation(si, yaw, Act.Sin)
                V.tensor_tensor(ux, l_, co, Alu.mult)
                V.tensor_tensor(vx, w_, si, Alu.mult)
                V.tensor_tensor(uy, l_, si, Alu.mult)
                V.tensor_tensor(vy, w_, co, Alu.mult)
                As2 = pc.tile([P, S], BF, name=f"As{qi}", tag="As")
                Ad2 = pc.tile([P, S], BF, name=f"Ad{qi}", tag="Ad")
                Ps2 = pc.tile([P, S], BF, name=f"Ps{qi}", tag="Ps")
                Pd2 = pc.tile([P, S], BF, name=f"Pd{qi}", tag="Pd")
                V.tensor_tensor(As2, ux, vx, Alu.add)
                V.tensor_tensor(Ad2, ux, vx, Alu.subtract)
                V.tensor_tensor(Ps2, uy, vy, Alu.add)
                V.tensor_tensor(Pd2, uy, vy, Alu.subtract)
                V.scalar_tensor_tensor(sl(CX, 0), As2, 0.5, cx, Alu.mult, Alu.add)
                V.scalar_tensor_tensor(sl(CX, 1), Ad2, -0.5, cx, Alu.mult, Alu.add)
                V.scalar_tensor_tensor(sl(CX, 2), As2, -0.5, cx, Alu.mult, Alu.add)
                V.scalar_tensor_tensor(sl(CX, 3), Ad2, 0.5, cx, Alu.mult, Alu.add)
                V.scalar_tensor_tensor(sl(CY, 0), Pd2, 0.5, cy, Alu.mult, Alu.add)
                V.scalar_tensor_tensor(sl(CY, 1), Ps2, -0.5, cy, Alu.mult, Alu.add)
                V.scalar_tensor_tensor(sl(CY, 2), Pd2, -0.5, cy, Alu.mult, Alu.add)
                V.scalar_tensor_tensor(sl(CY, 3), Ps2, 0.5, cy, Alu.mult, Alu.add)
            dump("AX4", AX4); dump("AY4", AY4)
            dump("BX4", BX4); dump("BY4", BY4)

            # crosses CA_k = cross(S_k, S_{k+1}) for both quads
            for (CX, CY, CR) in ((AX4, AY4, CA), (BX4, BY4, CB)):
                T1 = pc.tile([P, 4 * S], BF, name="crT1", tag="crT1")
                T2 = pc.tile([P, 4 * S], BF, name="crT2", tag="crT2")
                V.tensor_tensor(T1[:, 0:3 * S], CX[:, 0:3 * S], CY[:, S:4 * S], Alu.mult)
                V.tensor_tensor(sl(T1, 3), sl(CX, 3), sl(CY, 0), Alu.mult)
                V.tensor_tensor(T2[:, 0:3 * S], CY[:, 0:3 * S], CX[:, S:4 * S], Alu.mult)
                V.tensor_tensor(sl(T2, 3), sl(CY, 3), sl(CX, 0), Alu.mult)
                V.tensor_tensor(CR, T1, T2, Alu.subtract)
            dump("CA", CA); dump("CB", CB)

        # ============ clip passes ============
        # pass 0: segments A (corners AX4/AY4, crosses CA), constraints B
        # pass 1: segments B, constraints A
        if lvl >= 4:
          for pi, (SX, SY, CS, CQ, uvq) in enumerate(
                  (((AX4, AY4, CA, CB, 4), (BX4, BY4, CB, CA, 0)))[:(2 if lvl >= 5 else 1)]):
            with tc.tile_pool(name=f"clip{pi}", bufs=1) as pcl:
                uxq = sl(UVT, uvq + 0); vxq = sl(UVT, uvq + 1)
                uyq = sl(UVT, uvq + 2); vyq = sl(UVT, uvq + 3)
                G5 = []
                # --- G for both pairs ---
                for pair in range(2):
                    g5 = pcl.tile([P, 5 * S], BF, name=f"G5_{pi}_{pair}")
                    Gt = g5[:, 0:4 * S]
                    T1 = pcl.tile([P, 4 * S], BF, name=f"gT1_{pi}_{pair}", tag="gT1")
                    T2 = pcl.tile([P, 4 * S], BF, name=f"gT2_{pi}_{pair}", tag="gT2")
                    if pair == 0:
                        # E0 = (-ux, -uy): G = uy*SX - ux*SY
                        V.tensor_tensor(T1, bc(UVT, uvq + 2, 4), SX, Alu.mult)
                        V.tensor_tensor(T2, bc(UVT, uvq + 0, 4), SY, Alu.mult)
                        V.tensor_tensor(Gt, T1, T2, Alu.subtract)
                    else:
                        # E1 = (-vx, +vy): G = -vx*SY - vy*SX
                        V.tensor_tensor(T1, bc(UVT, uvq + 1, 4), SY, Alu.mult)
                        V.tensor_tensor(T2, bc(UVT, uvq + 3, 4), SX, Alu.mult)
                        V.scalar_tensor_tensor(Gt, T1, -1.0, T2, Alu.mult, Alu.subtract)
                    A.copy(g5[:, 4 * S:5 * S], g5[:, 0:S])
                    G5.append(g5)
                # --- recips for both pairs: |1/C1| = exp(-ln(|C1|+1e-30)) on
                # the Act engine, sign reapplied with one DVE mult ---
                C1s, RAs, SGs = [], [], []
                for pair in range(2):
                    C1 = pcl.tile([P, 4 * S], BF, name=f"C1_{pi}_{pair}", tag=f"C1_{pair}")
                    V.tensor_tensor(C1, G5[pair][:, S:5 * S], G5[pair][:, 0:4 * S],
                                    Alu.subtract)
                    C1s.append(C1)
                for pair in range(2):
                    AB1 = pcl.tile([P, 4 * S], BF, name=f"AB1_{pi}_{pair}", tag=f"AB1_{pair}")
                    A.activation(AB1, C1s[pair], Act.Abs)
                    SG = pcl.tile([P, 4 * S], BF, name=f"SG_{pi}_{pair}", tag=f"SG_{pair}")
                    A.activation(SG, C1s[pair], Act.Sign)
                    RAs.append(AB1); SGs.append(SG)
                for pair in range(2):
                    A.activation(RAs[pair], RAs[pair], Act.Ln, bias=1e-30)
                for pair in range(2):
                    A.activation(RAs[pair], RAs[pair], Act.Exp, scale=-1.0)
                RECs = []
                for pair in range(2):
                    REC = pcl.tile([P, 4 * S], BF, name=f"REC_{pi}_{pair}", tag=f"C1_{pair}")
                    V.tensor_tensor(REC, RAs[pair], SGs[pair], Alu.mult)
                    RECs.append(REC)
                # --- U, slab intervals ---
                LOHI = []
                for pair in range(2):
                    Gt = G5[pair][:, 0:4 * S]
                    REC = RECs[pair]
                    j0, j2 = (0, 2) if pair == 0 else (1, 3)
                    T0g = pcl.tile([P, 4 * S], BF, name=f"T0g_{pi}_{pair}", tag="gT1")
                    U0 = pcl.tile([P, 4 * S], BF, name=f"U0_{pi}_{pair}", tag=f"U0_{pair}")
                    T2g = pcl.tile([P, 4 * S], BF, name=f"T2g_{pi}_{pair}", tag="gT2")
                    U2 = pcl.tile([P, 4 * S], BF, name=f"U2_{pi}_{pair}", tag=f"U2_{pair}")
                    # U0 = -(CQ_j0 + G)*REC
                    V.tensor_tensor(T0g, Gt, bc(CQ, j0, 4), Alu.add)
                    V.scalar_tensor_tensor(U0, T0g, -1.0, REC, Alu.mult, Alu.mult)
                    # U2 = (CQ_j2 - G)*REC
                    V.tensor_tensor(T2g, bc(CQ, j2, 4), Gt, Alu.subtract)
                    V.tensor_tensor(U2, T2g, REC, Alu.mult)
                    lo = pcl.tile([P, 4 * S], BF, name=f"lo_{pi}_{pair}", tag=f"lo_{pair}")
                    hi = pcl.tile([P, 4 * S], BF, name=f"hi_{pi}_{pair}", tag=f"hi_{pair}")
                    V.tensor_tensor(lo, U0, U2, Alu.min)
                    V.tensor_tensor(hi, U0, U2, Alu.max)
                    LOHI.append((lo, hi))
                LO = LOHI[0][0]; HI = LOHI[0][1]
                V.tensor_tensor(LO, LO, LOHI[1][0], Alu.max)
                V.tensor_tensor(HI, HI, LOHI[1][1], Alu.min)
                # --- contribution ---
                T0 = pcl.tile([P, 4 * S], BF, name=f"T0_{pi}", tag="gT1")
                T1v = pcl.tile([P, 4 * S], BF, name=f"T1v_{pi}", tag="gT2")
                V.tensor_scalar(T0, LO, 0.0, 1.0, Alu.max, Alu.min)
                V.tensor_scalar(T1v, HI, 1.0, 0.0, Alu.min, Alu.max)
                DT = pcl.tile([P, 4 * S], BF, name=f"DT_{pi}", tag=f"U0_0")
                V.tensor_tensor(DT, T1v, T0, Alu.subtract)
                CONTR = pcl.tile([P, 4 * S], BF, name=f"CONTR_{pi}", tag=f"U2_0")
                V.scalar_tensor_tensor(CONTR, DT, 0.0, CS, Alu.max, Alu.mult)
                F2 = pcl.tile([P, 2 * S], BF, name=f"F2_{pi}", tag="lo_0")
                V.tensor_tensor(F2, CONTR[:, 0:2 * S], CONTR[:, 2 * S:4 * S], Alu.add)
                if pi == 0:
                    V.tensor_tensor(SAB, sl(F2, 0), sl(F2, 1), Alu.add)
                else:
                    SP2 = pcl.tile([P, S], BF, name="SP2", tag="hi_0")
                    V.tensor_tensor(SP2, sl(F2, 0), sl(F2, 1), Alu.add)
                    V.tensor_tensor(SAB, SAB, SP2, Alu.add)
                dump(f"LO_{pi}", LO); dump(f"HI_{pi}", HI)
                dump(f"CONTR_{pi}", CONTR)
          dump("SAB", SAB)

        # ============ extents -> c2, d2 ============
        if lvl >= 6:
          with tc.tile_pool(name="d2c2", bufs=1) as pd:
            EXT = pd.tile([P, 4 * S], BF, name="EXTf", tag="EXTf")
            XMM = pd.tile([P, 4], BF, name="XMM")  # placeholder unused
            exts = []
            for (CT, op, nm) in ((CORX, Alu.max, "xmax"), (CORX, Alu.min, "xmin"),
                                 (CORY, Alu.max, "ymax"), (CORY, Alu.min, "ymin")):
                f1 = pd.tile([P, 4 * S], BF, name=f"f1_{nm}", tag="f1")
                V.tensor_tensor(f1, CT[:, 0:4 * S], CT[:, 4 * S:8 * S], op)
                V.tensor_tensor(f1[:, 0:2 * S], f1[:, 0:2 * S], f1[:, 2 * S:4 * S], op)
                ex = pd.tile([P, S], BF, name=f"ext_{nm}", tag=f"ext_{nm}")
                V.tensor_tensor(ex, sl(f1, 0), sl(f1, 1), op)
                exts.append(ex)
            BW = pd.tile([P, S], BF, name="BW")
            BH = pd.tile([P, S], BF, name="BH")
            V.tensor_tensor(BW, exts[0], exts[1], Alu.subtract)
            V.tensor_tensor(BH, exts[2], exts[3], Alu.subtract)
            SQW = pd.tile([P, S], BF, name="SQW", tag="f1")
            SQH = pd.tile([P, S], BF, name="SQH", tag="EXTf")
            A.activation(SQW, BW, Act.Square)
            A.activation(SQH, BH, Act.Square)
            C2 = pd.tile([P, S], F32, name="C2")
            V.scalar_tensor_tensor(C2, SQW, EPS, SQH, Alu.add, Alu.add)
            RC2f = pd.tile([P, S], F32, name="RC2f")
            V.reciprocal_approx_fast(RC2f, C2)
            RC2 = pd.tile([P, S], BF, name="RC2")
            A.copy(RC2, RC2f)
            DXt = pd.tile([P, S], BF, name="DXt")
            DYt = pd.tile([P, S], BF, name="DYt")
            V.tensor_tensor(DXt, sl(GEOP, 0), sl(GEOT, 0), Alu.subtract)
            V.tensor_tensor(DYt, sl(GEOP, 1), sl(GEOT, 1), Alu.subtract)
            SQX = pd.tile([P, S], BF, name="SQX", tag="f1")
            SQY = pd.tile([P, S], BF, name="SQY", tag="EXTf")
            A.activation(SQX, DXt, Act.Square)
            A.activation(SQY, DYt, Act.Square)
            D2 = pd.tile([P, S], BF, name="D2t")
            V.tensor_tensor(D2, SQX, SQY, Alu.add)
            V.tensor_tensor(D2C2, D2, RC2, Alu.mult)
            dump("D2C2", D2C2)

        # ============ v term ============
        if lvl >= 7:
          with tc.tile_pool(name="vterm", bufs=1) as pv:
            ATs = []
            for qi, GEO in enumerate((GEOP, GEOT)):
                l_, w_ = sl(GEO, 2), sl(GEO, 3)
                rlf = pv.tile([P, S], F32, name=f"rlf{qi}", tag="rlf")
                rwf = pv.tile([P, S], F32, name=f"rwf{qi}", tag="rwf")
                V.tensor_scalar(rlf, l_, EPS, None, Alu.add)
                V.reciprocal_approx_fast(rlf, rlf)
                V.tensor_scalar(rwf, w_, 1e-30, None, Alu.add)
                V.reciprocal_approx_fast(rwf, rwf)
                rl = pv.tile([P, S], BF, name=f"rl{qi}", tag="rl")
                rw = pv.tile([P, S], BF, name=f"rw{qi}", tag="rw")
                A.copy(rl, rlf)
                A.copy(rw, rwf)
                x1 = pv.tile([P, S], BF, name=f"x1{qi}", tag="x1")
                x2 = pv.tile([P, S], BF, name=f"x2{qi}", tag="x2")
                V.tensor_tensor(x1, w_, rl, Alu.mult)
                V.tensor_tensor(x2, l_, rw, Alu.mult)
                mn = pv.tile([P, S], BF, name=f"mn{qi}", tag="mn")
                V.tensor_tensor(mn, x1, x2, Alu.min)
                aa = pv.tile([P, S], BF, name=f"aa{qi}", tag="aa")
                A.activation(aa, mn, Act.Arctan)
                m8 = pv.tile([P, S], U8, name=f"m8{qi}", tag="m8")
                V.tensor_scalar(m8, x1, 1.0, None, Alu.is_gt)
                tt2 = pv.tile([P, S], BF, name=f"tt2{qi}", tag="tt2")
                V.tensor_scalar(tt2, aa, -1.0, PI2, Alu.mult, Alu.add)
                AT = pv.tile([P, S], BF, name=f"AT{qi}")
                V.tensor_copy(AT, aa)
                V.copy_predicated(AT, m8, tt2)
                ATs.append(AT)
            DV = pv.tile([P, S], BF, name="DV", tag="rl")
            V.tensor_tensor(DV, ATs[0], ATs[1], Alu.subtract)
            A.activation(Vv, DV, Act.Square, scale=float(2.0 / np.pi))
            dump("Vv", Vv)

        # ============ iou + bev assembly ============
        if lvl >= 8:
          with tc.tile_pool(name="asm", bufs=1) as pa:
            INTER = pa.tile([P, S], BF, name="INTER")
            A.activation(INTER, SAB, Act.Abs, scale=0.5)
            ARA = pa.tile([P, S], BF, name="ARA")
            ARB = pa.tile([P, S], BF, name="ARB")
            V.tensor_tensor(ARA, sl(GEOP, 2), sl(GEOP, 3), Alu.mult)
            V.tensor_tensor(ARB, sl(GEOT, 2), sl(GEOT, 3), Alu.mult)
            UN = pa.tile([P, S], BF, name="UN")
            V.tensor_tensor(UN, ARA, ARB, Alu.add)
            V.tensor_tensor(UN, UN, INTER, Alu.subtract)
            V.tensor_scalar(UN, UN, EPS, None, Alu.max)
            UNf = pa.tile([P, S], F32, name="UNf")
            V.tensor_copy(UNf, UN)
            URCf = pa.tile([P, S], F32, name="URCf")
            V.reciprocal_approx_fast(URCf, UNf)
            URC = pa.tile([P, S], BF, name="URC")
            A.copy(URC, URCf)
            IOU = pa.tile([P, S], BF, name="IOU")
            V.tensor_tensor(IOU, INTER, URC, Alu.mult)
            OMI = pa.tile([P, S], BF, name="OMI")
            A.activation(OMI, IOU, Act.Relu, scale=-1.0, bias=1.0)
            DEN = pa.tile([P, S], F32, name="DEN")
            V.scalar_tensor_tensor(DEN, OMI, EPS, Vv, Alu.add, Alu.add)
            DRCf = pa.tile([P, S], F32, name="DRCf")
            V.reciprocal_approx_fast(DRCf, DEN)
            DRC = pa.tile([P, S], BF, name="DRC")
            A.copy(DRC, DRCf)
            ALC = pa.tile([P, S], BF, name="ALC")
            V.tensor_tensor(ALC, Vv, DRC, Alu.mult)
            V.tensor_tensor(ALC, ALC, Vv, Alu.mult)
            LB = pa.tile([P, S], BF, name="LB")
            V.tensor_tensor(LB, OMI, D2C2, Alu.add)
            V.tensor_tensor(LB, LB, ALC, Alu.add)
            V.tensor_tensor(LB, LB, W, Alu.mult)
            V.tensor_reduce(ACCS[:, 1:2], LB, AX_X, Alu.add)
            dump("IOU", IOU)
            dump("LBW", LB)

        # ============ focal loss ============
        if lvl >= 9:
          with tc.tile_pool(name="focal", bufs=1) as pf:
            CLS = pf.tile([P, 10 * S], BF, name="CLS")
            nc.sync.dma_start(out=CLS, in_=d_cls[:, :])
            E = pf.tile([P, 10 * S], BF, name="E")
            A.activation(E, CLS, Act.Exp)
            F1 = pf.tile([P, 5 * S], BF, name="F1")
            V.tensor_tensor(F1, E[:, 0:5 * S], E[:, 5 * S:10 * S], Alu.add)
            V.tensor_tensor(F1[:, 0:2 * S], F1[:, 0:2 * S], F1[:, 2 * S:4 * S], Alu.add)
            Ssum = pf.tile([P, S], BF, name="Ssum")
            V.tensor_tensor(Ssum, sl(F1, 0), sl(F1, 1), Alu.add)
            V.tensor_tensor(Ssum, Ssum, sl(F1, 4), Alu.add)
            ET = pf.tile([P, S], BF, name="ET")
            V.tensor_copy(ET, sl(E, 0))
            for c in range(1, 10):
                MC = pf.tile([P, S], U8, name=f"MC{c}", tag="MC")
                V.tensor_scalar(MC, CTF, float(c), None, Alu.is_equal)
                V.copy_predicated(ET, MC, sl(E, c))
            Ssf = pf.tile([P, S], F32, name="Ssf")
            V.tensor_copy(Ssf, Ssum)
            RSf = pf.tile([P, S], F32, name="RSf")
            V.reciprocal_approx_fast(RSf, Ssf)
            RS = pf.tile([P, S], BF, name="RS")
            A.copy(RS, RSf)
            PT = pf.tile([P, S], BF, name="PT")
            V.tensor_tensor(PT, ET, RS, Alu.mult)
            V.tensor_scalar(PT, PT, EPS, 1.0 - EPS, Alu.max, Alu.min)
            LG = pf.tile([P, S], BF, name="LG")
            A.activation(LG, PT, Act.Ln)
            OMP = pf.tile([P, S], BF, name="OMP")
            V.tensor_scalar(OMP, PT, -1.0, 1.0, Alu.mult, Alu.add)
            MPOS = pf.tile([P, S], U8, name="MPOS")
            V.tensor_scalar(MPOS, CTF, 0.0, None, Alu.is_gt)
            ALPH = pf.tile([P, S], BF, name="ALPH")
            QTR = pf.tile([P, S], BF, name="QTR")
            G.memset(ALPH, 0.75)
            G.memset(QTR, 0.25)
            V.copy_predicated(ALPH, MPOS, QTR)
            FL = pf.tile([P, S], BF, name="FL")
            V.tensor_tensor(FL, OMP, OMP, Alu.mult)
            V.tensor_tensor(FL, FL, LG, Alu.mult)
            V.scalar_tensor_tensor(FL, FL, -1.0, ALPH, Alu.mult, Alu.mult)
            V.tensor_reduce(ACCS[:, 0:1], FL, AX_X, Alu.add)
            dump("PT", PT)
            dump("FL", FL)

        A.memzero(ACCS[:, 7:8])
        nc.sync.dma_start(out=d_out[:, :], in_=ACCS)

    nc.compile()
    nc._dbg_outs = dbg_outs
    return nc


def _get_nc():
    if "nc" not in _CACHE:
        _ensure_ntff_hook()
        _CACHE["nc"] = _build()
    return _CACHE["nc"]


def _prep_core(cls_b, regp_b, regt_b, ioup_b, iout_b, ct_b, w_b):
    """Build one core's input map (bf16, [P, C*S] layouts) from f32 [C,H,W]."""
    import ml_dtypes
    BF = ml_dtypes.bfloat16

    def chans(x, idxs):
        # x [C,H,W] -> [P, len(idxs)*S] slot-interleaved
        sel = x[idxs].reshape(len(idxs), P, S)
        return np.ascontiguousarray(sel.transpose(1, 0, 2).reshape(P, len(idxs) * S)).astype(BF)

    return {
        "cls": chans(cls_b, list(range(10))),
        "geop": chans(regp_b, [0, 1, 3, 4, 6]),
        "geot": chans(regt_b, [0, 1, 3, 4, 6]),
        "zbp": chans(regp_b, [2, 5, 7, 8]),
        "zbt": chans(regt_b, [2, 5, 7, 8]),
        "ioup": ioup_b.reshape(P, S).astype(BF),
        "iout": iout_b.reshape(P, S).astype(BF),
        "ctf": ct_b.reshape(P, S).astype(np.float32).astype(BF),
        "w": w_b.reshape(P, S).astype(BF),
    }


def kernel(**inputs):
    from concourse.bass_utils import run_bass_kernel_spmd

    nc = _get_nc()
    cls_pred = np.asarray(inputs["cls_pred"], dtype=np.float32)
    reg_pred = np.asarray(inputs["reg_pred"], dtype=np.float32)
    iou_pred = np.asarray(inputs["iou_pred"], dtype=np.float32)
    cls_targets = np.asarray(inputs["cls_targets"], dtype=np.int32)
    reg_targets = np.asarray(inputs["reg_targets"], dtype=np.float32)
    reg_weights = np.asarray(inputs["reg_weights"], dtype=np.float32)
    iou_targets = np.asarray(inputs["iou_targets"], dtype=np.float32)

    B = cls_pred.shape[0]
    in_maps = []
    for b in range(B):
        in_maps.append(_prep_core(cls_pred[b], reg_pred[b], reg_targets[b],
                                  iou_pred[b], iou_targets[b],
                                  cls_targets[b], reg_weights[b]))
    res = run_bass_kernel_spmd(nc, in_maps, core_ids=list(range(8)))
    _CACHE["last_result"] = res
    sums = np.zeros(8, np.float64)
    for r in res.results:
        sums += r["out"].astype(np.float64).sum(axis=0)
    num_pos = max(sums[6], 1.0)
    out = np.array([sums[0], sums[1], sums[2], sums[3], sums[4], sums[5]],
                   np.float64) / num_pos
    return out.astype(np.float32)


# revision 7
# speedup vs baseline: 2.3321x; 1.2093x over previous
"""Trainium2 Bass kernel for nn_DetectionBEVLoss (bf16 pipeline, v2).

Takes FULL inputs (B=8,...), shards batch across 8 NeuronCores (one batch
element per core), computes per-core partial sums of the 6 loss terms plus the
positive count on-device, and finishes the tiny reduction on host.

Key optimizations over the f32 baseline:
 - host casts inputs to bf16 and pre-arranges [P, C*S] layouts (half the DMA,
   no on-device transposes/casts; bf16 doubles DVE tensor_tensor throughput)
 - rotated rects are parallelograms: opposite edges are +/-E, so each clip
   pass needs only 2 shared cross-product tensors G (not 4), and each edge
   pair forms a slab whose inside-interval is [min(U0,U2), max(U0,U2)]
 - Green's theorem: the boundary contribution of a clipped segment is
   (t1-t0)*cross(S_k, S_{k+1}); no intersection points are materialized
 - 1/C1 = C1*exp(-ln(C1^2+1e-30)) evaluated on the idle Act engine (sign
   comes out automatically, exact-0 C1 yields 0 -> no NaNs)
 - both passes' G/C1 issued before the U stages and the sl1/BCE vector work
   slotted in between, hiding the Act-chain latency
 - scalar_tensor_tensor avoided in hot @4S ops (STT doesn't get bf16 2x
   packing on the DVE; plain TT does) via pre-negated cross tiles
 - alpha_c denominator computed as relu(1-iou)+v to survive bf16 rounding
"""
import numpy as np

P = 128
S = 512          # free-dim elements per channel slot (65536 px per core)
NPX = P * S
EPS = 1e-7

_CACHE = {}


def _ensure_ntff_hook():
    import sys, types
    if "antenv.axon_hooks" in sys.modules:
        return
    try:
        import trn_agent_boot.trn_boot as tb
        hook = tb._ntff_profile_via_ctypes('/opt/axon/libaxon_pjrt.so')
        mod = types.ModuleType("antenv.axon_hooks")
        mod.get_axon_ntff_profile_hook = lambda: hook
        sys.modules["antenv.axon_hooks"] = mod
    except Exception:
        pass


def _build(debug=False, lvl=99):
    import concourse.bacc as bacc
    import concourse.tile as tile
    import concourse.mybir as mybir
    import concourse.bass as bass

    F32 = mybir.dt.float32
    BF = mybir.dt.bfloat16
    U8 = mybir.dt.uint8
    Alu = mybir.AluOpType
    Act = mybir.ActivationFunctionType
    AX_X = mybir.AxisListType.X
    PI2 = float(np.pi / 2)

    nc = bacc.Bacc("TRN2", target_bir_lowering=False, debug=False, num_devices=8)

    for v in [PI2, 1e-30, 1.0]:
        t = nc.alloc_sbuf_tensor(f"const-f32-{v}", [P, 1], F32)
        nc.gpsimd.memset(t.ap(), v)
        nc.const_aps.aps[(F32, v)] = t.ap()
    nc.all_engine_barrier()

    d_cls = nc.dram_tensor("cls", [P, 10 * S], BF, kind="ExternalInput")
    d_geop = nc.dram_tensor("geop", [P, 5 * S], BF, kind="ExternalInput")
    d_geot = nc.dram_tensor("geot", [P, 5 * S], BF, kind="ExternalInput")
    d_zbp = nc.dram_tensor("zbp", [P, 4 * S], BF, kind="ExternalInput")
    d_zbt = nc.dram_tensor("zbt", [P, 4 * S], BF, kind="ExternalInput")
    d_ioup = nc.dram_tensor("ioup", [P, S], BF, kind="ExternalInput")
    d_iout = nc.dram_tensor("iout", [P, S], BF, kind="ExternalInput")
    d_ctf = nc.dram_tensor("ctf", [P, S], BF, kind="ExternalInput")
    d_w = nc.dram_tensor("w", [P, S], BF, kind="ExternalInput")
    d_out = nc.dram_tensor("out", [P, 8], F32, kind="ExternalOutput")

    V = nc.vector
    A = nc.scalar
    G = nc.gpsimd

    dbg_outs = []

    def dump(name, t):
        if not debug:
            return
        shp = [t.shape[0], int(np.prod(t.shape[1:]))]
        d = nc.dram_tensor(f"dbg_{name}", shp, t.dtype, kind="ExternalOutput")
        nc.sync.dma_start(out=d[:, :], in_=t)
        dbg_outs.append(name)

    def bc(t, i, k):
        # broadcast S-slice i of tile t over k slots
        b_ = t[:, i * S:(i + 1) * S]
        return bass.AP(tensor=b_.tensor, offset=b_.offset,
                       ap=[b_.ap[0], [0, k], [1, S]])

    def sl(t, i, k=1):
        return t[:, i * S:(i + k) * S]

    with tile.TileContext(nc) as tc:
      with tc.tile_pool(name="persist", bufs=1) as pp:
        ACCS = pp.tile([P, 8], F32, name="ACCS")
        ZACC = pp.tile([P, 4], F32, name="ZACC")
        W = pp.tile([P, S], BF, name="W")
        CTF = pp.tile([P, S], BF, name="CTF")
        IOUP = pp.tile([P, S], BF, name="IOUP")
        IOUT = pp.tile([P, S], BF, name="IOUT")
        GEOP = pp.tile([P, 5 * S], BF, name="GEOP")
        GEOT = pp.tile([P, 5 * S], BF, name="GEOT")
        CORX = pp.tile([P, 8 * S], BF, name="CORX")   # [AX(4S) | BX(4S)]
        CORY = pp.tile([P, 8 * S], BF, name="CORY")
        CA = pp.tile([P, 4 * S], BF, name="CA")
        CB = pp.tile([P, 4 * S], BF, name="CB")
        CAn = pp.tile([P, 4 * S], BF, name="CAn")
        CBn = pp.tile([P, 4 * S], BF, name="CBn")
        UVT = pp.tile([P, 8 * S], BF, name="UVT")     # uxA vxA uyA vyA uxB vxB uyB vyB
        SAB = pp.tile([P, S], BF, name="SAB")
        Vv = pp.tile([P, S], BF, name="Vv")
        D2C2 = pp.tile([P, S], BF, name="D2C2")

        AX4 = CORX[:, 0:4 * S]; BX4 = CORX[:, 4 * S:8 * S]
        AY4 = CORY[:, 0:4 * S]; BY4 = CORY[:, 4 * S:8 * S]

        nc.sync.dma_start(out=W, in_=d_w[:, :])
        nc.sync.dma_start(out=GEOP, in_=d_geop[:, :])
        nc.sync.dma_start(out=GEOT, in_=d_geot[:, :])
        nc.sync.dma_start(out=CTF, in_=d_ctf[:, :])
        nc.sync.dma_start(out=IOUP, in_=d_ioup[:, :])
        nc.sync.dma_start(out=IOUT, in_=d_iout[:, :])

        # ============ corners + uv smalls ============
        # GEO layout: [cx, cy, l, w, yaw] at slots 0..4
        with tc.tile_pool(name="corn", bufs=1) as pc:
            for qi, (GEO, CX, CY, uvoff) in enumerate(
                    ((GEOP, AX4, AY4, 0), (GEOT, BX4, BY4, 4))):
                cx, cy = sl(GEO, 0), sl(GEO, 1)
                l_, w_ = sl(GEO, 2), sl(GEO, 3)
                yaw = sl(GEO, 4)
                ux = sl(UVT, uvoff + 0); vx = sl(UVT, uvoff + 1)
                uy = sl(UVT, uvoff + 2); vy = sl(UVT, uvoff + 3)
                co = pc.tile([P, S], BF, name=f"co{qi}", tag="co")
                si = pc.tile([P, S], BF, name=f"si{qi}", tag="si")
                A.activation(co, yaw, Act.Sin, bias=PI2)
                A.activation(si, yaw, Act.Sin)
                V.tensor_tensor(ux, l_, co, Alu.mult)
                V.tensor_tensor(vx, w_, si, Alu.mult)
                V.tensor_tensor(uy, l_, si, Alu.mult)
                V.tensor_tensor(vy, w_, co, Alu.mult)
                As2 = pc.tile([P, S], BF, name=f"As{qi}", tag="As")
                Ad2 = pc.tile([P, S], BF, name=f"Ad{qi}", tag="Ad")
                Ps2 = pc.tile([P, S], BF, name=f"Ps{qi}", tag="Ps")
                Pd2 = pc.tile([P, S], BF, name=f"Pd{qi}", tag="Pd")
                V.tensor_tensor(As2, ux, vx, Alu.add)
                V.tensor_tensor(Ad2, ux, vx, Alu.subtract)
                V.tensor_tensor(Ps2, uy, vy, Alu.add)
                V.tensor_tensor(Pd2, uy, vy, Alu.subtract)
                V.scalar_tensor_tensor(sl(CX, 0), As2, 0.5, cx, Alu.mult, Alu.add)
                V.scalar_tensor_tensor(sl(CX, 1), Ad2, -0.5, cx, Alu.mult, Alu.add)
                V.scalar_tensor_tensor(sl(CX, 2), As2, -0.5, cx, Alu.mult, Alu.add)
                V.scalar_tensor_tensor(sl(CX, 3), Ad2, 0.5, cx, Alu.mult, Alu.add)
                V.scalar_tensor_tensor(sl(CY, 0), Pd2, 0.5, cy, Alu.mult, Alu.add)
                V.scalar_tensor_tensor(sl(CY, 1), Ps2, -0.5, cy, Alu.mult, Alu.add)
                V.scalar_tensor_tensor(sl(CY, 2), Pd2, -0.5, cy, Alu.mult, Alu.add)
                V.scalar_tensor_tensor(sl(CY, 3), Ps2, 0.5, cy, Alu.mult, Alu.add)
            dump("AX4", AX4); dump("AY4", AY4)
            dump("BX4", BX4); dump("BY4", BY4)

            # crosses CA_k = cross(S_k, S_{k+1}) and negated copies
            for (CX, CY, CR, CRn) in ((AX4, AY4, CA, CAn), (BX4, BY4, CB, CBn)):
                T1 = pc.tile([P, 4 * S], BF, name="crT1", tag="crT1")
                T2 = pc.tile([P, 4 * S], BF, name="crT2", tag="crT2")
                V.tensor_tensor(T1[:, 0:3 * S], CX[:, 0:3 * S], CY[:, S:4 * S], Alu.mult)
                V.tensor_tensor(sl(T1, 3), sl(CX, 3), sl(CY, 0), Alu.mult)
                V.tensor_tensor(T2[:, 0:3 * S], CY[:, 0:3 * S], CX[:, S:4 * S], Alu.mult)
                V.tensor_tensor(sl(T2, 3), sl(CY, 3), sl(CX, 0), Alu.mult)
                V.tensor_tensor(CR, T1, T2, Alu.subtract)
                V.tensor_tensor(CRn, T2, T1, Alu.subtract)
            dump("CA", CA); dump("CB", CB)

        # ============ clip passes: G + C1 for both passes first ============
        # pass 0: segments A (corners AX4/AY4, crosses CA), constraints B
        # pass 1: segments B, constraints A
        # pair0 G is cross(E0,S) with E0=(-ux,-uy); pair1 stores NEGATED G
        # (plain add) with C1 subtraction reversed so REC keeps true sign.
        PASSES = ((AX4, AY4, CA, CB, CBn, 4), (BX4, BY4, CB, CA, CAn, 0))
        with tc.tile_pool(name="clip", bufs=1) as pcl:
            G5s = {}
            C1s = {}
            RS2s = {}
            for pi, (SX, SY, CS, CQ, CQn, uvq) in enumerate(PASSES):
                for pair in range(2):
                    g5 = pcl.tile([P, 5 * S], BF, name=f"G5_{pi}_{pair}")
                    Gt = g5[:, 0:4 * S]
                    T1 = pcl.tile([P, 4 * S], BF, name=f"gT1_{pi}_{pair}", tag="gT1")
                    T2 = pcl.tile([P, 4 * S], BF, name=f"gT2_{pi}_{pair}", tag="gT2")
                    if pair == 0:
                        # E0 = (-ux, -uy): G = uy*SX - ux*SY
                        V.tensor_tensor(T1, bc(UVT, uvq + 2, 4), SX, Alu.mult)
                        V.tensor_tensor(T2, bc(UVT, uvq + 0, 4), SY, Alu.mult)
                        V.tensor_tensor(Gt, T1, T2, Alu.subtract)
                    else:
                        # E1 = (-vx, +vy): G = -(vx*SY + vy*SX); store Gneg
                        V.tensor_tensor(T1, bc(UVT, uvq + 1, 4), SY, Alu.mult)
                        V.tensor_tensor(T2, bc(UVT, uvq + 3, 4), SX, Alu.mult)
                        V.tensor_tensor(Gt, T1, T2, Alu.add)
                    A.copy(g5[:, 4 * S:5 * S], g5[:, 0:S])
                    C1 = pcl.tile([P, 4 * S], BF, name=f"C1_{pi}_{pair}")
                    if pair == 0:
                        V.tensor_tensor(C1, g5[:, S:5 * S], g5[:, 0:4 * S], Alu.subtract)
                    else:
                        # G stored negated: C1_true = Gneg_k - Gneg_{k+1}
                        V.tensor_tensor(C1, g5[:, 0:4 * S], g5[:, S:5 * S], Alu.subtract)
                    G5s[(pi, pair)] = g5
                    C1s[(pi, pair)] = C1
            # Act chains: 1/C1 = C1 * exp(-ln(C1^2 + 1e-30))
            for pi in range(2):
                for pair in range(2):
                    RS2 = pcl.tile([P, 4 * S], BF, name=f"RS2_{pi}_{pair}")
                    A.activation(RS2, C1s[(pi, pair)], Act.Square)
                    RS2s[(pi, pair)] = RS2
            for pi in range(2):
                for pair in range(2):
                    A.activation(RS2s[(pi, pair)], RS2s[(pi, pair)], Act.Ln, bias=1e-30)
            for pi in range(2):
                for pair in range(2):
                    A.activation(RS2s[(pi, pair)], RS2s[(pi, pair)], Act.Exp, scale=-1.0)

            # ---- vector work to hide the Act chains: smooth-L1 + BCE ----
            V.tensor_reduce(ACCS[:, 6:7], W, AX_X, Alu.add)
            with tc.tile_pool(name="sl1", bufs=1) as ps:
                ZBP = ps.tile([P, 4 * S], BF, name="ZBP")
                ZBT = ps.tile([P, 4 * S], BF, name="ZBT")
                nc.sync.dma_start(out=ZBP, in_=d_zbp[:, :])
                nc.sync.dma_start(out=ZBT, in_=d_zbt[:, :])
                D = ps.tile([P, 4 * S], BF, name="D")
                AD = ps.tile([P, 4 * S], BF, name="AD")
                M = ps.tile([P, 4 * S], BF, name="M")
                MD = ps.tile([P, 4 * S], BF, name="MD")
                M2H = ps.tile([P, 4 * S], BF, name="M2H")
                SL1 = ps.tile([P, 4 * S], BF, name="SL1")
                V.tensor_tensor(D, ZBP, ZBT, Alu.subtract)
                A.activation(AD, D, Act.Abs)
                V.tensor_scalar(M, AD, 1.0, None, Alu.min)
                V.tensor_tensor(MD, M, AD, Alu.mult)
                A.activation(M2H, M, Act.Square, scale=float(np.sqrt(0.5)))
                V.tensor_tensor(SL1, MD, M2H, Alu.subtract)
                V.tensor_tensor(SL1, SL1, bc(W, 0, 4), Alu.mult)
                V.tensor_reduce(ZACC, SL1.rearrange("p (c f) -> p c f", c=4),
                                AX_X, Alu.add)
                V.tensor_copy(ACCS[:, 2:3], ZACC[:, 0:1])
                V.tensor_copy(ACCS[:, 3:4], ZACC[:, 1:2])
                V.tensor_tensor(ACCS[:, 4:5], ZACC[:, 2:3], ZACC[:, 3:4], Alu.add)
                dump("SL1", SL1)

            with tc.tile_pool(name="bce", bufs=1) as pb:
                AXb = pb.tile([P, S], BF, name="AXb")
                SP = pb.tile([P, S], BF, name="SP")
                RL = pb.tile([P, S], BF, name="RL")
                XY = pb.tile([P, S], BF, name="XY")
                A.activation(AXb, IOUP, Act.Abs)
                EB = pb.tile([P, S], BF, name="EB")
                A.activation(EB, AXb, Act.Exp, scale=-1.0)
                A.activation(SP, EB, Act.Ln, bias=1.0)
                A.activation(RL, IOUP, Act.Relu)
                V.tensor_tensor(XY, IOUP, IOUT, Alu.mult)
                V.tensor_tensor(RL, RL, XY, Alu.subtract)
                V.tensor_tensor(RL, RL, SP, Alu.add)
                V.tensor_tensor(RL, RL, W, Alu.mult)
                V.tensor_reduce(ACCS[:, 5:6], RL, AX_X, Alu.add)
                dump("BCE", RL)

            # ---- U, slab intervals, contributions ----
            for pi, (SX, SY, CS, CQ, CQn, uvq) in enumerate(PASSES):
                LOHI = []
                for pair in range(2):
                    Gt = G5s[(pi, pair)][:, 0:4 * S]
                    C1 = C1s[(pi, pair)]
                    RS2 = RS2s[(pi, pair)]
                    REC = pcl.tile([P, 4 * S], BF, name=f"REC_{pi}_{pair}", tag=f"REC_{pair}")
                    V.tensor_tensor(REC, C1, RS2, Alu.mult)
                    j0, j2 = (0, 2) if pair == 0 else (1, 3)
                    T0g = pcl.tile([P, 4 * S], BF, name=f"T0g_{pi}_{pair}", tag="gT1")
                    U0 = pcl.tile([P, 4 * S], BF, name=f"U0_{pi}_{pair}", tag=f"U0_{pair}")
                    T2g = pcl.tile([P, 4 * S], BF, name=f"T2g_{pi}_{pair}", tag="gT2")
                    U2 = pcl.tile([P, 4 * S], BF, name=f"U2_{pi}_{pair}", tag=f"U2_{pair}")
                    if pair == 0:
                        # U0 = (-CQ_j0 - G)*REC ; U2 = (CQ_j2 - G)*REC
                        V.tensor_tensor(T0g, bc(CQn, j0, 4), Gt, Alu.subtract)
                        V.tensor_tensor(T2g, bc(CQ, j2, 4), Gt, Alu.subtract)
                    else:
                        # G stored negated: U0 = (Gneg - CQ_j0)*REC
                        #                   U2 = (Gneg + CQ_j2)*REC
                        V.tensor_tensor(T0g, Gt, bc(CQ, j0, 4), Alu.subtract)
                        V.tensor_tensor(T2g, Gt, bc(CQ, j2, 4), Alu.add)
                    V.tensor_tensor(U0, T0g, REC, Alu.mult)
                    V.tensor_tensor(U2, T2g, REC, Alu.mult)
                    lo = pcl.tile([P, 4 * S], BF, name=f"lo_{pi}_{pair}", tag=f"lo_{pair}")
                    hi = pcl.tile([P, 4 * S], BF, name=f"hi_{pi}_{pair}", tag=f"hi_{pair}")
                    V.tensor_tensor(lo, U0, U2, Alu.min)
                    V.tensor_tensor(hi, U0, U2, Alu.max)
                    LOHI.append((lo, hi))
                LO = LOHI[0][0]; HI = LOHI[0][1]
                V.tensor_tensor(LO, LO, LOHI[1][0], Alu.max)
                V.tensor_tensor(HI, HI, LOHI[1][1], Alu.min)
                T0 = pcl.tile([P, 4 * S], BF, name=f"T0_{pi}", tag="gT1")
                T1v = pcl.tile([P, 4 * S], BF, name=f"T1v_{pi}", tag="gT2")
                V.tensor_scalar(T0, LO, 0.0, 1.0, Alu.max, Alu.min)
                V.tensor_scalar(T1v, HI, 1.0, 0.0, Alu.min, Alu.max)
                DT = pcl.tile([P, 4 * S], BF, name=f"DT_{pi}", tag="U0_0")
                V.tensor_tensor(DT, T1v, T0, Alu.subtract)
                V.tensor_scalar(DT, DT, 0.0, None, Alu.max)
                CONTR = pcl.tile([P, 4 * S], BF, name=f"CONTR_{pi}", tag="U2_0")
                V.tensor_tensor(CONTR, DT, CS, Alu.mult)
                F2 = pcl.tile([P, 2 * S], BF, name=f"F2_{pi}", tag="lo_0")
                V.tensor_tensor(F2, CONTR[:, 0:2 * S], CONTR[:, 2 * S:4 * S], Alu.add)
                if pi == 0:
                    V.tensor_tensor(SAB, sl(F2, 0), sl(F2, 1), Alu.add)
                else:
                    SP2 = pcl.tile([P, S], BF, name="SP2", tag="hi_0")
                    V.tensor_tensor(SP2, sl(F2, 0), sl(F2, 1), Alu.add)
                    V.tensor_tensor(SAB, SAB, SP2, Alu.add)
                dump(f"LO_{pi}", LO); dump(f"HI_{pi}", HI)
                dump(f"CONTR_{pi}", CONTR)
            dump("SAB", SAB)

        # ============ extents -> c2, d2 ============
        with tc.tile_pool(name="d2c2", bufs=1) as pd:
            exts = []
            for (CT, op, nm) in ((CORX, Alu.max, "xmax"), (CORX, Alu.min, "xmin"),
                                 (CORY, Alu.max, "ymax"), (CORY, Alu.min, "ymin")):
                f1 = pd.tile([P, 4 * S], BF, name=f"f1_{nm}", tag="f1")
                V.tensor_tensor(f1, CT[:, 0:4 * S], CT[:, 4 * S:8 * S], op)
                V.tensor_tensor(f1[:, 0:2 * S], f1[:, 0:2 * S], f1[:, 2 * S:4 * S], op)
                ex = pd.tile([P, S], BF, name=f"ext_{nm}", tag=f"ext_{nm}")
                V.tensor_tensor(ex, sl(f1, 0), sl(f1, 1), op)
                exts.append(ex)
            BW = pd.tile([P, S], BF, name="BW")
            BH = pd.tile([P, S], BF, name="BH")
            V.tensor_tensor(BW, exts[0], exts[1], Alu.subtract)
            V.tensor_tensor(BH, exts[2], exts[3], Alu.subtract)
            SQW = pd.tile([P, S], BF, name="SQW", tag="f1")
            SQH = pd.tile([P, S], BF, name="SQH", tag="sqh")
            A.activation(SQW, BW, Act.Square)
            A.activation(SQH, BH, Act.Square)
            C2 = pd.tile([P, S], F32, name="C2")
            V.scalar_tensor_tensor(C2, SQW, EPS, SQH, Alu.add, Alu.add)
            RC2f = pd.tile([P, S], F32, name="RC2f")
            V.reciprocal_approx_fast(RC2f, C2)
            RC2 = pd.tile([P, S], BF, name="RC2")
            A.copy(RC2, RC2f)
            DXt = pd.tile([P, S], BF, name="DXt")
            DYt = pd.tile([P, S], BF, name="DYt")
            V.tensor_tensor(DXt, sl(GEOP, 0), sl(GEOT, 0), Alu.subtract)
            V.tensor_tensor(DYt, sl(GEOP, 1), sl(GEOT, 1), Alu.subtract)
            SQX = pd.tile([P, S], BF, name="SQX", tag="f1")
            SQY = pd.tile([P, S], BF, name="SQY", tag="sqh")
            A.activation(SQX, DXt, Act.Square)
            A.activation(SQY, DYt, Act.Square)
            D2 = pd.tile([P, S], BF, name="D2t")
            V.tensor_tensor(D2, SQX, SQY, Alu.add)
            V.tensor_tensor(D2C2, D2, RC2, Alu.mult)
            dump("D2C2", D2C2)

        # ============ v term ============
        with tc.tile_pool(name="vterm", bufs=1) as pv:
            ATs = []
            for qi, GEO in enumerate((GEOP, GEOT)):
                l_, w_ = sl(GEO, 2), sl(GEO, 3)
                rlf = pv.tile([P, S], F32, name=f"rlf{qi}", tag="rlf")
                rwf = pv.tile([P, S], F32, name=f"rwf{qi}", tag="rwf")
                V.tensor_scalar(rlf, l_, EPS, None, Alu.add)
                V.reciprocal_approx_fast(rlf, rlf)
                V.tensor_scalar(rwf, w_, 1e-30, None, Alu.add)
                V.reciprocal_approx_fast(rwf, rwf)
                rl = pv.tile([P, S], BF, name=f"rl{qi}", tag="rl")
                rw = pv.tile([P, S], BF, name=f"rw{qi}", tag="rw")
                A.copy(rl, rlf)
                A.copy(rw, rwf)
                x1 = pv.tile([P, S], BF, name=f"x1{qi}", tag="x1")
                x2 = pv.tile([P, S], BF, name=f"x2{qi}", tag="x2")
                V.tensor_tensor(x1, w_, rl, Alu.mult)
                V.tensor_tensor(x2, l_, rw, Alu.mult)
                mn = pv.tile([P, S], BF, name=f"mn{qi}", tag="mn")
                V.tensor_tensor(mn, x1, x2, Alu.min)
                aa = pv.tile([P, S], BF, name=f"aa{qi}", tag="aa")
                A.activation(aa, mn, Act.Arctan)
                m8 = pv.tile([P, S], U8, name=f"m8{qi}", tag="m8")
                V.tensor_scalar(m8, x1, 1.0, None, Alu.is_gt)
                tt2 = pv.tile([P, S], BF, name=f"tt2{qi}", tag="tt2")
                V.tensor_scalar(tt2, aa, -1.0, PI2, Alu.mult, Alu.add)
                AT = pv.tile([P, S], BF, name=f"AT{qi}")
                A.copy(AT, aa)
                V.copy_predicated(AT, m8, tt2)
                ATs.append(AT)
            DV = pv.tile([P, S], BF, name="DV", tag="rl")
            V.tensor_tensor(DV, ATs[0], ATs[1], Alu.subtract)
            A.activation(Vv, DV, Act.Square, scale=float(2.0 / np.pi))
            dump("Vv", Vv)

        # ============ iou + bev assembly ============
        with tc.tile_pool(name="asm", bufs=1) as pa:
            INTER = pa.tile([P, S], BF, name="INTER")
            A.activation(INTER, SAB, Act.Abs, scale=0.5)
            ARA = pa.tile([P, S], BF, name="ARA")
            ARB = pa.tile([P, S], BF, name="ARB")
            V.tensor_tensor(ARA, sl(GEOP, 2), sl(GEOP, 3), Alu.mult)
            V.tensor_tensor(ARB, sl(GEOT, 2), sl(GEOT, 3), Alu.mult)
            UN = pa.tile([P, S], BF, name="UN")
            V.tensor_tensor(UN, ARA, ARB, Alu.add)
            V.tensor_tensor(UN, UN, INTER, Alu.subtract)
            UNf = pa.tile([P, S], F32, name="UNf")
            V.tensor_scalar(UNf, UN, EPS, None, Alu.max)
            URCf = pa.tile([P, S], F32, name="URCf")
            V.reciprocal_approx_fast(URCf, UNf)
            URC = pa.tile([P, S], BF, name="URC")
            A.copy(URC, URCf)
            IOU = pa.tile([P, S], BF, name="IOU")
            V.tensor_tensor(IOU, INTER, URC, Alu.mult)
            OMI = pa.tile([P, S], BF, name="OMI")
            A.activation(OMI, IOU, Act.Relu, scale=-1.0, bias=1.0)
            DEN = pa.tile([P, S], F32, name="DEN")
            V.scalar_tensor_tensor(DEN, OMI, EPS, Vv, Alu.add, Alu.add)
            DRCf = pa.tile([P, S], F32, name="DRCf")
            V.reciprocal_approx_fast(DRCf, DEN)
            DRC = pa.tile([P, S], BF, name="DRC")
            A.copy(DRC, DRCf)
            ALC = pa.tile([P, S], BF, name="ALC")
            V.tensor_tensor(ALC, Vv, DRC, Alu.mult)
            V.tensor_tensor(ALC, ALC, Vv, Alu.mult)
            LB = pa.tile([P, S], BF, name="LB")
            V.tensor_tensor(LB, OMI, D2C2, Alu.add)
            V.tensor_tensor(LB, LB, ALC, Alu.add)
            V.tensor_tensor(LB, LB, W, Alu.mult)
            V.tensor_reduce(ACCS[:, 1:2], LB, AX_X, Alu.add)
            dump("IOU", IOU)
            dump("LBW", LB)

        # ============ focal loss ============
        with tc.tile_pool(name="focal", bufs=1) as pf:
            CLS = pf.tile([P, 10 * S], BF, name="CLS")
            nc.sync.dma_start(out=CLS, in_=d_cls[:, :])
            E = pf.tile([P, 10 * S], BF, name="E")
            A.activation(E, CLS, Act.Exp)
            F1 = pf.tile([P, 5 * S], BF, name="F1")
            V.tensor_tensor(F1, E[:, 0:5 * S], E[:, 5 * S:10 * S], Alu.add)
            V.tensor_tensor(F1[:, 0:2 * S], F1[:, 0:2 * S], F1[:, 2 * S:4 * S], Alu.add)
            Ssum = pf.tile([P, S], BF, name="Ssum")
            V.tensor_tensor(Ssum, sl(F1, 0), sl(F1, 1), Alu.add)
            V.tensor_tensor(Ssum, Ssum, sl(F1, 4), Alu.add)
            ET = pf.tile([P, S], BF, name="ET")
            A.copy(ET, sl(E, 0))
            for c in range(1, 10):
                MC = pf.tile([P, S], U8, name=f"MC{c}", tag="MC")
                V.tensor_scalar(MC, CTF, float(c), None, Alu.is_equal)
                V.copy_predicated(ET, MC, sl(E, c))
            Ssf = pf.tile([P, S], F32, name="Ssf")
            V.tensor_copy(Ssf, Ssum)
            RSf = pf.tile([P, S], F32, name="RSf")
            V.reciprocal_approx_fast(RSf, Ssf)
            RS = pf.tile([P, S], BF, name="RS")
            A.copy(RS, RSf)
            PT = pf.tile([P, S], BF, name="PT")
            V.tensor_tensor(PT, ET, RS, Alu.mult)
            V.tensor_scalar(PT, PT, EPS, 1.0 - EPS, Alu.max, Alu.min)
            LG = pf.tile([P, S], BF, name="LG")
            A.activation(LG, PT, Act.Ln)
            OMP = pf.tile([P, S], BF, name="OMP")
            V.tensor_scalar(OMP, PT, -1.0, 1.0, Alu.mult, Alu.add)
            MPOS = pf.tile([P, S], U8, name="MPOS")
            V.tensor_scalar(MPOS, CTF, 0.0, None, Alu.is_gt)
            ALPH = pf.tile([P, S], BF, name="ALPH")
            QTR = pf.tile([P, S], BF, name="QTR")
            G.memset(ALPH, 0.75)
            G.memset(QTR, 0.25)
            V.copy_predicated(ALPH, MPOS, QTR)
            FL = pf.tile([P, S], BF, name="FL")
            V.tensor_tensor(FL, OMP, OMP, Alu.mult)
            V.tensor_tensor(FL, FL, LG, Alu.mult)
            V.scalar_tensor_tensor(FL, FL, -1.0, ALPH, Alu.mult, Alu.mult)
            V.tensor_reduce(ACCS[:, 0:1], FL, AX_X, Alu.add)
            dump("PT", PT)
            dump("FL", FL)

        A.memzero(ACCS[:, 7:8])
        nc.sync.dma_start(out=d_out[:, :], in_=ACCS)

    nc.compile()
    nc._dbg_outs = dbg_outs
    return nc


def _get_nc():
    if "nc" not in _CACHE:
        _ensure_ntff_hook()
        _CACHE["nc"] = _build()
    return _CACHE["nc"]


def _prep_core(cls_b, regp_b, regt_b, ioup_b, iout_b, ct_b, w_b):
    """Build one core's input map (bf16, [P, C*S] layouts) from f32 [C,H,W]."""
    import ml_dtypes
    BF = ml_dtypes.bfloat16

    def chans(x, idxs):
        # x [C,H,W] -> [P, len(idxs)*S] slot-interleaved
        sel = x[idxs].reshape(len(idxs), P, S)
        return np.ascontiguousarray(sel.transpose(1, 0, 2).reshape(P, len(idxs) * S)).astype(BF)

    return {
        "cls": chans(cls_b, list(range(10))),
        "geop": chans(regp_b, [0, 1, 3, 4, 6]),
        "geot": chans(regt_b, [0, 1, 3, 4, 6]),
        "zbp": chans(regp_b, [2, 5, 7, 8]),
        "zbt": chans(regt_b, [2, 5, 7, 8]),
        "ioup": ioup_b.reshape(P, S).astype(BF),
        "iout": iout_b.reshape(P, S).astype(BF),
        "ctf": ct_b.reshape(P, S).astype(np.float32).astype(BF),
        "w": w_b.reshape(P, S).astype(BF),
    }


def kernel(**inputs):
    from concourse.bass_utils import run_bass_kernel_spmd

    nc = _get_nc()
    cls_pred = np.asarray(inputs["cls_pred"], dtype=np.float32)
    reg_pred = np.asarray(inputs["reg_pred"], dtype=np.float32)
    iou_pred = np.asarray(inputs["iou_pred"], dtype=np.float32)
    cls_targets = np.asarray(inputs["cls_targets"], dtype=np.int32)
    reg_targets = np.asarray(inputs["reg_targets"], dtype=np.float32)
    reg_weights = np.asarray(inputs["reg_weights"], dtype=np.float32)
    iou_targets = np.asarray(inputs["iou_targets"], dtype=np.float32)

    B = cls_pred.shape[0]
    in_maps = []
    for b in range(B):
        in_maps.append(_prep_core(cls_pred[b], reg_pred[b], reg_targets[b],
                                  iou_pred[b], iou_targets[b],
                                  cls_targets[b], reg_weights[b]))
    res = run_bass_kernel_spmd(nc, in_maps, core_ids=list(range(8)))
    _CACHE["last_result"] = res
    sums = np.zeros(8, np.float64)
    for r in res.results:
        sums += r["out"].astype(np.float64).sum(axis=0)
    num_pos = max(sums[6], 1.0)
    out = np.array([sums[0], sums[1], sums[2], sums[3], sums[4], sums[5]],
                   np.float64) / num_pos
    return out.astype(np.float32)


# revision 9
# speedup vs baseline: 2.5468x; 1.0921x over previous
"""Trainium2 Bass kernel for nn_DetectionBEVLoss (bf16 pipeline, v3).

Takes FULL inputs (B=8,...), shards batch across 8 NeuronCores (one batch
element per core), computes per-core partial sums of the 6 loss terms plus the
positive count on-device, and finishes the tiny reduction on host.

Key optimizations over the f32 baseline:
 - host casts inputs to bf16 and pre-arranges [P, C*S] layouts (half the DMA,
   no on-device transposes/casts; bf16 doubles DVE tensor_tensor throughput)
 - pred/target channels interleaved on host so per-quad element ops run at
   2S width (halves per-instruction fixed overhead); corner tiles likewise
   interleaved, giving a contiguous 8-way extent fold
 - rotated rects are parallelograms: opposite edges are +/-E, so each clip
   pass needs only 2 shared cross-product tensors G (not 4), and each edge
   pair forms a slab whose inside-interval is [min(U0,U2), max(U0,U2)]
 - Green's theorem: the boundary contribution of a clipped segment is
   (t1-t0)*cross(S_k, S_{k+1}); no intersection points are materialized
 - 1/C1 = C1*exp(-ln(C1^2+1e-30)) evaluated on the idle Act engine (sign
   comes out automatically, exact-0 C1 yields 0 -> no NaNs)
 - both passes' G/C1 issued before the U stages with sl1/BCE vector work
   slotted in between, hiding the Act-chain latency; focal split around the
   extent/vterm work for the same reason
 - scalar_tensor_tensor avoided in hot @4S ops (STT doesn't get bf16 2x
   packing on the DVE; plain TT does) via pre-negated cross tiles
 - alpha_c denominator computed as relu(1-iou)+v to survive bf16 rounding
"""
import numpy as np

P = 128
S = 512          # free-dim elements per channel slot (65536 px per core)
NPX = P * S
EPS = 1e-7

_CACHE = {}


def _ensure_ntff_hook():
    import sys, types
    if "antenv.axon_hooks" in sys.modules:
        return
    try:
        import trn_agent_boot.trn_boot as tb
        hook = tb._ntff_profile_via_ctypes('/opt/axon/libaxon_pjrt.so')
        mod = types.ModuleType("antenv.axon_hooks")
        mod.get_axon_ntff_profile_hook = lambda: hook
        sys.modules["antenv.axon_hooks"] = mod
    except Exception:
        pass


def _build(debug=False, lvl=99):
    import concourse.bacc as bacc
    import concourse.tile as tile
    import concourse.mybir as mybir
    import concourse.bass as bass

    F32 = mybir.dt.float32
    BF = mybir.dt.bfloat16
    U8 = mybir.dt.uint8
    Alu = mybir.AluOpType
    Act = mybir.ActivationFunctionType
    AX_X = mybir.AxisListType.X
    PI2 = float(np.pi / 2)

    nc = bacc.Bacc("TRN2", target_bir_lowering=False, debug=False, num_devices=8)

    for v in [PI2, 1e-30, 1.0]:
        t = nc.alloc_sbuf_tensor(f"const-f32-{v}", [P, 1], F32)
        nc.gpsimd.memset(t.ap(), v)
        nc.const_aps.aps[(F32, v)] = t.ap()
    nc.all_engine_barrier()

    # GEO layout (interleaved pred/target):
    #   slots: 0 cxP 1 cxT 2 cyP 3 cyT 4 lP 5 lT 6 wP 7 wT 8 yawP 9 yawT
    d_geoy = nc.dram_tensor("geoy", [P, 2 * S], BF, kind="ExternalInput")
    d_geo = nc.dram_tensor("geo", [P, 8 * S], BF, kind="ExternalInput")
    d_cls = nc.dram_tensor("cls", [P, 10 * S], BF, kind="ExternalInput")
    d_zbp = nc.dram_tensor("zbp", [P, 4 * S], BF, kind="ExternalInput")
    d_zbt = nc.dram_tensor("zbt", [P, 4 * S], BF, kind="ExternalInput")
    d_ioup = nc.dram_tensor("ioup", [P, S], BF, kind="ExternalInput")
    d_iout = nc.dram_tensor("iout", [P, S], BF, kind="ExternalInput")
    d_ctf = nc.dram_tensor("ctf", [P, S], BF, kind="ExternalInput")
    d_w = nc.dram_tensor("w", [P, S], BF, kind="ExternalInput")
    d_out = nc.dram_tensor("out", [P, 8], F32, kind="ExternalOutput")

    V = nc.vector
    A = nc.scalar
    G = nc.gpsimd

    dbg_outs = []

    def dump(name, t):
        if not debug:
            return
        shp = [t.shape[0], int(np.prod(t.shape[1:]))]
        d = nc.dram_tensor(f"dbg_{name}", shp, t.dtype, kind="ExternalOutput")
        nc.sync.dma_start(out=d[:, :], in_=t)
        dbg_outs.append(name)

    def bc(t, i, k):
        # broadcast S-slice i of tile t over k slots
        b_ = t[:, i * S:(i + 1) * S]
        return bass.AP(tensor=b_.tensor, offset=b_.offset,
                       ap=[b_.ap[0], [0, k], [1, S]])

    def strided(t, start, num, step=2):
        # [P][num][S] view of S-slots start, start+step, ... of tile t
        b_ = t[:, start * S:(start + 1) * S]
        return bass.AP(tensor=b_.tensor, offset=b_.offset,
                       ap=[b_.ap[0], [step * S, num], [1, S]])

    def sl(t, i, k=1):
        return t[:, i * S:(i + k) * S]

    with tile.TileContext(nc) as tc:
      with tc.tile_pool(name="persist", bufs=1) as pp:
        ACCS = pp.tile([P, 8], F32, name="ACCS")
        ZACC = pp.tile([P, 4], F32, name="ZACC")
        W = pp.tile([P, S], BF, name="W")
        CTF = pp.tile([P, S], BF, name="CTF")
        IOUP = pp.tile([P, S], BF, name="IOUP")
        IOUT = pp.tile([P, S], BF, name="IOUT")
        GEO = pp.tile([P, 10 * S], BF, name="GEO")
        # corner tiles, interleaved: slot 2k = quad A corner k, 2k+1 = quad B
        CORX = pp.tile([P, 8 * S], BF, name="CORX")
        CORY = pp.tile([P, 8 * S], BF, name="CORY")
        CA = pp.tile([P, 4 * S], BF, name="CA")
        CB = pp.tile([P, 4 * S], BF, name="CB")
        CAn = pp.tile([P, 4 * S], BF, name="CAn")
        CBn = pp.tile([P, 4 * S], BF, name="CBn")
        # uv smalls, interleaved: 0 uxP 1 uxT 2 vxP 3 vxT 4 uyP 5 uyT 6 vyP 7 vyT
        UVT = pp.tile([P, 8 * S], BF, name="UVT")
        SAB = pp.tile([P, S], BF, name="SAB")
        Vv = pp.tile([P, S], BF, name="Vv")
        D2C2 = pp.tile([P, S], BF, name="D2C2")

        GY = GEO[:, 8 * S:10 * S]
        l2 = sl(GEO, 4, 2); w2 = sl(GEO, 6, 2)

        nc.sync.dma_start(out=GY, in_=d_geoy[:, :])
        nc.sync.dma_start(out=GEO[:, 0:8 * S], in_=d_geo[:, :])
        nc.sync.dma_start(out=W, in_=d_w[:, :])
        nc.sync.dma_start(out=CTF, in_=d_ctf[:, :])
        nc.sync.dma_start(out=IOUP, in_=d_ioup[:, :])
        nc.sync.dma_start(out=IOUT, in_=d_iout[:, :])

        # ============ corners + uv smalls (both quads at 2S width) ============
        with tc.tile_pool(name="corn", bufs=1) as pc:
            co2 = pc.tile([P, 2 * S], BF, name="co2")
            si2 = pc.tile([P, 2 * S], BF, name="si2")
            A.activation(co2, GY, Act.Sin, bias=PI2)
            A.activation(si2, GY, Act.Sin)
            UX2 = sl(UVT, 0, 2); VX2 = sl(UVT, 2, 2)
            UY2 = sl(UVT, 4, 2); VY2 = sl(UVT, 6, 2)
            V.tensor_tensor(UX2, l2, co2, Alu.mult)
            V.tensor_tensor(VX2, w2, si2, Alu.mult)
            V.tensor_tensor(UY2, l2, si2, Alu.mult)
            V.tensor_tensor(VY2, w2, co2, Alu.mult)
            As2 = pc.tile([P, 2 * S], BF, name="As2")
            Ad2 = pc.tile([P, 2 * S], BF, name="Ad2")
            Ps2 = pc.tile([P, 2 * S], BF, name="Ps2")
            Pd2 = pc.tile([P, 2 * S], BF, name="Pd2")
            V.tensor_tensor(As2, UX2, VX2, Alu.add)
            V.tensor_tensor(Ad2, UX2, VX2, Alu.subtract)
            V.tensor_tensor(Ps2, UY2, VY2, Alu.add)
            V.tensor_tensor(Pd2, UY2, VY2, Alu.subtract)
            cx2 = sl(GEO, 0, 2); cy2 = sl(GEO, 2, 2)
            V.scalar_tensor_tensor(sl(CORX, 0, 2), As2, 0.5, cx2, Alu.mult, Alu.add)
            V.scalar_tensor_tensor(sl(CORX, 2, 2), Ad2, -0.5, cx2, Alu.mult, Alu.add)
            V.scalar_tensor_tensor(sl(CORX, 4, 2), As2, -0.5, cx2, Alu.mult, Alu.add)
            V.scalar_tensor_tensor(sl(CORX, 6, 2), Ad2, 0.5, cx2, Alu.mult, Alu.add)
            V.scalar_tensor_tensor(sl(CORY, 0, 2), Pd2, 0.5, cy2, Alu.mult, Alu.add)
            V.scalar_tensor_tensor(sl(CORY, 2, 2), Ps2, -0.5, cy2, Alu.mult, Alu.add)
            V.scalar_tensor_tensor(sl(CORY, 4, 2), Pd2, -0.5, cy2, Alu.mult, Alu.add)
            V.scalar_tensor_tensor(sl(CORY, 6, 2), Ps2, 0.5, cy2, Alu.mult, Alu.add)
            dump("CORX", CORX); dump("CORY", CORY)

            # crosses CR_k = cross(S_k, S_{k+1}) per quad + negated copies
            for qi, (CR, CRn) in enumerate(((CA, CAn), (CB, CBn))):
                # quad qi corners: slots 2k+qi of CORX/CORY
                T1 = pc.tile([P, 4 * S], BF, name=f"crT1{qi}", tag="crT1")
                T2 = pc.tile([P, 4 * S], BF, name=f"crT2{qi}", tag="crT2")
                V.tensor_tensor(T1[:, 0:3 * S], strided(CORX, qi, 3),
                                strided(CORY, qi + 2, 3), Alu.mult)
                V.tensor_tensor(sl(T1, 3), sl(CORX, qi + 6), sl(CORY, qi), Alu.mult)
                V.tensor_tensor(T2[:, 0:3 * S], strided(CORY, qi, 3),
                                strided(CORX, qi + 2, 3), Alu.mult)
                V.tensor_tensor(sl(T2, 3), sl(CORY, qi + 6), sl(CORX, qi), Alu.mult)
                V.tensor_tensor(CR, T1, T2, Alu.subtract)
                V.tensor_tensor(CRn, T2, T1, Alu.subtract)
            dump("CA", CA); dump("CB", CB)

        # ============ clip passes: G + C1 for both passes first ============
        # pass 0: segments A (even corner slots, crosses CA), constraints B
        # pass 1: segments B (odd slots), constraints A
        # uv slice index of (ux, vx, uy, vy) for quad q: (0+q, 2+q, 4+q, 6+q)
        PASSES = ((0, CA, CB, CBn, 1), (1, CB, CA, CAn, 0))
        with tc.tile_pool(name="clip", bufs=1) as pcl:
            CLS = pcl.tile([P, 10 * S], BF, name="CLS")
            nc.sync.dma_start(out=CLS, in_=d_cls[:, :])
            G5s = {}
            C1s = {}
            RS2s = {}
            for pi, (sq, CS, CQ, CQn, qq) in enumerate(PASSES):
                SX = strided(CORX, sq, 4)
                SY = strided(CORY, sq, 4)
                for pair in range(2):
                    g5 = pcl.tile([P, 5 * S], BF, name=f"G5_{pi}_{pair}")
                    Gt = g5[:, 0:4 * S]
                    T1 = pcl.tile([P, 4 * S], BF, name=f"gT1_{pi}_{pair}", tag="gT1")
                    T2 = pcl.tile([P, 4 * S], BF, name=f"gT2_{pi}_{pair}", tag="gT2")
                    if pair == 0:
                        # E0 = (-ux, -uy): G = uy*SX - ux*SY
                        V.tensor_tensor(T1, bc(UVT, 4 + qq, 4), SX, Alu.mult)
                        V.tensor_tensor(T2, bc(UVT, 0 + qq, 4), SY, Alu.mult)
                        V.tensor_tensor(Gt, T1, T2, Alu.subtract)
                    else:
                        # E1 = (-vx, +vy): G = -(vx*SY + vy*SX); store Gneg
                        V.tensor_tensor(T1, bc(UVT, 2 + qq, 4), SY, Alu.mult)
                        V.tensor_tensor(T2, bc(UVT, 6 + qq, 4), SX, Alu.mult)
                        V.tensor_tensor(Gt, T1, T2, Alu.add)
                    A.copy(g5[:, 4 * S:5 * S], g5[:, 0:S])
                    C1 = pcl.tile([P, 4 * S], BF, name=f"C1_{pi}_{pair}")
                    if pair == 0:
                        V.tensor_tensor(C1, g5[:, S:5 * S], g5[:, 0:4 * S], Alu.subtract)
                    else:
                        # G stored negated: C1_true = Gneg_k - Gneg_{k+1}
                        V.tensor_tensor(C1, g5[:, 0:4 * S], g5[:, S:5 * S], Alu.subtract)
                    G5s[(pi, pair)] = g5
                    C1s[(pi, pair)] = C1
            # Act chains: 1/C1 = C1 * exp(-ln(C1^2 + 1e-30))
            for pi in range(2):
                for pair in range(2):
                    RS2 = pcl.tile([P, 4 * S], BF, name=f"RS2_{pi}_{pair}")
                    A.activation(RS2, C1s[(pi, pair)], Act.Square)
                    RS2s[(pi, pair)] = RS2
            for pi in range(2):
                for pair in range(2):
                    A.activation(RS2s[(pi, pair)], RS2s[(pi, pair)], Act.Ln, bias=1e-30)
            for pi in range(2):
                for pair in range(2):
                    A.activation(RS2s[(pi, pair)], RS2s[(pi, pair)], Act.Exp, scale=-1.0)
            # focal exp rides the already-loaded exp table
            E = pp.tile([P, 10 * S], BF, name="E")
            A.activation(E, CLS, Act.Exp)

            # ---- vector work to hide the Act chains: smooth-L1 + BCE ----
            V.tensor_reduce(ACCS[:, 6:7], W, AX_X, Alu.add)
            with tc.tile_pool(name="sl1", bufs=1) as ps:
                ZBP = ps.tile([P, 4 * S], BF, name="ZBP", tag="ZBP")
                ZBT = ps.tile([P, 4 * S], BF, name="ZBT", tag="ZBT")
                nc.sync.dma_start(out=ZBP, in_=d_zbp[:, :])
                nc.sync.dma_start(out=ZBT, in_=d_zbt[:, :])
                D = ps.tile([P, 4 * S], BF, name="D", tag="D")
                AD = ps.tile([P, 4 * S], BF, name="AD", tag="AD")
                V.tensor_tensor(D, ZBP, ZBT, Alu.subtract)
                A.activation(AD, D, Act.Abs)
                M = ps.tile([P, 4 * S], BF, name="M", tag="ZBP")
                MD = ps.tile([P, 4 * S], BF, name="MD", tag="ZBT")
                V.tensor_scalar(M, AD, 1.0, None, Alu.min)
                V.tensor_tensor(MD, M, AD, Alu.mult)
                M2H = ps.tile([P, 4 * S], BF, name="M2H", tag="D")
                SL1 = ps.tile([P, 4 * S], BF, name="SL1", tag="AD")
                A.activation(M2H, M, Act.Square, scale=float(np.sqrt(0.5)))
                V.tensor_tensor(SL1, MD, M2H, Alu.subtract)
                V.tensor_tensor(SL1, SL1, bc(W, 0, 4), Alu.mult)
                V.tensor_reduce(ZACC, SL1.rearrange("p (c f) -> p c f", c=4),
                                AX_X, Alu.add)
                V.tensor_copy(ACCS[:, 2:3], ZACC[:, 0:1])
                V.tensor_copy(ACCS[:, 3:4], ZACC[:, 1:2])
                V.tensor_tensor(ACCS[:, 4:5], ZACC[:, 2:3], ZACC[:, 3:4], Alu.add)
                dump("SL1", SL1)

            with tc.tile_pool(name="bce", bufs=1) as pb:
                AXb = pb.tile([P, S], BF, name="AXb")
                SP = pb.tile([P, S], BF, name="SP")
                RL = pb.tile([P, S], BF, name="RL")
                XY = pb.tile([P, S], BF, name="XY")
                A.activation(AXb, IOUP, Act.Abs)
                EB = pb.tile([P, S], BF, name="EB")
                A.activation(EB, AXb, Act.Exp, scale=-1.0)
                A.activation(SP, EB, Act.Ln, bias=1.0)
                A.activation(RL, IOUP, Act.Relu)
                V.tensor_tensor(XY, IOUP, IOUT, Alu.mult)
                V.tensor_tensor(RL, RL, XY, Alu.subtract)
                V.tensor_tensor(RL, RL, SP, Alu.add)
                V.tensor_tensor(RL, RL, W, Alu.mult)
                V.tensor_reduce(ACCS[:, 5:6], RL, AX_X, Alu.add)
                dump("BCE", RL)

            # ---- U, slab intervals, contributions ----
            CONTRS = []
            for pi, (sq, CS, CQ, CQn, qq) in enumerate(PASSES):
                LOHI = []
                for pair in range(2):
                    Gt = G5s[(pi, pair)][:, 0:4 * S]
                    C1 = C1s[(pi, pair)]
                    RS2 = RS2s[(pi, pair)]
                    REC = pcl.tile([P, 4 * S], BF, name=f"REC_{pi}_{pair}", tag=f"REC_{pair}")
                    V.tensor_tensor(REC, C1, RS2, Alu.mult)
                    j0, j2 = (0, 2) if pair == 0 else (1, 3)
                    T0g = pcl.tile([P, 4 * S], BF, name=f"T0g_{pi}_{pair}", tag="gT1")
                    U0 = pcl.tile([P, 4 * S], BF, name=f"U0_{pi}_{pair}", tag=f"U0_{pair}")
                    T2g = pcl.tile([P, 4 * S], BF, name=f"T2g_{pi}_{pair}", tag="gT2")
                    U2 = pcl.tile([P, 4 * S], BF, name=f"U2_{pi}_{pair}", tag=f"U2_{pair}")
                    if pair == 0:
                        # U0 = (-CQ_j0 - G)*REC ; U2 = (CQ_j2 - G)*REC
                        V.tensor_tensor(T0g, bc(CQn, j0, 4), Gt, Alu.subtract)
                        V.tensor_tensor(T2g, bc(CQ, j2, 4), Gt, Alu.subtract)
                    else:
                        # G stored negated: U0 = (Gneg - CQ_j0)*REC
                        #                   U2 = (Gneg + CQ_j2)*REC
                        V.tensor_tensor(T0g, Gt, bc(CQ, j0, 4), Alu.subtract)
                        V.tensor_tensor(T2g, Gt, bc(CQ, j2, 4), Alu.add)
                    V.tensor_tensor(U0, T0g, REC, Alu.mult)
                    V.tensor_tensor(U2, T2g, REC, Alu.mult)
                    lo = pcl.tile([P, 4 * S], BF, name=f"lo_{pi}_{pair}", tag=f"lo_{pair}")
                    hi = pcl.tile([P, 4 * S], BF, name=f"hi_{pi}_{pair}", tag=f"hi_{pair}")
                    V.tensor_tensor(lo, U0, U2, Alu.min)
                    V.tensor_tensor(hi, U0, U2, Alu.max)
                    LOHI.append((lo, hi))
                LO = LOHI[0][0]; HI = LOHI[0][1]
                V.tensor_tensor(LO, LO, LOHI[1][0], Alu.max)
                V.tensor_tensor(HI, HI, LOHI[1][1], Alu.min)
                T0 = pcl.tile([P, 4 * S], BF, name=f"T0_{pi}", tag="gT1")
                T1v = pcl.tile([P, 4 * S], BF, name=f"T1v_{pi}", tag="gT2")
                V.tensor_scalar(T0, LO, 0.0, 1.0, Alu.max, Alu.min)
                V.tensor_scalar(T1v, HI, 1.0, 0.0, Alu.min, Alu.max)
                DT = pcl.tile([P, 4 * S], BF, name=f"DT_{pi}", tag="U0_0")
                V.tensor_tensor(DT, T1v, T0, Alu.subtract)
                V.tensor_scalar(DT, DT, 0.0, None, Alu.max)
                CONTR = pcl.tile([P, 4 * S], BF, name=f"CONTR_{pi}", tag=f"CONTR_{pi}")
                V.tensor_tensor(CONTR, DT, CS, Alu.mult)
                CONTRS.append(CONTR)
                dump(f"CONTR_{pi}", CONTR)
            # joint fold of both passes' contributions
            FF = pcl.tile([P, 2 * S], BF, name="FF", tag="gT1")
            GGt = pcl.tile([P, 2 * S], BF, name="GGt", tag="gT2")
            V.tensor_tensor(FF, CONTRS[0][:, 0:2 * S], CONTRS[0][:, 2 * S:4 * S], Alu.add)
            V.tensor_tensor(GGt, CONTRS[1][:, 0:2 * S], CONTRS[1][:, 2 * S:4 * S], Alu.add)
            V.tensor_tensor(FF, FF, GGt, Alu.add)
            V.tensor_tensor(SAB, sl(FF, 0), sl(FF, 1), Alu.add)
            dump("SAB", SAB)

        # ============ focal part 1: folds, select, pt ============
        with tc.tile_pool(name="focal", bufs=1) as pf:
            F1 = pf.tile([P, 5 * S], BF, name="F1")
            V.tensor_tensor(F1, E[:, 0:5 * S], E[:, 5 * S:10 * S], Alu.add)
            V.tensor_tensor(F1[:, 0:2 * S], F1[:, 0:2 * S], F1[:, 2 * S:4 * S], Alu.add)
            Ssum = pf.tile([P, S], BF, name="Ssum")
            V.tensor_tensor(Ssum, sl(F1, 0), sl(F1, 1), Alu.add)
            V.tensor_tensor(Ssum, Ssum, sl(F1, 4), Alu.add)
            ET = pf.tile([P, S], BF, name="ET")
            A.copy(ET, sl(E, 0))
            for c in range(1, 10):
                MC = pf.tile([P, S], U8, name=f"MC{c}", tag="MC")
                V.tensor_scalar(MC, CTF, float(c), None, Alu.is_equal)
                V.copy_predicated(ET, MC, sl(E, c))
            Ssf = pf.tile([P, S], F32, name="Ssf")
            V.tensor_copy(Ssf, Ssum)
            RSf = pf.tile([P, S], F32, name="RSf")
            V.reciprocal_approx_fast(RSf, Ssf)
            RS = pf.tile([P, S], BF, name="RS")
            A.copy(RS, RSf)
            PT = pf.tile([P, S], BF, name="PT")
            V.tensor_tensor(PT, ET, RS, Alu.mult)
            V.tensor_scalar(PT, PT, EPS, 1.0 - EPS, Alu.max, Alu.min)
            LG = pf.tile([P, S], BF, name="LG")
            A.activation(LG, PT, Act.Ln)

            # ============ extents -> c2, d2 ============
            with tc.tile_pool(name="d2c2", bufs=1) as pd:
                exts = []
                for (CT, op, nm) in ((CORX, Alu.max, "xmax"), (CORX, Alu.min, "xmin"),
                                     (CORY, Alu.max, "ymax"), (CORY, Alu.min, "ymin")):
                    f1 = pd.tile([P, 4 * S], BF, name=f"f1_{nm}", tag="f1")
                    V.tensor_tensor(f1, CT[:, 0:4 * S], CT[:, 4 * S:8 * S], op)
                    V.tensor_tensor(f1[:, 0:2 * S], f1[:, 0:2 * S], f1[:, 2 * S:4 * S], op)
                    ex = pd.tile([P, S], BF, name=f"ext_{nm}", tag=f"ext_{nm}")
                    V.tensor_tensor(ex, sl(f1, 0), sl(f1, 1), op)
                    exts.append(ex)
                BW = pd.tile([P, S], BF, name="BW")
                BH = pd.tile([P, S], BF, name="BH")
                V.tensor_tensor(BW, exts[0], exts[1], Alu.subtract)
                V.tensor_tensor(BH, exts[2], exts[3], Alu.subtract)
                SQW = pd.tile([P, S], BF, name="SQW", tag="f1")
                SQH = pd.tile([P, S], BF, name="SQH", tag="sqh")
                A.activation(SQW, BW, Act.Square)
                A.activation(SQH, BH, Act.Square)
                C2 = pd.tile([P, S], F32, name="C2")
                V.scalar_tensor_tensor(C2, SQW, EPS, SQH, Alu.add, Alu.add)
                RC2f = pd.tile([P, S], F32, name="RC2f")
                V.reciprocal_approx_fast(RC2f, C2)
                RC2 = pd.tile([P, S], BF, name="RC2")
                A.copy(RC2, RC2f)
                # d2: (cxP-cxT)^2 + (cyP-cyT)^2 via one 2S-wide pass
                DXY = pd.tile([P, 2 * S], BF, name="DXY")
                V.tensor_tensor(DXY, strided(GEO, 0, 2), strided(GEO, 1, 2),
                                Alu.subtract)
                SQ2 = pd.tile([P, 2 * S], BF, name="SQ2")
                A.activation(SQ2, DXY, Act.Square)
                D2 = pd.tile([P, S], BF, name="D2t")
                V.tensor_tensor(D2, sl(SQ2, 0), sl(SQ2, 1), Alu.add)
                V.tensor_tensor(D2C2, D2, RC2, Alu.mult)
                dump("D2C2", D2C2)

            # ============ v term (both quads at 2S width) ============
            with tc.tile_pool(name="vterm", bufs=1) as pv:
                rlf = pv.tile([P, 2 * S], F32, name="rlf")
                rwf = pv.tile([P, 2 * S], F32, name="rwf")
                V.tensor_scalar(rlf, l2, EPS, None, Alu.add)
                V.reciprocal_approx_fast(rlf, rlf)
                V.tensor_scalar(rwf, w2, 1e-30, None, Alu.add)
                V.reciprocal_approx_fast(rwf, rwf)
                rl = pv.tile([P, 2 * S], BF, name="rl")
                rw = pv.tile([P, 2 * S], BF, name="rw")
                A.copy(rl, rlf)
                A.copy(rw, rwf)
                x1 = pv.tile([P, 2 * S], BF, name="x1")
                x2 = pv.tile([P, 2 * S], BF, name="x2")
                V.tensor_tensor(x1, w2, rl, Alu.mult)
                V.tensor_tensor(x2, l2, rw, Alu.mult)
                mn = pv.tile([P, 2 * S], BF, name="mn")
                V.tensor_tensor(mn, x1, x2, Alu.min)
                aa = pv.tile([P, 2 * S], BF, name="aa")
                A.activation(aa, mn, Act.Arctan)
                m8 = pv.tile([P, 2 * S], U8, name="m8")
                V.tensor_scalar(m8, x1, 1.0, None, Alu.is_gt)
                tt2 = pv.tile([P, 2 * S], BF, name="tt2")
                V.tensor_scalar(tt2, aa, -1.0, PI2, Alu.mult, Alu.add)
                AT = pv.tile([P, 2 * S], BF, name="AT")
                A.copy(AT, aa)
                V.copy_predicated(AT, m8, tt2)
                DV = pv.tile([P, S], BF, name="DV")
                V.tensor_tensor(DV, sl(AT, 0), sl(AT, 1), Alu.subtract)
                A.activation(Vv, DV, Act.Square, scale=float(2.0 / np.pi))
                dump("Vv", Vv)

            # ============ iou + bev assembly ============
            with tc.tile_pool(name="asm", bufs=1) as pa:
                INTER = pa.tile([P, S], BF, name="INTER")
                A.activation(INTER, SAB, Act.Abs, scale=0.5)
                AR2 = pa.tile([P, 2 * S], BF, name="AR2")
                V.tensor_tensor(AR2, l2, w2, Alu.mult)
                UN = pa.tile([P, S], BF, name="UN")
                V.tensor_tensor(UN, sl(AR2, 0), sl(AR2, 1), Alu.add)
                V.tensor_tensor(UN, UN, INTER, Alu.subtract)
                UNf = pa.tile([P, S], F32, name="UNf")
                V.tensor_scalar(UNf, UN, EPS, None, Alu.max)
                URCf = pa.tile([P, S], F32, name="URCf")
                V.reciprocal_approx_fast(URCf, UNf)
                URC = pa.tile([P, S], BF, name="URC")
                A.copy(URC, URCf)
                IOU = pa.tile([P, S], BF, name="IOU")
                V.tensor_tensor(IOU, INTER, URC, Alu.mult)
                OMI = pa.tile([P, S], BF, name="OMI")
                A.activation(OMI, IOU, Act.Relu, scale=-1.0, bias=1.0)
                DEN = pa.tile([P, S], F32, name="DEN")
                V.scalar_tensor_tensor(DEN, OMI, EPS, Vv, Alu.add, Alu.add)
                DRCf = pa.tile([P, S], F32, name="DRCf")
                V.reciprocal_approx_fast(DRCf, DEN)
                DRC = pa.tile([P, S], BF, name="DRC")
                A.copy(DRC, DRCf)
                ALC = pa.tile([P, S], BF, name="ALC")
                V.tensor_tensor(ALC, Vv, DRC, Alu.mult)
                V.tensor_tensor(ALC, ALC, Vv, Alu.mult)
                LB = pa.tile([P, S], BF, name="LB")
                V.tensor_tensor(LB, OMI, D2C2, Alu.add)
                V.tensor_tensor(LB, LB, ALC, Alu.add)
                V.tensor_tensor(LB, LB, W, Alu.mult)
                V.tensor_reduce(ACCS[:, 1:2], LB, AX_X, Alu.add)
                dump("IOU", IOU)
                dump("LBW", LB)

            # ============ focal part 2 ============
            OMP = pf.tile([P, S], BF, name="OMP")
            V.tensor_scalar(OMP, PT, -1.0, 1.0, Alu.mult, Alu.add)
            MPOS = pf.tile([P, S], U8, name="MPOS")
            V.tensor_scalar(MPOS, CTF, 0.0, None, Alu.is_gt)
            ALPH = pf.tile([P, S], BF, name="ALPH")
            QTR = pf.tile([P, S], BF, name="QTR")
            G.memset(ALPH, 0.75)
            G.memset(QTR, 0.25)
            V.copy_predicated(ALPH, MPOS, QTR)
            FL = pf.tile([P, S], BF, name="FL")
            V.tensor_tensor(FL, OMP, OMP, Alu.mult)
            V.tensor_tensor(FL, FL, LG, Alu.mult)
            V.scalar_tensor_tensor(FL, FL, -1.0, ALPH, Alu.mult, Alu.mult)
            V.tensor_reduce(ACCS[:, 0:1], FL, AX_X, Alu.add)
            dump("PT", PT)
            dump("FL", FL)

        A.memzero(ACCS[:, 7:8])
        nc.sync.dma_start(out=d_out[:, :], in_=ACCS)

    nc.compile()
    nc._dbg_outs = dbg_outs
    return nc


def _get_nc():
    if "nc" not in _CACHE:
        _ensure_ntff_hook()
        _CACHE["nc"] = _build()
    return _CACHE["nc"]


def _prep_core(cls_b, regp_b, regt_b, ioup_b, iout_b, ct_b, w_b):
    """Build one core's input map (bf16, [P, C*S] layouts) from f32 [C,H,W]."""
    import ml_dtypes
    BF = ml_dtypes.bfloat16

    def chans(x, idxs):
        # x [C,H,W] -> [P, len(idxs)*S] slot-interleaved
        sel = x[idxs].reshape(len(idxs), P, S)
        return np.ascontiguousarray(sel.transpose(1, 0, 2).reshape(P, len(idxs) * S)).astype(BF)

    def geo_interleave(xp, xt, idxs):
        # slots [cP, cT] per channel: [P, 2*len(idxs)*S]
        selp = xp[idxs].reshape(len(idxs), P, S)
        selt = xt[idxs].reshape(len(idxs), P, S)
        inter = np.stack([selp, selt], axis=1)  # [C, 2, P, S]
        return np.ascontiguousarray(
            inter.transpose(2, 0, 1, 3).reshape(P, 2 * len(idxs) * S)).astype(BF)

    return {
        "cls": chans(cls_b, list(range(10))),
        "geo": geo_interleave(regp_b, regt_b, [0, 1, 3, 4]),
        "geoy": geo_interleave(regp_b, regt_b, [6]),
        "zbp": chans(regp_b, [2, 5, 7, 8]),
        "zbt": chans(regt_b, [2, 5, 7, 8]),
        "ioup": ioup_b.reshape(P, S).astype(BF),
        "iout": iout_b.reshape(P, S).astype(BF),
        "ctf": ct_b.reshape(P, S).astype(np.float32).astype(BF),
        "w": w_b.reshape(P, S).astype(BF),
    }


def kernel(**inputs):
    from concourse.bass_utils import run_bass_kernel_spmd

    nc = _get_nc()
    cls_pred = np.asarray(inputs["cls_pred"], dtype=np.float32)
    reg_pred = np.asarray(inputs["reg_pred"], dtype=np.float32)
    iou_pred = np.asarray(inputs["iou_pred"], dtype=np.float32)
    cls_targets = np.asarray(inputs["cls_targets"], dtype=np.int32)
    reg_targets = np.asarray(inputs["reg_targets"], dtype=np.float32)
    reg_weights = np.asarray(inputs["reg_weights"], dtype=np.float32)
    iou_targets = np.asarray(inputs["iou_targets"], dtype=np.float32)

    B = cls_pred.shape[0]
    in_maps = []
    for b in range(B):
        in_maps.append(_prep_core(cls_pred[b], reg_pred[b], reg_targets[b],
                                  iou_pred[b], iou_targets[b],
                                  cls_targets[b], reg_weights[b]))
    res = run_bass_kernel_spmd(nc, in_maps, core_ids=list(range(8)))
    _CACHE["last_result"] = res
    sums = np.zeros(8, np.float64)
    for r in res.results:
        sums += r["out"].astype(np.float64).sum(axis=0)
    num_pos = max(sums[6], 1.0)
    out = np.array([sums[0], sums[1], sums[2], sums[3], sums[4], sums[5]],
                   np.float64) / num_pos
    return out.astype(np.float32)


# revision 14
# speedup vs baseline: 2.6470x; 1.0393x over previous
"""Trainium2 Bass kernel for nn_DetectionBEVLoss (bf16 pipeline, v3).

Takes FULL inputs (B=8,...), shards batch across 8 NeuronCores (one batch
element per core), computes per-core partial sums of the 6 loss terms plus the
positive count on-device, and finishes the tiny reduction on host.

Key optimizations over the f32 baseline:
 - host casts inputs to bf16 and pre-arranges [P, C*S] layouts (half the DMA,
   no on-device transposes/casts; bf16 doubles DVE tensor_tensor throughput)
 - pred/target channels interleaved on host so per-quad element ops run at
   2S width (halves per-instruction fixed overhead); corner tiles likewise
   interleaved, giving a contiguous 8-way extent fold
 - rotated rects are parallelograms: opposite edges are +/-E, so each clip
   pass needs only 2 shared cross-product tensors G (not 4), and each edge
   pair forms a slab whose inside-interval is [min(U0,U2), max(U0,U2)]
 - Green's theorem: the boundary contribution of a clipped segment is
   (t1-t0)*cross(S_k, S_{k+1}); no intersection points are materialized
 - 1/C1 = C1*exp(-ln(C1^2+1e-30)) evaluated on the idle Act engine (sign
   comes out automatically, exact-0 C1 yields 0 -> no NaNs)
 - both passes' G/C1 issued before the U stages with sl1/BCE vector work
   slotted in between, hiding the Act-chain latency; focal split around the
   extent/vterm work for the same reason
 - scalar_tensor_tensor avoided in hot @4S ops (STT doesn't get bf16 2x
   packing on the DVE; plain TT does) via pre-negated cross tiles
 - alpha_c denominator computed as relu(1-iou)+v to survive bf16 rounding
"""
import numpy as np

P = 128
S = 512          # free-dim elements per channel slot (65536 px per core)
NPX = P * S
EPS = 1e-7

_CACHE = {}


def _ensure_ntff_hook():
    import sys, types
    if "antenv.axon_hooks" in sys.modules:
        return
    try:
        import trn_agent_boot.trn_boot as tb
        hook = tb._ntff_profile_via_ctypes('/opt/axon/libaxon_pjrt.so')
        mod = types.ModuleType("antenv.axon_hooks")
        mod.get_axon_ntff_profile_hook = lambda: hook
        sys.modules["antenv.axon_hooks"] = mod
    except Exception:
        pass


def _build(debug=False, lvl=99):
    import concourse.bacc as bacc
    import concourse.tile as tile
    import concourse.mybir as mybir
    import concourse.bass as bass

    F32 = mybir.dt.float32
    BF = mybir.dt.bfloat16
    U8 = mybir.dt.uint8
    Alu = mybir.AluOpType
    Act = mybir.ActivationFunctionType
    AX_X = mybir.AxisListType.X
    PI2 = float(np.pi / 2)

    nc = bacc.Bacc("TRN2", target_bir_lowering=False, debug=False, num_devices=8)

    for v in [PI2, 1e-30, 1.0]:
        t = nc.alloc_sbuf_tensor(f"const-f32-{v}", [P, 1], F32)
        nc.gpsimd.memset(t.ap(), v)
        nc.const_aps.aps[(F32, v)] = t.ap()
    nc.all_engine_barrier()

    # GEO layout (interleaved pred/target):
    #   slots: 0 cxP 1 cxT 2 cyP 3 cyT 4 lP 5 lT 6 wP 7 wT 8 yawP 9 yawT
    d_geoy = nc.dram_tensor("geoy", [P, 2 * S], BF, kind="ExternalInput")
    d_geo = nc.dram_tensor("geo", [P, 8 * S], BF, kind="ExternalInput")
    d_cls = nc.dram_tensor("cls", [P, 10 * S], BF, kind="ExternalInput")
    d_zbp = nc.dram_tensor("zbp", [P, 4 * S], BF, kind="ExternalInput")
    d_zbt = nc.dram_tensor("zbt", [P, 4 * S], BF, kind="ExternalInput")
    d_ioup = nc.dram_tensor("ioup", [P, S], BF, kind="ExternalInput")
    d_iout = nc.dram_tensor("iout", [P, S], BF, kind="ExternalInput")
    d_ctf = nc.dram_tensor("ctf", [P, S], BF, kind="ExternalInput")
    d_w = nc.dram_tensor("w", [P, S], BF, kind="ExternalInput")
    d_out = nc.dram_tensor("out", [P, 8], F32, kind="ExternalOutput")

    V = nc.vector
    A = nc.scalar
    G = nc.gpsimd

    dbg_outs = []

    def dump(name, t):
        if not debug:
            return
        shp = [t.shape[0], int(np.prod(t.shape[1:]))]
        d = nc.dram_tensor(f"dbg_{name}", shp, t.dtype, kind="ExternalOutput")
        nc.sync.dma_start(out=d[:, :], in_=t)
        dbg_outs.append(name)

    def bc(t, i, k):
        # broadcast S-slice i of tile t over k slots
        b_ = t[:, i * S:(i + 1) * S]
        return bass.AP(tensor=b_.tensor, offset=b_.offset,
                       ap=[b_.ap[0], [0, k], [1, S]])

    def strided(t, start, num, step=2):
        # [P][num][S] view of S-slots start, start+step, ... of tile t
        b_ = t[:, start * S:(start + 1) * S]
        return bass.AP(tensor=b_.tensor, offset=b_.offset,
                       ap=[b_.ap[0], [step * S, num], [1, S]])

    def sl(t, i, k=1):
        return t[:, i * S:(i + k) * S]

    with tile.TileContext(nc) as tc:
      with tc.tile_pool(name="persist", bufs=1) as pp:
        ACCS = pp.tile([P, 8], F32, name="ACCS")
        ZACC = pp.tile([P, 4], F32, name="ZACC")
        W = pp.tile([P, S], BF, name="W")
        CTF = pp.tile([P, S], BF, name="CTF")
        IOUP = pp.tile([P, S], BF, name="IOUP")
        IOUT = pp.tile([P, S], BF, name="IOUT")
        GEO = pp.tile([P, 10 * S], BF, name="GEO")
        # corner tiles, interleaved: slot 2k = quad A corner k, 2k+1 = quad B
        CORX = pp.tile([P, 8 * S], BF, name="CORX")
        CORY = pp.tile([P, 8 * S], BF, name="CORY")
        # crosses, interleaved like the corners: slot 2j+q = quad q, edge j
        CAB = pp.tile([P, 8 * S], BF, name="CAB")
        CABn = pp.tile([P, 8 * S], BF, name="CABn")
        # uv smalls, interleaved: 0 uxP 1 uxT 2 vxP 3 vxT 4 uyP 5 uyT 6 vyP 7 vyT
        UVT = pp.tile([P, 8 * S], BF, name="UVT")
        SAB = pp.tile([P, S], BF, name="SAB")
        Vv = pp.tile([P, S], BF, name="Vv")
        D2C2 = pp.tile([P, S], BF, name="D2C2")

        GY = GEO[:, 8 * S:10 * S]
        l2 = sl(GEO, 4, 2); w2 = sl(GEO, 6, 2)

        nc.sync.dma_start(out=W, in_=d_w[:, :])
        nc.sync.dma_start(out=GY, in_=d_geoy[:, :])
        nc.sync.dma_start(out=GEO[:, 0:8 * S], in_=d_geo[:, :])
        V.tensor_reduce(ACCS[:, 6:7], W, AX_X, Alu.add)
        nc.sync.dma_start(out=CTF, in_=d_ctf[:, :])
        nc.sync.dma_start(out=IOUP, in_=d_ioup[:, :])
        nc.sync.dma_start(out=IOUT, in_=d_iout[:, :])

        # ============ corners + uv smalls (both quads at 2S width) ============
        with tc.tile_pool(name="corn", bufs=1) as pc:
            co2 = pc.tile([P, 2 * S], BF, name="co2")
            si2 = pc.tile([P, 2 * S], BF, name="si2")
            A.activation(co2, GY, Act.Sin, bias=PI2)
            A.activation(si2, GY, Act.Sin)
            UX2 = sl(UVT, 0, 2); VX2 = sl(UVT, 2, 2)
            UY2 = sl(UVT, 4, 2); VY2 = sl(UVT, 6, 2)
            V.tensor_tensor(UX2, l2, co2, Alu.mult)
            V.tensor_tensor(VX2, w2, si2, Alu.mult)
            V.tensor_tensor(UY2, l2, si2, Alu.mult)
            V.tensor_tensor(VY2, w2, co2, Alu.mult)
            As2 = pc.tile([P, 2 * S], BF, name="As2")
            Ad2 = pc.tile([P, 2 * S], BF, name="Ad2")
            Ps2 = pc.tile([P, 2 * S], BF, name="Ps2")
            Pd2 = pc.tile([P, 2 * S], BF, name="Pd2")
            V.tensor_tensor(As2, UX2, VX2, Alu.add)
            V.tensor_tensor(Ad2, UX2, VX2, Alu.subtract)
            V.tensor_tensor(Ps2, UY2, VY2, Alu.add)
            V.tensor_tensor(Pd2, UY2, VY2, Alu.subtract)
            cx2 = sl(GEO, 0, 2); cy2 = sl(GEO, 2, 2)
            V.scalar_tensor_tensor(sl(CORX, 0, 2), As2, 0.5, cx2, Alu.mult, Alu.add)
            V.scalar_tensor_tensor(sl(CORX, 2, 2), Ad2, -0.5, cx2, Alu.mult, Alu.add)
            V.scalar_tensor_tensor(sl(CORX, 4, 2), As2, -0.5, cx2, Alu.mult, Alu.add)
            V.scalar_tensor_tensor(sl(CORX, 6, 2), Ad2, 0.5, cx2, Alu.mult, Alu.add)
            V.scalar_tensor_tensor(sl(CORY, 0, 2), Pd2, 0.5, cy2, Alu.mult, Alu.add)
            V.scalar_tensor_tensor(sl(CORY, 2, 2), Ps2, -0.5, cy2, Alu.mult, Alu.add)
            V.scalar_tensor_tensor(sl(CORY, 4, 2), Pd2, -0.5, cy2, Alu.mult, Alu.add)
            V.scalar_tensor_tensor(sl(CORY, 6, 2), Ps2, 0.5, cy2, Alu.mult, Alu.add)
            dump("CORX", CORX); dump("CORY", CORY)

            # crosses CR_{q,k} = cross(S_k, S_{k+1}), both quads jointly
            T1 = pc.tile([P, 8 * S], BF, name="crT1")
            T2 = pc.tile([P, 8 * S], BF, name="crT2")
            V.tensor_tensor(T1[:, 0:6 * S], CORX[:, 0:6 * S], CORY[:, 2 * S:8 * S], Alu.mult)
            V.tensor_tensor(T1[:, 6 * S:8 * S], CORX[:, 6 * S:8 * S], CORY[:, 0:2 * S], Alu.mult)
            V.tensor_tensor(T2[:, 0:6 * S], CORY[:, 0:6 * S], CORX[:, 2 * S:8 * S], Alu.mult)
            V.tensor_tensor(T2[:, 6 * S:8 * S], CORY[:, 6 * S:8 * S], CORX[:, 0:2 * S], Alu.mult)
            V.tensor_tensor(CAB, T1, T2, Alu.subtract)
            V.tensor_tensor(CABn, T2, T1, Alu.subtract)
            dump("CAB", CAB)

        # ============ clip passes: G + C1 for both passes first ============
        # pass 0: segments A (even corner slots, crosses CA), constraints B
        # pass 1: segments B (odd slots), constraints A
        # uv slice index of (ux, vx, uy, vy) for quad q: (0+q, 2+q, 4+q, 6+q)
        # pass tuples: (corner slot parity, CS slot base, CQ parity)
        PASSES = ((0, 1), (1, 0))
        with tc.tile_pool(name="clip", bufs=1) as pcl:
            CLS = pcl.tile([P, 10 * S], BF, name="CLS")
            nc.sync.dma_start(out=CLS, in_=d_cls[:, :])
            G5s = {}
            C1s = {}
            RS2s = {}
            for pi, (sq, qq) in enumerate(PASSES):
                SX = strided(CORX, sq, 4)
                SY = strided(CORY, sq, 4)
                for pair in range(2):
                    g5 = pcl.tile([P, 5 * S], BF, name=f"G5_{pi}_{pair}")
                    Gt = g5[:, 0:4 * S]
                    T1 = pcl.tile([P, 4 * S], BF, name=f"gT1_{pi}_{pair}", tag="gT1")
                    T2 = pcl.tile([P, 4 * S], BF, name=f"gT2_{pi}_{pair}", tag="gT2")
                    if pair == 0:
                        # E0 = (-ux, -uy): G = uy*SX - ux*SY
                        V.tensor_tensor(T1, bc(UVT, 4 + qq, 4), SX, Alu.mult)
                        V.tensor_tensor(T2, bc(UVT, 0 + qq, 4), SY, Alu.mult)
                        V.tensor_tensor(Gt, T1, T2, Alu.subtract)
                    else:
                        # E1 = (-vx, +vy): G = -(vx*SY + vy*SX); store Gneg
                        V.tensor_tensor(T1, bc(UVT, 2 + qq, 4), SY, Alu.mult)
                        V.tensor_tensor(T2, bc(UVT, 6 + qq, 4), SX, Alu.mult)
                        V.tensor_tensor(Gt, T1, T2, Alu.add)
                    A.copy(g5[:, 4 * S:5 * S], g5[:, 0:S])
                    C1 = pcl.tile([P, 4 * S], BF, name=f"C1_{pi}_{pair}")
                    if pair == 0:
                        V.tensor_tensor(C1, g5[:, S:5 * S], g5[:, 0:4 * S], Alu.subtract)
                    else:
                        # G stored negated: C1_true = Gneg_k - Gneg_{k+1}
                        V.tensor_tensor(C1, g5[:, 0:4 * S], g5[:, S:5 * S], Alu.subtract)
                    G5s[(pi, pair)] = g5
                    C1s[(pi, pair)] = C1
                # per-pass Act chain so pass pi's RECs are ready while the
                # vector engine builds pass pi+1's G/C1
                for pair in range(2):
                    RS2 = pcl.tile([P, 4 * S], BF, name=f"RS2_{pi}_{pair}")
                    A.activation(RS2, C1s[(pi, pair)], Act.Square)
                    RS2s[(pi, pair)] = RS2
                for pair in range(2):
                    A.activation(RS2s[(pi, pair)], RS2s[(pi, pair)], Act.Ln, bias=1e-30)
                for pair in range(2):
                    A.activation(RS2s[(pi, pair)], RS2s[(pi, pair)], Act.Exp, scale=-1.0)
            # focal exp rides the already-loaded exp table
            E = pp.tile([P, 10 * S], BF, name="E")
            A.activation(E, CLS, Act.Exp)

            # ---- U, slab intervals, contributions ----
            CONTRS = []
            for pi, (sq, qq) in enumerate(PASSES):
                LOHI = []
                for pair in range(2):
                    Gt = G5s[(pi, pair)][:, 0:4 * S]
                    C1 = C1s[(pi, pair)]
                    RS2 = RS2s[(pi, pair)]
                    REC = pcl.tile([P, 4 * S], BF, name=f"REC_{pi}_{pair}", tag=f"REC_{pair}")
                    V.tensor_tensor(REC, C1, RS2, Alu.mult)
                    j0, j2 = (0, 2) if pair == 0 else (1, 3)
                    T0g = pcl.tile([P, 4 * S], BF, name=f"T0g_{pi}_{pair}", tag="gT1")
                    U0 = pcl.tile([P, 4 * S], BF, name=f"U0_{pi}_{pair}", tag=f"U0_{pair}")
                    T2g = pcl.tile([P, 4 * S], BF, name=f"T2g_{pi}_{pair}", tag="gT2")
                    U2 = pcl.tile([P, 4 * S], BF, name=f"U2_{pi}_{pair}", tag=f"U2_{pair}")
                    if pair == 0:
                        # U0 = (-CQ_j0 - G)*REC ; U2 = (CQ_j2 - G)*REC
                        V.tensor_tensor(T0g, bc(CABn, 2 * j0 + qq, 4), Gt, Alu.subtract)
                        V.tensor_tensor(T2g, bc(CAB, 2 * j2 + qq, 4), Gt, Alu.subtract)
                    else:
                        # G stored negated: U0 = (Gneg - CQ_j0)*REC
                        #                   U2 = (Gneg + CQ_j2)*REC
                        V.tensor_tensor(T0g, Gt, bc(CAB, 2 * j0 + qq, 4), Alu.subtract)
                        V.tensor_tensor(T2g, Gt, bc(CAB, 2 * j2 + qq, 4), Alu.add)
                    V.tensor_tensor(U0, T0g, REC, Alu.mult)
                    V.tensor_tensor(U2, T2g, REC, Alu.mult)
                    lo = pcl.tile([P, 4 * S], BF, name=f"lo_{pi}_{pair}", tag=f"lo_{pair}")
                    hi = pcl.tile([P, 4 * S], BF, name=f"hi_{pi}_{pair}", tag=f"hi_{pair}")
                    V.tensor_tensor(lo, U0, U2, Alu.min)
                    V.tensor_tensor(hi, U0, U2, Alu.max)
                    LOHI.append((lo, hi))
                LO = LOHI[0][0]; HI = LOHI[0][1]
                V.tensor_tensor(LO, LO, LOHI[1][0], Alu.max)
                V.tensor_tensor(HI, HI, LOHI[1][1], Alu.min)
                T0 = pcl.tile([P, 4 * S], BF, name=f"T0_{pi}", tag="gT1")
                T1v = pcl.tile([P, 4 * S], BF, name=f"T1v_{pi}", tag="gT2")
                V.tensor_scalar(T0, LO, 0.0, 1.0, Alu.max, Alu.min)
                V.tensor_scalar(T1v, HI, 1.0, 0.0, Alu.min, Alu.max)
                DT = pcl.tile([P, 4 * S], BF, name=f"DT_{pi}", tag="U0_0")
                V.tensor_tensor(DT, T1v, T0, Alu.subtract)
                V.tensor_scalar(DT, DT, 0.0, None, Alu.max)
                CONTR = pcl.tile([P, 4 * S], BF, name=f"CONTR_{pi}", tag=f"CONTR_{pi}")
                V.tensor_tensor(CONTR, DT, strided(CAB, sq, 4), Alu.mult)
                CONTRS.append(CONTR)
                dump(f"CONTR_{pi}", CONTR)
            # joint fold of both passes' contributions
            FF = pcl.tile([P, 2 * S], BF, name="FF", tag="gT1")
            GGt = pcl.tile([P, 2 * S], BF, name="GGt", tag="gT2")
            V.tensor_tensor(FF, CONTRS[0][:, 0:2 * S], CONTRS[0][:, 2 * S:4 * S], Alu.add)
            V.tensor_tensor(GGt, CONTRS[1][:, 0:2 * S], CONTRS[1][:, 2 * S:4 * S], Alu.add)
            V.tensor_tensor(FF, FF, GGt, Alu.add)
            V.tensor_tensor(SAB, sl(FF, 0), sl(FF, 1), Alu.add)
            dump("SAB", SAB)
            # ---- smooth-L1 + BCE ----
            with tc.tile_pool(name="sl1", bufs=1) as ps:
                ZBP = ps.tile([P, 4 * S], BF, name="ZBP", tag="ZBP")
                ZBT = ps.tile([P, 4 * S], BF, name="ZBT", tag="ZBT")
                nc.sync.dma_start(out=ZBP, in_=d_zbp[:, :])
                nc.sync.dma_start(out=ZBT, in_=d_zbt[:, :])
                D = ps.tile([P, 4 * S], BF, name="D", tag="D")
                AD = ps.tile([P, 4 * S], BF, name="AD", tag="AD")
                V.tensor_tensor(D, ZBP, ZBT, Alu.subtract)
                A.activation(AD, D, Act.Abs)
                M = ps.tile([P, 4 * S], BF, name="M", tag="ZBP")
                MD = ps.tile([P, 4 * S], BF, name="MD", tag="ZBT")
                V.tensor_scalar(M, AD, 1.0, None, Alu.min)
                V.tensor_tensor(MD, M, AD, Alu.mult)
                M2H = ps.tile([P, 4 * S], BF, name="M2H", tag="D")
                SL1 = ps.tile([P, 4 * S], BF, name="SL1", tag="AD")
                A.activation(M2H, M, Act.Square, scale=float(np.sqrt(0.5)))
                V.tensor_tensor(SL1, MD, M2H, Alu.subtract)
                V.tensor_tensor(SL1, SL1, bc(W, 0, 4), Alu.mult)
                V.tensor_reduce(ZACC, SL1.rearrange("p (c f) -> p c f", c=4),
                                AX_X, Alu.add)
                V.tensor_copy(ACCS[:, 2:3], ZACC[:, 0:1])
                V.tensor_copy(ACCS[:, 3:4], ZACC[:, 1:2])
                V.tensor_tensor(ACCS[:, 4:5], ZACC[:, 2:3], ZACC[:, 3:4], Alu.add)
                dump("SL1", SL1)

            with tc.tile_pool(name="bce", bufs=1) as pb:
                AXb = pb.tile([P, S], BF, name="AXb")
                SP = pb.tile([P, S], BF, name="SP")
                RL = pb.tile([P, S], BF, name="RL")
                XY = pb.tile([P, S], BF, name="XY")
                A.activation(AXb, IOUP, Act.Abs)
                EB = pb.tile([P, S], BF, name="EB")
                A.activation(EB, AXb, Act.Exp, scale=-1.0)
                A.activation(SP, EB, Act.Ln, bias=1.0)
                A.activation(RL, IOUP, Act.Relu)
                V.tensor_tensor(XY, IOUP, IOUT, Alu.mult)
                V.tensor_tensor(RL, RL, XY, Alu.subtract)
                V.tensor_tensor(RL, RL, SP, Alu.add)
                V.tensor_tensor(RL, RL, W, Alu.mult)
                V.tensor_reduce(ACCS[:, 5:6], RL, AX_X, Alu.add)
                dump("BCE", RL)


        # ============ focal part 1: folds, mask-select, pt ============
        with tc.tile_pool(name="focal", bufs=1) as pf:
            IDX10 = pf.tile([P, 10 * S], BF, name="IDX10")
            for c in range(10):
                G.memset(sl(IDX10, c), float(c))
            MK10 = pf.tile([P, 10 * S], BF, name="MK10")
            V.tensor_tensor(MK10, IDX10, bc(CTF, 0, 10), Alu.is_equal)
            EM = pf.tile([P, 10 * S], BF, name="EM")
            V.tensor_tensor(EM, E, MK10, Alu.mult)
            F1 = pf.tile([P, 5 * S], BF, name="F1")
            V.tensor_tensor(F1, E[:, 0:5 * S], E[:, 5 * S:10 * S], Alu.add)
            V.tensor_tensor(F1[:, 0:2 * S], F1[:, 0:2 * S], F1[:, 2 * S:4 * S], Alu.add)
            Ssum = pf.tile([P, S], BF, name="Ssum")
            V.tensor_tensor(Ssum, sl(F1, 0), sl(F1, 1), Alu.add)
            V.tensor_tensor(Ssum, Ssum, sl(F1, 4), Alu.add)
            F2 = pf.tile([P, 5 * S], BF, name="F2", tag="IDXr")
            V.tensor_tensor(F2, EM[:, 0:5 * S], EM[:, 5 * S:10 * S], Alu.add)
            V.tensor_tensor(F2[:, 0:2 * S], F2[:, 0:2 * S], F2[:, 2 * S:4 * S], Alu.add)
            ET = pf.tile([P, S], BF, name="ET")
            V.tensor_tensor(ET, sl(F2, 0), sl(F2, 1), Alu.add)
            V.tensor_tensor(ET, ET, sl(F2, 4), Alu.add)
            Ssf = pf.tile([P, S], F32, name="Ssf")
            V.tensor_copy(Ssf, Ssum)
            RSf = pf.tile([P, S], F32, name="RSf")
            V.reciprocal_approx_fast(RSf, Ssf)
            RS = pf.tile([P, S], BF, name="RS")
            A.copy(RS, RSf)
            PT = pf.tile([P, S], BF, name="PT")
            V.tensor_tensor(PT, ET, RS, Alu.mult)
            V.tensor_scalar(PT, PT, EPS, 1.0 - EPS, Alu.max, Alu.min)
            LG = pf.tile([P, S], BF, name="LG")
            A.activation(LG, PT, Act.Ln)

            # ============ v term part 1 (both quads at 2S width) ============
            with tc.tile_pool(name="vterm", bufs=1) as pv:
                rlf = pv.tile([P, 2 * S], F32, name="rlf")
                rwf = pv.tile([P, 2 * S], F32, name="rwf")
                V.tensor_scalar(rlf, l2, EPS, None, Alu.add)
                V.reciprocal_approx_fast(rlf, rlf)
                V.tensor_scalar(rwf, w2, 1e-30, None, Alu.add)
                V.reciprocal_approx_fast(rwf, rwf)
                rl = pv.tile([P, 2 * S], BF, name="rl")
                rw = pv.tile([P, 2 * S], BF, name="rw")
                A.copy(rl, rlf)
                A.copy(rw, rwf)
                x1 = pv.tile([P, 2 * S], BF, name="x1")
                x2 = pv.tile([P, 2 * S], BF, name="x2")
                V.tensor_tensor(x1, w2, rl, Alu.mult)
                V.tensor_tensor(x2, l2, rw, Alu.mult)
                mn = pv.tile([P, 2 * S], BF, name="mn")
                V.tensor_tensor(mn, x1, x2, Alu.min)
                aa = pv.tile([P, 2 * S], BF, name="aa")
                A.activation(aa, mn, Act.Arctan)

                # ============ extents -> c2, d2 (covers the arctan) ============
                with tc.tile_pool(name="d2c2", bufs=1) as pd:
                    exts = []
                    for (CT, op, nm) in ((CORX, Alu.max, "xmax"), (CORX, Alu.min, "xmin"),
                                         (CORY, Alu.max, "ymax"), (CORY, Alu.min, "ymin")):
                        f1 = pd.tile([P, 4 * S], BF, name=f"f1_{nm}", tag="f1")
                        V.tensor_tensor(f1, CT[:, 0:4 * S], CT[:, 4 * S:8 * S], op)
                        V.tensor_tensor(f1[:, 0:2 * S], f1[:, 0:2 * S], f1[:, 2 * S:4 * S], op)
                        ex = pd.tile([P, S], BF, name=f"ext_{nm}", tag=f"ext_{nm}")
                        V.tensor_tensor(ex, sl(f1, 0), sl(f1, 1), op)
                        exts.append(ex)
                    BW = pd.tile([P, S], BF, name="BW")
                    BH = pd.tile([P, S], BF, name="BH")
                    V.tensor_tensor(BW, exts[0], exts[1], Alu.subtract)
                    V.tensor_tensor(BH, exts[2], exts[3], Alu.subtract)
                    SQW = pd.tile([P, S], BF, name="SQW", tag="f1")
                    SQH = pd.tile([P, S], BF, name="SQH", tag="sqh")
                    A.activation(SQW, BW, Act.Square)
                    A.activation(SQH, BH, Act.Square)
                    C2 = pd.tile([P, S], F32, name="C2")
                    V.scalar_tensor_tensor(C2, SQW, EPS, SQH, Alu.add, Alu.add)
                    RC2f = pd.tile([P, S], F32, name="RC2f")
                    V.reciprocal_approx_fast(RC2f, C2)
                    RC2 = pd.tile([P, S], BF, name="RC2")
                    A.copy(RC2, RC2f)
                    # d2: (cxP-cxT)^2 + (cyP-cyT)^2 via one 2S-wide pass
                    DXY = pd.tile([P, 2 * S], BF, name="DXY")
                    V.tensor_tensor(DXY, strided(GEO, 0, 2), strided(GEO, 1, 2),
                                    Alu.subtract)
                    SQ2 = pd.tile([P, 2 * S], BF, name="SQ2")
                    A.activation(SQ2, DXY, Act.Square)
                    D2 = pd.tile([P, S], BF, name="D2t")
                    V.tensor_tensor(D2, sl(SQ2, 0), sl(SQ2, 1), Alu.add)
                    V.tensor_tensor(D2C2, D2, RC2, Alu.mult)
                    dump("D2C2", D2C2)

                # ============ v term part 2 ============
                m8 = pv.tile([P, 2 * S], U8, name="m8")
                V.tensor_scalar(m8, x1, 1.0, None, Alu.is_gt)
                tt2 = pv.tile([P, 2 * S], BF, name="tt2")
                V.tensor_scalar(tt2, aa, -1.0, PI2, Alu.mult, Alu.add)
                AT = pv.tile([P, 2 * S], BF, name="AT")
                A.copy(AT, aa)
                V.copy_predicated(AT, m8, tt2)
                DV = pv.tile([P, S], BF, name="DV")
                V.tensor_tensor(DV, sl(AT, 0), sl(AT, 1), Alu.subtract)
                A.activation(Vv, DV, Act.Square, scale=float(2.0 / np.pi))
                dump("Vv", Vv)

            # ============ iou + bev assembly, focal tail interleaved ============
            with tc.tile_pool(name="asm", bufs=1) as pa:
                INTER = pa.tile([P, S], BF, name="INTER")
                A.activation(INTER, SAB, Act.Abs, scale=0.5)
                AR2 = pa.tile([P, 2 * S], BF, name="AR2")
                V.tensor_tensor(AR2, l2, w2, Alu.mult)
                UN = pa.tile([P, S], BF, name="UN")
                V.tensor_tensor(UN, sl(AR2, 0), sl(AR2, 1), Alu.add)
                V.tensor_tensor(UN, UN, INTER, Alu.subtract)
                UNf = pa.tile([P, S], F32, name="UNf")
                V.tensor_scalar(UNf, UN, EPS, None, Alu.max)
                URCf = pa.tile([P, S], F32, name="URCf")
                V.reciprocal_approx_fast(URCf, UNf)
                URC = pa.tile([P, S], BF, name="URC")
                A.copy(URC, URCf)
                # focal tail filler while URC/OMI round-trip the Act engine
                OMP = pf.tile([P, S], BF, name="OMP")
                V.tensor_scalar(OMP, PT, -1.0, 1.0, Alu.mult, Alu.add)
                MPOSF = pf.tile([P, S], BF, name="MPOSF")
                V.tensor_scalar(MPOSF, CTF, 0.0, None, Alu.is_gt)
                ALPHn = pf.tile([P, S], BF, name="ALPHn")
                # negated alpha_t: 0.5*mpos - 0.75  (cls sum negated; host flips)
                V.tensor_scalar(ALPHn, MPOSF, 0.5, -0.75, Alu.mult, Alu.add)
                IOU = pa.tile([P, S], BF, name="IOU")
                V.tensor_tensor(IOU, INTER, URC, Alu.mult)
                OMI = pa.tile([P, S], BF, name="OMI")
                A.activation(OMI, IOU, Act.Relu, scale=-1.0, bias=1.0)
                FL = pf.tile([P, S], BF, name="FL")
                V.tensor_tensor(FL, OMP, OMP, Alu.mult)
                V.tensor_tensor(FL, FL, LG, Alu.mult)
                DEN = pa.tile([P, S], F32, name="DEN")
                V.scalar_tensor_tensor(DEN, OMI, EPS, Vv, Alu.add, Alu.add)
                DRCf = pa.tile([P, S], F32, name="DRCf")
                V.reciprocal_approx_fast(DRCf, DEN)
                DRC = pa.tile([P, S], BF, name="DRC")
                A.copy(DRC, DRCf)
                V.tensor_tensor(FL, FL, ALPHn, Alu.mult)
                V.tensor_reduce(ACCS[:, 0:1], FL, AX_X, Alu.add)
                ALC = pa.tile([P, S], BF, name="ALC")
                V.tensor_tensor(ALC, Vv, DRC, Alu.mult)
                V.tensor_tensor(ALC, ALC, Vv, Alu.mult)
                LB = pa.tile([P, S], BF, name="LB")
                V.tensor_tensor(LB, OMI, D2C2, Alu.add)
                V.tensor_tensor(LB, LB, ALC, Alu.add)
                V.tensor_tensor(LB, LB, W, Alu.mult)
                V.tensor_reduce(ACCS[:, 1:2], LB, AX_X, Alu.add)
                dump("IOU", IOU)
                dump("LBW", LB)
            dump("PT", PT)

        A.memzero(ACCS[:, 7:8])
        nc.sync.dma_start(out=d_out[:, :], in_=ACCS)

    nc.compile()
    nc._dbg_outs = dbg_outs
    return nc


def _get_nc():
    if "nc" not in _CACHE:
        _ensure_ntff_hook()
        _CACHE["nc"] = _build()
    return _CACHE["nc"]


def _prep_core(cls_b, regp_b, regt_b, ioup_b, iout_b, ct_b, w_b):
    """Build one core's input map (bf16, [P, C*S] layouts) from f32 [C,H,W]."""
    import ml_dtypes
    BF = ml_dtypes.bfloat16

    def chans(x, idxs):
        # x [C,H,W] -> [P, len(idxs)*S] slot-interleaved
        sel = x[idxs].reshape(len(idxs), P, S)
        return np.ascontiguousarray(sel.transpose(1, 0, 2).reshape(P, len(idxs) * S)).astype(BF)

    def geo_interleave(xp, xt, idxs):
        # slots [cP, cT] per channel: [P, 2*len(idxs)*S]
        selp = xp[idxs].reshape(len(idxs), P, S)
        selt = xt[idxs].reshape(len(idxs), P, S)
        inter = np.stack([selp, selt], axis=1)  # [C, 2, P, S]
        return np.ascontiguousarray(
            inter.transpose(2, 0, 1, 3).reshape(P, 2 * len(idxs) * S)).astype(BF)

    return {
        "cls": chans(cls_b, list(range(10))),
        "geo": geo_interleave(regp_b, regt_b, [0, 1, 3, 4]),
        "geoy": geo_interleave(regp_b, regt_b, [6]),
        "zbp": chans(regp_b, [2, 5, 7, 8]),
        "zbt": chans(regt_b, [2, 5, 7, 8]),
        "ioup": ioup_b.reshape(P, S).astype(BF),
        "iout": iout_b.reshape(P, S).astype(BF),
        "ctf": ct_b.reshape(P, S).astype(np.float32).astype(BF),
        "w": w_b.reshape(P, S).astype(BF),
    }


def kernel(**inputs):
    from concourse.bass_utils import run_bass_kernel_spmd

    nc = _get_nc()
    cls_pred = np.asarray(inputs["cls_pred"], dtype=np.float32)
    reg_pred = np.asarray(inputs["reg_pred"], dtype=np.float32)
    iou_pred = np.asarray(inputs["iou_pred"], dtype=np.float32)
    cls_targets = np.asarray(inputs["cls_targets"], dtype=np.int32)
    reg_targets = np.asarray(inputs["reg_targets"], dtype=np.float32)
    reg_weights = np.asarray(inputs["reg_weights"], dtype=np.float32)
    iou_targets = np.asarray(inputs["iou_targets"], dtype=np.float32)

    B = cls_pred.shape[0]
    in_maps = []
    for b in range(B):
        in_maps.append(_prep_core(cls_pred[b], reg_pred[b], reg_targets[b],
                                  iou_pred[b], iou_targets[b],
                                  cls_targets[b], reg_weights[b]))
    res = run_bass_kernel_spmd(nc, in_maps, core_ids=list(range(8)))
    _CACHE["last_result"] = res
    sums = np.zeros(8, np.float64)
    for r in res.results:
        sums += r["out"].astype(np.float64).sum(axis=0)
    num_pos = max(sums[6], 1.0)
    out = np.array([sums[0], sums[1], sums[2], sums[3], sums[4], sums[5]],
                   np.float64) / num_pos
    return out.astype(np.float32)


# revision 15
# speedup vs baseline: 2.7036x; 1.0214x over previous
"""Trainium2 Bass kernel for nn_DetectionBEVLoss (bf16 pipeline, v3).

Takes FULL inputs (B=8,...), shards batch across 8 NeuronCores (one batch
element per core), computes per-core partial sums of the 6 loss terms plus the
positive count on-device, and finishes the tiny reduction on host.

Key optimizations over the f32 baseline:
 - host casts inputs to bf16 and pre-arranges [P, C*S] layouts (half the DMA,
   no on-device transposes/casts; bf16 doubles DVE tensor_tensor throughput)
 - pred/target channels interleaved on host so per-quad element ops run at
   2S width (halves per-instruction fixed overhead); corner tiles likewise
   interleaved, giving a contiguous 8-way extent fold
 - rotated rects are parallelograms: opposite edges are +/-E, so each clip
   pass needs only 2 shared cross-product tensors G (not 4), and each edge
   pair forms a slab whose inside-interval is [min(U0,U2), max(U0,U2)]
 - Green's theorem: the boundary contribution of a clipped segment is
   (t1-t0)*cross(S_k, S_{k+1}); no intersection points are materialized
 - 1/C1 = C1*exp(-ln(C1^2+1e-30)) evaluated on the idle Act engine (sign
   comes out automatically, exact-0 C1 yields 0 -> no NaNs)
 - both passes' G/C1 issued before the U stages with sl1/BCE vector work
   slotted in between, hiding the Act-chain latency; focal split around the
   extent/vterm work for the same reason
 - scalar_tensor_tensor avoided in hot @4S ops (STT doesn't get bf16 2x
   packing on the DVE; plain TT does) via pre-negated cross tiles
 - alpha_c denominator computed as relu(1-iou)+v to survive bf16 rounding
"""
import numpy as np

P = 128
S = 512          # free-dim elements per channel slot (65536 px per core)
NPX = P * S
EPS = 1e-7

_CACHE = {}


def _ensure_ntff_hook():
    import sys, types
    if "antenv.axon_hooks" in sys.modules:
        return
    try:
        import trn_agent_boot.trn_boot as tb
        hook = tb._ntff_profile_via_ctypes('/opt/axon/libaxon_pjrt.so')
        mod = types.ModuleType("antenv.axon_hooks")
        mod.get_axon_ntff_profile_hook = lambda: hook
        sys.modules["antenv.axon_hooks"] = mod
    except Exception:
        pass


def _build(debug=False, lvl=99):
    import concourse.bacc as bacc
    import concourse.tile as tile
    import concourse.mybir as mybir
    import concourse.bass as bass

    F32 = mybir.dt.float32
    BF = mybir.dt.bfloat16
    U8 = mybir.dt.uint8
    Alu = mybir.AluOpType
    Act = mybir.ActivationFunctionType
    AX_X = mybir.AxisListType.X
    PI2 = float(np.pi / 2)

    nc = bacc.Bacc("TRN2", target_bir_lowering=False, debug=False, num_devices=8)

    for v in [PI2, 1e-30, 1.0]:
        t = nc.alloc_sbuf_tensor(f"const-f32-{v}", [P, 1], F32)
        nc.gpsimd.memset(t.ap(), v)
        nc.const_aps.aps[(F32, v)] = t.ap()
    nc.all_engine_barrier()

    # GEO layout (interleaved pred/target):
    #   slots: 0 cxP 1 cxT 2 cyP 3 cyT 4 lP 5 lT 6 wP 7 wT 8 yawP 9 yawT
    d_geoy = nc.dram_tensor("geoy", [P, 2 * S], BF, kind="ExternalInput")
    d_geo = nc.dram_tensor("geo", [P, 8 * S], BF, kind="ExternalInput")
    d_cls = nc.dram_tensor("cls", [P, 10 * S], BF, kind="ExternalInput")
    d_zbp = nc.dram_tensor("zbp", [P, 4 * S], BF, kind="ExternalInput")
    d_zbt = nc.dram_tensor("zbt", [P, 4 * S], BF, kind="ExternalInput")
    d_ioup = nc.dram_tensor("ioup", [P, S], BF, kind="ExternalInput")
    d_iout = nc.dram_tensor("iout", [P, S], BF, kind="ExternalInput")
    d_ctf = nc.dram_tensor("ctf", [P, S], BF, kind="ExternalInput")
    d_w = nc.dram_tensor("w", [P, S], BF, kind="ExternalInput")
    d_out = nc.dram_tensor("out", [P, 8], F32, kind="ExternalOutput")

    V = nc.vector
    A = nc.scalar
    G = nc.gpsimd

    dbg_outs = []

    def dump(name, t):
        if not debug:
            return
        shp = [t.shape[0], int(np.prod(t.shape[1:]))]
        d = nc.dram_tensor(f"dbg_{name}", shp, t.dtype, kind="ExternalOutput")
        nc.sync.dma_start(out=d[:, :], in_=t)
        dbg_outs.append(name)

    def bc(t, i, k):
        # broadcast S-slice i of tile t over k slots
        b_ = t[:, i * S:(i + 1) * S]
        return bass.AP(tensor=b_.tensor, offset=b_.offset,
                       ap=[b_.ap[0], [0, k], [1, S]])

    def strided(t, start, num, step=2):
        # [P][num][S] view of S-slots start, start+step, ... of tile t
        b_ = t[:, start * S:(start + 1) * S]
        return bass.AP(tensor=b_.tensor, offset=b_.offset,
                       ap=[b_.ap[0], [step * S, num], [1, S]])

    def sl(t, i, k=1):
        return t[:, i * S:(i + k) * S]

    with tile.TileContext(nc) as tc:
      with tc.tile_pool(name="persist", bufs=1) as pp:
        ACCS = pp.tile([P, 8], F32, name="ACCS")
        ZACC = pp.tile([P, 4], F32, name="ZACC")
        W = pp.tile([P, S], BF, name="W")
        CTF = pp.tile([P, S], BF, name="CTF")
        IOUP = pp.tile([P, S], BF, name="IOUP")
        IOUT = pp.tile([P, S], BF, name="IOUT")
        GEO = pp.tile([P, 10 * S], BF, name="GEO")
        # corner tiles, interleaved: slot 2k = quad A corner k, 2k+1 = quad B
        CORX = pp.tile([P, 8 * S], BF, name="CORX")
        CORY = pp.tile([P, 8 * S], BF, name="CORY")
        # crosses, interleaved like the corners: slot 2j+q = quad q, edge j
        CAB = pp.tile([P, 8 * S], BF, name="CAB")
        CABn = pp.tile([P, 8 * S], BF, name="CABn")
        # uv smalls, interleaved: 0 uxP 1 uxT 2 vxP 3 vxT 4 uyP 5 uyT 6 vyP 7 vyT
        UVT = pp.tile([P, 8 * S], BF, name="UVT")
        SAB = pp.tile([P, S], BF, name="SAB")
        Vv = pp.tile([P, S], BF, name="Vv")
        D2C2 = pp.tile([P, S], BF, name="D2C2")

        GY = GEO[:, 8 * S:10 * S]
        l2 = sl(GEO, 4, 2); w2 = sl(GEO, 6, 2)

        # yaw first (Sin dep), then l/w (UV-product dep), then the rest
        nc.sync.dma_start(out=GY, in_=d_geoy[:, :])
        nc.sync.dma_start(out=GEO[:, 4 * S:8 * S], in_=d_geo[:, 4 * S:8 * S])
        nc.sync.dma_start(out=W, in_=d_w[:, :])
        nc.sync.dma_start(out=GEO[:, 0:4 * S], in_=d_geo[:, 0:4 * S])
        nc.sync.dma_start(out=CTF, in_=d_ctf[:, :])
        nc.sync.dma_start(out=IOUP, in_=d_ioup[:, :])
        nc.sync.dma_start(out=IOUT, in_=d_iout[:, :])

        # ============ corners + uv smalls (both quads at 2S width) ============
        with tc.tile_pool(name="corn", bufs=1) as pc:
            co2 = pc.tile([P, 2 * S], BF, name="co2")
            si2 = pc.tile([P, 2 * S], BF, name="si2")
            A.activation(co2, GY, Act.Sin, bias=PI2)
            A.activation(si2, GY, Act.Sin)
            UX2 = sl(UVT, 0, 2); VX2 = sl(UVT, 2, 2)
            UY2 = sl(UVT, 4, 2); VY2 = sl(UVT, 6, 2)
            V.tensor_tensor(UX2, l2, co2, Alu.mult)
            V.tensor_tensor(VX2, w2, si2, Alu.mult)
            V.tensor_tensor(UY2, l2, si2, Alu.mult)
            V.tensor_tensor(VY2, w2, co2, Alu.mult)
            V.tensor_reduce(ACCS[:, 6:7], W, AX_X, Alu.add)
            As2 = pc.tile([P, 2 * S], BF, name="As2")
            Ad2 = pc.tile([P, 2 * S], BF, name="Ad2")
            Ps2 = pc.tile([P, 2 * S], BF, name="Ps2")
            Pd2 = pc.tile([P, 2 * S], BF, name="Pd2")
            V.tensor_tensor(As2, UX2, VX2, Alu.add)
            V.tensor_tensor(Ad2, UX2, VX2, Alu.subtract)
            V.tensor_tensor(Ps2, UY2, VY2, Alu.add)
            V.tensor_tensor(Pd2, UY2, VY2, Alu.subtract)
            cx2 = sl(GEO, 0, 2); cy2 = sl(GEO, 2, 2)
            V.scalar_tensor_tensor(sl(CORX, 0, 2), As2, 0.5, cx2, Alu.mult, Alu.add)
            V.scalar_tensor_tensor(sl(CORX, 2, 2), Ad2, -0.5, cx2, Alu.mult, Alu.add)
            V.scalar_tensor_tensor(sl(CORX, 4, 2), As2, -0.5, cx2, Alu.mult, Alu.add)
            V.scalar_tensor_tensor(sl(CORX, 6, 2), Ad2, 0.5, cx2, Alu.mult, Alu.add)
            V.scalar_tensor_tensor(sl(CORY, 0, 2), Pd2, 0.5, cy2, Alu.mult, Alu.add)
            V.scalar_tensor_tensor(sl(CORY, 2, 2), Ps2, -0.5, cy2, Alu.mult, Alu.add)
            V.scalar_tensor_tensor(sl(CORY, 4, 2), Pd2, -0.5, cy2, Alu.mult, Alu.add)
            V.scalar_tensor_tensor(sl(CORY, 6, 2), Ps2, 0.5, cy2, Alu.mult, Alu.add)
            dump("CORX", CORX); dump("CORY", CORY)

            # crosses CR_{q,k} = cross(S_k, S_{k+1}), both quads jointly
            T1 = pc.tile([P, 8 * S], BF, name="crT1")
            T2 = pc.tile([P, 8 * S], BF, name="crT2")
            V.tensor_tensor(T1[:, 0:6 * S], CORX[:, 0:6 * S], CORY[:, 2 * S:8 * S], Alu.mult)
            V.tensor_tensor(T1[:, 6 * S:8 * S], CORX[:, 6 * S:8 * S], CORY[:, 0:2 * S], Alu.mult)
            V.tensor_tensor(T2[:, 0:6 * S], CORY[:, 0:6 * S], CORX[:, 2 * S:8 * S], Alu.mult)
            V.tensor_tensor(T2[:, 6 * S:8 * S], CORY[:, 6 * S:8 * S], CORX[:, 0:2 * S], Alu.mult)
            V.tensor_tensor(CAB, T1, T2, Alu.subtract)
            V.tensor_tensor(CABn, T2, T1, Alu.subtract)
            dump("CAB", CAB)

        # ============ clip passes: G + C1 for both passes first ============
        # pass 0: segments A (even corner slots, crosses CA), constraints B
        # pass 1: segments B (odd slots), constraints A
        # uv slice index of (ux, vx, uy, vy) for quad q: (0+q, 2+q, 4+q, 6+q)
        # pass tuples: (corner slot parity, CS slot base, CQ parity)
        PASSES = ((0, 1), (1, 0))
        with tc.tile_pool(name="clip", bufs=1) as pcl:
            CLS = pcl.tile([P, 10 * S], BF, name="CLS")
            nc.sync.dma_start(out=CLS, in_=d_cls[:, :])
            G5s = {}
            C1s = {}
            RS2s = {}
            for pi, (sq, qq) in enumerate(PASSES):
                SX = strided(CORX, sq, 4)
                SY = strided(CORY, sq, 4)
                for pair in range(2):
                    g5 = pcl.tile([P, 5 * S], BF, name=f"G5_{pi}_{pair}")
                    Gt = g5[:, 0:4 * S]
                    T1 = pcl.tile([P, 4 * S], BF, name=f"gT1_{pi}_{pair}", tag="gT1")
                    T2 = pcl.tile([P, 4 * S], BF, name=f"gT2_{pi}_{pair}", tag="gT2")
                    if pair == 0:
                        # E0 = (-ux, -uy): G = uy*SX - ux*SY
                        V.tensor_tensor(T1, bc(UVT, 4 + qq, 4), SX, Alu.mult)
                        V.tensor_tensor(T2, bc(UVT, 0 + qq, 4), SY, Alu.mult)
                        V.tensor_tensor(Gt, T1, T2, Alu.subtract)
                    else:
                        # E1 = (-vx, +vy): G = -(vx*SY + vy*SX); store Gneg
                        V.tensor_tensor(T1, bc(UVT, 2 + qq, 4), SY, Alu.mult)
                        V.tensor_tensor(T2, bc(UVT, 6 + qq, 4), SX, Alu.mult)
                        V.tensor_tensor(Gt, T1, T2, Alu.add)
                    A.copy(g5[:, 4 * S:5 * S], g5[:, 0:S])
                    C1 = pcl.tile([P, 4 * S], BF, name=f"C1_{pi}_{pair}")
                    if pair == 0:
                        V.tensor_tensor(C1, g5[:, S:5 * S], g5[:, 0:4 * S], Alu.subtract)
                    else:
                        # G stored negated: C1_true = Gneg_k - Gneg_{k+1}
                        V.tensor_tensor(C1, g5[:, 0:4 * S], g5[:, S:5 * S], Alu.subtract)
                    G5s[(pi, pair)] = g5
                    C1s[(pi, pair)] = C1
                # per-pass Act chain so pass pi's RECs are ready while the
                # vector engine builds pass pi+1's G/C1
                for pair in range(2):
                    RS2 = pcl.tile([P, 4 * S], BF, name=f"RS2_{pi}_{pair}")
                    A.activation(RS2, C1s[(pi, pair)], Act.Square)
                    RS2s[(pi, pair)] = RS2
                for pair in range(2):
                    A.activation(RS2s[(pi, pair)], RS2s[(pi, pair)], Act.Ln, bias=1e-30)
                for pair in range(2):
                    A.activation(RS2s[(pi, pair)], RS2s[(pi, pair)], Act.Exp, scale=-1.0)
            # focal exp rides the already-loaded exp table
            E = pp.tile([P, 10 * S], BF, name="E")
            A.activation(E, CLS, Act.Exp)

            # ---- U, slab intervals, contributions ----
            CONTRS = []
            for pi, (sq, qq) in enumerate(PASSES):
                LOHI = []
                for pair in range(2):
                    Gt = G5s[(pi, pair)][:, 0:4 * S]
                    C1 = C1s[(pi, pair)]
                    RS2 = RS2s[(pi, pair)]
                    REC = pcl.tile([P, 4 * S], BF, name=f"REC_{pi}_{pair}", tag=f"REC_{pair}")
                    V.tensor_tensor(REC, C1, RS2, Alu.mult)
                    j0, j2 = (0, 2) if pair == 0 else (1, 3)
                    T0g = pcl.tile([P, 4 * S], BF, name=f"T0g_{pi}_{pair}", tag="gT1")
                    U0 = pcl.tile([P, 4 * S], BF, name=f"U0_{pi}_{pair}", tag=f"U0_{pair}")
                    T2g = pcl.tile([P, 4 * S], BF, name=f"T2g_{pi}_{pair}", tag="gT2")
                    U2 = pcl.tile([P, 4 * S], BF, name=f"U2_{pi}_{pair}", tag=f"U2_{pair}")
                    if pair == 0:
                        # U0 = (-CQ_j0 - G)*REC ; U2 = (CQ_j2 - G)*REC
                        V.tensor_tensor(T0g, bc(CABn, 2 * j0 + qq, 4), Gt, Alu.subtract)
                        V.tensor_tensor(T2g, bc(CAB, 2 * j2 + qq, 4), Gt, Alu.subtract)
                    else:
                        # G stored negated: U0 = (Gneg - CQ_j0)*REC
                        #                   U2 = (Gneg + CQ_j2)*REC
                        V.tensor_tensor(T0g, Gt, bc(CAB, 2 * j0 + qq, 4), Alu.subtract)
                        V.tensor_tensor(T2g, Gt, bc(CAB, 2 * j2 + qq, 4), Alu.add)
                    V.tensor_tensor(U0, T0g, REC, Alu.mult)
                    V.tensor_tensor(U2, T2g, REC, Alu.mult)
                    lo = pcl.tile([P, 4 * S], BF, name=f"lo_{pi}_{pair}", tag=f"lo_{pair}")
                    hi = pcl.tile([P, 4 * S], BF, name=f"hi_{pi}_{pair}", tag=f"hi_{pair}")
                    V.tensor_tensor(lo, U0, U2, Alu.min)
                    V.tensor_tensor(hi, U0, U2, Alu.max)
                    LOHI.append((lo, hi))
                LO = LOHI[0][0]; HI = LOHI[0][1]
                V.tensor_tensor(LO, LO, LOHI[1][0], Alu.max)
                V.tensor_tensor(HI, HI, LOHI[1][1], Alu.min)
                T0 = pcl.tile([P, 4 * S], BF, name=f"T0_{pi}", tag="gT1")
                T1v = pcl.tile([P, 4 * S], BF, name=f"T1v_{pi}", tag="gT2")
                V.tensor_scalar(T0, LO, 0.0, 1.0, Alu.max, Alu.min)
                V.tensor_scalar(T1v, HI, 1.0, 0.0, Alu.min, Alu.max)
                DT = pcl.tile([P, 4 * S], BF, name=f"DT_{pi}", tag="U0_0")
                V.tensor_tensor(DT, T1v, T0, Alu.subtract)
                V.tensor_scalar(DT, DT, 0.0, None, Alu.max)
                CONTR = pcl.tile([P, 4 * S], BF, name=f"CONTR_{pi}", tag=f"CONTR_{pi}")
                V.tensor_tensor(CONTR, DT, strided(CAB, sq, 4), Alu.mult)
                CONTRS.append(CONTR)
                dump(f"CONTR_{pi}", CONTR)
            # joint fold of both passes' contributions
            FF = pcl.tile([P, 2 * S], BF, name="FF", tag="gT1")
            GGt = pcl.tile([P, 2 * S], BF, name="GGt", tag="gT2")
            V.tensor_tensor(FF, CONTRS[0][:, 0:2 * S], CONTRS[0][:, 2 * S:4 * S], Alu.add)
            V.tensor_tensor(GGt, CONTRS[1][:, 0:2 * S], CONTRS[1][:, 2 * S:4 * S], Alu.add)
            V.tensor_tensor(FF, FF, GGt, Alu.add)
            V.tensor_tensor(SAB, sl(FF, 0), sl(FF, 1), Alu.add)
            dump("SAB", SAB)
            # ---- smooth-L1 + BCE ----
            with tc.tile_pool(name="sl1", bufs=1) as ps:
                ZBP = ps.tile([P, 4 * S], BF, name="ZBP", tag="ZBP")
                ZBT = ps.tile([P, 4 * S], BF, name="ZBT", tag="ZBT")
                nc.sync.dma_start(out=ZBP, in_=d_zbp[:, :])
                nc.sync.dma_start(out=ZBT, in_=d_zbt[:, :])
                D = ps.tile([P, 4 * S], BF, name="D", tag="D")
                AD = ps.tile([P, 4 * S], BF, name="AD", tag="AD")
                V.tensor_tensor(D, ZBP, ZBT, Alu.subtract)
                A.activation(AD, D, Act.Abs)
                M = ps.tile([P, 4 * S], BF, name="M", tag="ZBP")
                MD = ps.tile([P, 4 * S], BF, name="MD", tag="ZBT")
                V.tensor_scalar(M, AD, 1.0, None, Alu.min)
                V.tensor_tensor(MD, M, AD, Alu.mult)
                M2H = ps.tile([P, 4 * S], BF, name="M2H", tag="D")
                SL1 = ps.tile([P, 4 * S], BF, name="SL1", tag="AD")
                A.activation(M2H, M, Act.Square, scale=float(np.sqrt(0.5)))
                V.tensor_tensor(SL1, MD, M2H, Alu.subtract)
                V.tensor_tensor(SL1, SL1, bc(W, 0, 4), Alu.mult)
                V.tensor_reduce(ZACC, SL1.rearrange("p (c f) -> p c f", c=4),
                                AX_X, Alu.add)
                V.tensor_copy(ACCS[:, 2:3], ZACC[:, 0:1])
                V.tensor_copy(ACCS[:, 3:4], ZACC[:, 1:2])
                V.tensor_tensor(ACCS[:, 4:5], ZACC[:, 2:3], ZACC[:, 3:4], Alu.add)
                dump("SL1", SL1)

            with tc.tile_pool(name="bce", bufs=1) as pb:
                AXb = pb.tile([P, S], BF, name="AXb")
                SP = pb.tile([P, S], BF, name="SP")
                RL = pb.tile([P, S], BF, name="RL")
                XY = pb.tile([P, S], BF, name="XY")
                A.activation(AXb, IOUP, Act.Abs)
                EB = pb.tile([P, S], BF, name="EB")
                A.activation(EB, AXb, Act.Exp, scale=-1.0)
                A.activation(SP, EB, Act.Ln, bias=1.0)
                A.activation(RL, IOUP, Act.Relu)
                V.tensor_tensor(XY, IOUP, IOUT, Alu.mult)
                V.tensor_tensor(RL, RL, XY, Alu.subtract)
                V.tensor_tensor(RL, RL, SP, Alu.add)
                V.tensor_tensor(RL, RL, W, Alu.mult)
                V.tensor_reduce(ACCS[:, 5:6], RL, AX_X, Alu.add)
                dump("BCE", RL)


        # ============ focal part 1: folds, mask-select, pt ============
        with tc.tile_pool(name="focal", bufs=1) as pf:
            IDX10 = pf.tile([P, 10 * S], BF, name="IDX10")
            for c in range(10):
                G.memset(sl(IDX10, c), float(c))
            MK10 = pf.tile([P, 10 * S], BF, name="MK10")
            V.tensor_tensor(MK10, IDX10, bc(CTF, 0, 10), Alu.is_equal)
            EM = pf.tile([P, 10 * S], BF, name="EM")
            V.tensor_tensor(EM, E, MK10, Alu.mult)
            F1 = pf.tile([P, 5 * S], BF, name="F1")
            V.tensor_tensor(F1, E[:, 0:5 * S], E[:, 5 * S:10 * S], Alu.add)
            V.tensor_tensor(F1[:, 0:2 * S], F1[:, 0:2 * S], F1[:, 2 * S:4 * S], Alu.add)
            Ssum = pf.tile([P, S], BF, name="Ssum")
            V.tensor_tensor(Ssum, sl(F1, 0), sl(F1, 1), Alu.add)
            V.tensor_tensor(Ssum, Ssum, sl(F1, 4), Alu.add)
            F2 = pf.tile([P, 5 * S], BF, name="F2", tag="IDXr")
            V.tensor_tensor(F2, EM[:, 0:5 * S], EM[:, 5 * S:10 * S], Alu.add)
            V.tensor_tensor(F2[:, 0:2 * S], F2[:, 0:2 * S], F2[:, 2 * S:4 * S], Alu.add)
            ET = pf.tile([P, S], BF, name="ET")
            V.tensor_tensor(ET, sl(F2, 0), sl(F2, 1), Alu.add)
            V.tensor_tensor(ET, ET, sl(F2, 4), Alu.add)
            Ssf = pf.tile([P, S], F32, name="Ssf")
            V.tensor_copy(Ssf, Ssum)
            RSf = pf.tile([P, S], F32, name="RSf")
            V.reciprocal_approx_fast(RSf, Ssf)
            RS = pf.tile([P, S], BF, name="RS")
            A.copy(RS, RSf)
            PT = pf.tile([P, S], BF, name="PT")
            V.tensor_tensor(PT, ET, RS, Alu.mult)
            V.tensor_scalar(PT, PT, EPS, 1.0 - EPS, Alu.max, Alu.min)
            LG = pf.tile([P, S], BF, name="LG")
            A.activation(LG, PT, Act.Ln)

            # ============ v term part 1 (both quads at 2S width) ============
            with tc.tile_pool(name="vterm", bufs=1) as pv:
                rlf = pv.tile([P, 2 * S], F32, name="rlf")
                rwf = pv.tile([P, 2 * S], F32, name="rwf")
                V.tensor_scalar(rlf, l2, EPS, None, Alu.add)
                V.reciprocal_approx_fast(rlf, rlf)
                V.tensor_scalar(rwf, w2, 1e-30, None, Alu.add)
                V.reciprocal_approx_fast(rwf, rwf)
                rl = pv.tile([P, 2 * S], BF, name="rl")
                rw = pv.tile([P, 2 * S], BF, name="rw")
                A.copy(rl, rlf)
                A.copy(rw, rwf)
                x1 = pv.tile([P, 2 * S], BF, name="x1")
                x2 = pv.tile([P, 2 * S], BF, name="x2")
                V.tensor_tensor(x1, w2, rl, Alu.mult)
                V.tensor_tensor(x2, l2, rw, Alu.mult)
                mn = pv.tile([P, 2 * S], BF, name="mn")
                V.tensor_tensor(mn, x1, x2, Alu.min)
                aa = pv.tile([P, 2 * S], BF, name="aa")
                A.activation(aa, mn, Act.Arctan)

                # ============ extents -> c2, d2 (covers the arctan) ============
                with tc.tile_pool(name="d2c2", bufs=1) as pd:
                    exts = []
                    for (CT, op, nm) in ((CORX, Alu.max, "xmax"), (CORX, Alu.min, "xmin"),
                                         (CORY, Alu.max, "ymax"), (CORY, Alu.min, "ymin")):
                        f1 = pd.tile([P, 4 * S], BF, name=f"f1_{nm}", tag="f1")
                        V.tensor_tensor(f1, CT[:, 0:4 * S], CT[:, 4 * S:8 * S], op)
                        V.tensor_tensor(f1[:, 0:2 * S], f1[:, 0:2 * S], f1[:, 2 * S:4 * S], op)
                        ex = pd.tile([P, S], BF, name=f"ext_{nm}", tag=f"ext_{nm}")
                        V.tensor_tensor(ex, sl(f1, 0), sl(f1, 1), op)
                        exts.append(ex)
                    BW = pd.tile([P, S], BF, name="BW")
                    BH = pd.tile([P, S], BF, name="BH")
                    V.tensor_tensor(BW, exts[0], exts[1], Alu.subtract)
                    V.tensor_tensor(BH, exts[2], exts[3], Alu.subtract)
                    SQW = pd.tile([P, S], BF, name="SQW", tag="f1")
                    SQH = pd.tile([P, S], BF, name="SQH", tag="sqh")
                    A.activation(SQW, BW, Act.Square)
                    A.activation(SQH, BH, Act.Square)
                    C2 = pd.tile([P, S], F32, name="C2")
                    V.scalar_tensor_tensor(C2, SQW, EPS, SQH, Alu.add, Alu.add)
                    RC2f = pd.tile([P, S], F32, name="RC2f")
                    V.reciprocal_approx_fast(RC2f, C2)
                    RC2 = pd.tile([P, S], BF, name="RC2")
                    A.copy(RC2, RC2f)
                    # d2: (cxP-cxT)^2 + (cyP-cyT)^2 via one 2S-wide pass
                    DXY = pd.tile([P, 2 * S], BF, name="DXY")
                    V.tensor_tensor(DXY, strided(GEO, 0, 2), strided(GEO, 1, 2),
                                    Alu.subtract)
                    SQ2 = pd.tile([P, 2 * S], BF, name="SQ2")
                    A.activation(SQ2, DXY, Act.Square)
                    D2 = pd.tile([P, S], BF, name="D2t")
                    V.tensor_tensor(D2, sl(SQ2, 0), sl(SQ2, 1), Alu.add)
                    V.tensor_tensor(D2C2, D2, RC2, Alu.mult)
                    dump("D2C2", D2C2)

                # ============ v term part 2 ============
                m8 = pv.tile([P, 2 * S], U8, name="m8")
                V.tensor_scalar(m8, x1, 1.0, None, Alu.is_gt)
                tt2 = pv.tile([P, 2 * S], BF, name="tt2")
                V.tensor_scalar(tt2, aa, -1.0, PI2, Alu.mult, Alu.add)
                AT = pv.tile([P, 2 * S], BF, name="AT")
                A.copy(AT, aa)
                V.copy_predicated(AT, m8, tt2)
                DV = pv.tile([P, S], BF, name="DV")
                V.tensor_tensor(DV, sl(AT, 0), sl(AT, 1), Alu.subtract)
                A.activation(Vv, DV, Act.Square, scale=float(2.0 / np.pi))
                dump("Vv", Vv)

            # ============ iou + bev assembly, focal tail interleaved ============
            with tc.tile_pool(name="asm", bufs=1) as pa:
                INTER = pa.tile([P, S], BF, name="INTER")
                A.activation(INTER, SAB, Act.Abs, scale=0.5)
                AR2 = pa.tile([P, 2 * S], BF, name="AR2")
                V.tensor_tensor(AR2, l2, w2, Alu.mult)
                UN = pa.tile([P, S], BF, name="UN")
                V.tensor_tensor(UN, sl(AR2, 0), sl(AR2, 1), Alu.add)
                V.tensor_tensor(UN, UN, INTER, Alu.subtract)
                UNf = pa.tile([P, S], F32, name="UNf")
                V.tensor_scalar(UNf, UN, EPS, None, Alu.max)
                URCf = pa.tile([P, S], F32, name="URCf")
                V.reciprocal_approx_fast(URCf, UNf)
                URC = pa.tile([P, S], BF, name="URC")
                A.copy(URC, URCf)
                # focal tail filler while URC/OMI round-trip the Act engine
                OMP = pf.tile([P, S], BF, name="OMP")
                V.tensor_scalar(OMP, PT, -1.0, 1.0, Alu.mult, Alu.add)
                MPOSF = pf.tile([P, S], BF, name="MPOSF")
                V.tensor_scalar(MPOSF, CTF, 0.0, None, Alu.is_gt)
                ALPHn = pf.tile([P, S], BF, name="ALPHn")
                # negated alpha_t: 0.5*mpos - 0.75  (cls sum negated; host flips)
                V.tensor_scalar(ALPHn, MPOSF, 0.5, -0.75, Alu.mult, Alu.add)
                IOU = pa.tile([P, S], BF, name="IOU")
                V.tensor_tensor(IOU, INTER, URC, Alu.mult)
                OMI = pa.tile([P, S], BF, name="OMI")
                A.activation(OMI, IOU, Act.Relu, scale=-1.0, bias=1.0)
                FL = pf.tile([P, S], BF, name="FL")
                V.tensor_tensor(FL, OMP, OMP, Alu.mult)
                V.tensor_tensor(FL, FL, LG, Alu.mult)
                DEN = pa.tile([P, S], F32, name="DEN")
                V.scalar_tensor_tensor(DEN, OMI, EPS, Vv, Alu.add, Alu.add)
                DRCf = pa.tile([P, S], F32, name="DRCf")
                V.reciprocal_approx_fast(DRCf, DEN)
                DRC = pa.tile([P, S], BF, name="DRC")
                A.copy(DRC, DRCf)
                V.tensor_tensor(FL, FL, ALPHn, Alu.mult)
                V.tensor_reduce(ACCS[:, 0:1], FL, AX_X, Alu.add)
                ALC = pa.tile([P, S], BF, name="ALC")
                V.tensor_tensor(ALC, Vv, DRC, Alu.mult)
                V.tensor_tensor(ALC, ALC, Vv, Alu.mult)
                LB = pa.tile([P, S], BF, name="LB")
                V.tensor_tensor(LB, OMI, D2C2, Alu.add)
                V.tensor_tensor(LB, LB, ALC, Alu.add)
                V.tensor_tensor(LB, LB, W, Alu.mult)
                V.tensor_reduce(ACCS[:, 1:2], LB, AX_X, Alu.add)
                dump("IOU", IOU)
                dump("LBW", LB)
            dump("PT", PT)

        A.memzero(ACCS[:, 7:8])
        nc.sync.dma_start(out=d_out[:, :], in_=ACCS)

    nc.compile()
    nc._dbg_outs = dbg_outs
    return nc


def _get_nc():
    if "nc" not in _CACHE:
        _ensure_ntff_hook()
        _CACHE["nc"] = _build()
    return _CACHE["nc"]


def _prep_core(cls_b, regp_b, regt_b, ioup_b, iout_b, ct_b, w_b):
    """Build one core's input map (bf16, [P, C*S] layouts) from f32 [C,H,W]."""
    import ml_dtypes
    BF = ml_dtypes.bfloat16

    def chans(x, idxs):
        # x [C,H,W] -> [P, len(idxs)*S] slot-interleaved
        sel = x[idxs].reshape(len(idxs), P, S)
        return np.ascontiguousarray(sel.transpose(1, 0, 2).reshape(P, len(idxs) * S)).astype(BF)

    def geo_interleave(xp, xt, idxs):
        # slots [cP, cT] per channel: [P, 2*len(idxs)*S]
        selp = xp[idxs].reshape(len(idxs), P, S)
        selt = xt[idxs].reshape(len(idxs), P, S)
        inter = np.stack([selp, selt], axis=1)  # [C, 2, P, S]
        return np.ascontiguousarray(
            inter.transpose(2, 0, 1, 3).reshape(P, 2 * len(idxs) * S)).astype(BF)

    return {
        "cls": chans(cls_b, list(range(10))),
        "geo": geo_interleave(regp_b, regt_b, [0, 1, 3, 4]),
        "geoy": geo_interleave(regp_b, regt_b, [6]),
        "zbp": chans(regp_b, [2, 5, 7, 8]),
        "zbt": chans(regt_b, [2, 5, 7, 8]),
        "ioup": ioup_b.reshape(P, S).astype(BF),
        "iout": iout_b.reshape(P, S).astype(BF),
        "ctf": ct_b.reshape(P, S).astype(np.float32).astype(BF),
        "w": w_b.reshape(P, S).astype(BF),
    }


def kernel(**inputs):
    from concourse.bass_utils import run_bass_kernel_spmd

    nc = _get_nc()
    cls_pred = np.asarray(inputs["cls_pred"], dtype=np.float32)
    reg_pred = np.asarray(inputs["reg_pred"], dtype=np.float32)
    iou_pred = np.asarray(inputs["iou_pred"], dtype=np.float32)
    cls_targets = np.asarray(inputs["cls_targets"], dtype=np.int32)
    reg_targets = np.asarray(inputs["reg_targets"], dtype=np.float32)
    reg_weights = np.asarray(inputs["reg_weights"], dtype=np.float32)
    iou_targets = np.asarray(inputs["iou_targets"], dtype=np.float32)

    B = cls_pred.shape[0]
    in_maps = []
    for b in range(B):
        in_maps.append(_prep_core(cls_pred[b], reg_pred[b], reg_targets[b],
                                  iou_pred[b], iou_targets[b],
                                  cls_targets[b], reg_weights[b]))
    res = run_bass_kernel_spmd(nc, in_maps, core_ids=list(range(8)))
    _CACHE["last_result"] = res
    sums = np.zeros(8, np.float64)
    for r in res.results:
        sums += r["out"].astype(np.float64).sum(axis=0)
    num_pos = max(sums[6], 1.0)
    out = np.array([sums[0], sums[1], sums[2], sums[3], sums[4], sums[5]],
                   np.float64) / num_pos
    return out.astype(np.float32)


# revision 16
# speedup vs baseline: 2.7057x; 1.0008x over previous
"""Trainium2 Bass kernel for nn_DetectionBEVLoss (bf16 pipeline, v3).

Takes FULL inputs (B=8,...), shards batch across 8 NeuronCores (one batch
element per core), computes per-core partial sums of the 6 loss terms plus the
positive count on-device, and finishes the tiny reduction on host.

Key optimizations over the f32 baseline:
 - host casts inputs to bf16 and pre-arranges [P, C*S] layouts (half the DMA,
   no on-device transposes/casts; bf16 doubles DVE tensor_tensor throughput)
 - pred/target channels interleaved on host so per-quad element ops run at
   2S width (halves per-instruction fixed overhead); corner tiles likewise
   interleaved, giving a contiguous 8-way extent fold
 - rotated rects are parallelograms: opposite edges are +/-E, so each clip
   pass needs only 2 shared cross-product tensors G (not 4), and each edge
   pair forms a slab whose inside-interval is [min(U0,U2), max(U0,U2)]
 - Green's theorem: the boundary contribution of a clipped segment is
   (t1-t0)*cross(S_k, S_{k+1}); no intersection points are materialized
 - 1/C1 = C1*exp(-ln(C1^2+1e-30)) evaluated on the idle Act engine (sign
   comes out automatically, exact-0 C1 yields 0 -> no NaNs)
 - both passes' G/C1 issued before the U stages with sl1/BCE vector work
   slotted in between, hiding the Act-chain latency; focal split around the
   extent/vterm work for the same reason
 - scalar_tensor_tensor avoided in hot @4S ops (STT doesn't get bf16 2x
   packing on the DVE; plain TT does) via pre-negated cross tiles
 - alpha_c denominator computed as relu(1-iou)+v to survive bf16 rounding
"""
import numpy as np

P = 128
S = 512          # free-dim elements per channel slot (65536 px per core)
NPX = P * S
EPS = 1e-7

_CACHE = {}


def _ensure_ntff_hook():
    import sys, types
    if "antenv.axon_hooks" in sys.modules:
        return
    try:
        import trn_agent_boot.trn_boot as tb
        hook = tb._ntff_profile_via_ctypes('/opt/axon/libaxon_pjrt.so')
        mod = types.ModuleType("antenv.axon_hooks")
        mod.get_axon_ntff_profile_hook = lambda: hook
        sys.modules["antenv.axon_hooks"] = mod
    except Exception:
        pass


def _build(debug=False, lvl=99):
    import concourse.bacc as bacc
    import concourse.tile as tile
    import concourse.mybir as mybir
    import concourse.bass as bass

    F32 = mybir.dt.float32
    BF = mybir.dt.bfloat16
    U8 = mybir.dt.uint8
    Alu = mybir.AluOpType
    Act = mybir.ActivationFunctionType
    AX_X = mybir.AxisListType.X
    PI2 = float(np.pi / 2)

    nc = bacc.Bacc("TRN2", target_bir_lowering=False, debug=False, num_devices=8)

    for v in [PI2, 1e-30, 1.0]:
        t = nc.alloc_sbuf_tensor(f"const-f32-{v}", [P, 1], F32)
        nc.gpsimd.memset(t.ap(), v)
        nc.const_aps.aps[(F32, v)] = t.ap()
    nc.all_engine_barrier()

    # GEO layout (interleaved pred/target):
    #   slots: 0 cxP 1 cxT 2 cyP 3 cyT 4 lP 5 lT 6 wP 7 wT 8 yawP 9 yawT
    d_geoy = nc.dram_tensor("geoy", [P, 2 * S], BF, kind="ExternalInput")
    d_geo = nc.dram_tensor("geo", [P, 8 * S], BF, kind="ExternalInput")
    d_cls = nc.dram_tensor("cls", [P, 10 * S], BF, kind="ExternalInput")
    d_zbp = nc.dram_tensor("zbp", [P, 4 * S], BF, kind="ExternalInput")
    d_zbt = nc.dram_tensor("zbt", [P, 4 * S], BF, kind="ExternalInput")
    d_ioup = nc.dram_tensor("ioup", [P, S], BF, kind="ExternalInput")
    d_iout = nc.dram_tensor("iout", [P, S], BF, kind="ExternalInput")
    d_ctf = nc.dram_tensor("ctf", [P, S], BF, kind="ExternalInput")
    d_w = nc.dram_tensor("w", [P, S], BF, kind="ExternalInput")
    d_out = nc.dram_tensor("out", [P, 8], F32, kind="ExternalOutput")

    V = nc.vector
    A = nc.scalar
    G = nc.gpsimd

    dbg_outs = []

    def dump(name, t):
        if not debug:
            return
        shp = [t.shape[0], int(np.prod(t.shape[1:]))]
        d = nc.dram_tensor(f"dbg_{name}", shp, t.dtype, kind="ExternalOutput")
        nc.sync.dma_start(out=d[:, :], in_=t)
        dbg_outs.append(name)

    def bc(t, i, k):
        # broadcast S-slice i of tile t over k slots
        b_ = t[:, i * S:(i + 1) * S]
        return bass.AP(tensor=b_.tensor, offset=b_.offset,
                       ap=[b_.ap[0], [0, k], [1, S]])

    def strided(t, start, num, step=2):
        # [P][num][S] view of S-slots start, start+step, ... of tile t
        b_ = t[:, start * S:(start + 1) * S]
        return bass.AP(tensor=b_.tensor, offset=b_.offset,
                       ap=[b_.ap[0], [step * S, num], [1, S]])

    def sl(t, i, k=1):
        return t[:, i * S:(i + k) * S]

    with tile.TileContext(nc) as tc:
      with tc.tile_pool(name="persist", bufs=1) as pp:
        ACCS = pp.tile([P, 8], F32, name="ACCS")
        ZACC = pp.tile([P, 4], F32, name="ZACC")
        W = pp.tile([P, S], BF, name="W")
        CTF = pp.tile([P, S], BF, name="CTF")
        IOUP = pp.tile([P, S], BF, name="IOUP")
        IOUT = pp.tile([P, S], BF, name="IOUT")
        GEO = pp.tile([P, 10 * S], BF, name="GEO")
        # corner tiles, interleaved: slot 2k = quad A corner k, 2k+1 = quad B
        CORX = pp.tile([P, 8 * S], BF, name="CORX")
        CORY = pp.tile([P, 8 * S], BF, name="CORY")
        # crosses, interleaved like the corners: slot 2j+q = quad q, edge j
        CAB = pp.tile([P, 8 * S], BF, name="CAB")
        CABn = pp.tile([P, 8 * S], BF, name="CABn")
        # uv smalls, interleaved: 0 uxP 1 uxT 2 vxP 3 vxT 4 uyP 5 uyT 6 vyP 7 vyT
        UVT = pp.tile([P, 8 * S], BF, name="UVT")
        SAB = pp.tile([P, S], BF, name="SAB")
        Vv = pp.tile([P, S], BF, name="Vv")
        D2C2 = pp.tile([P, S], BF, name="D2C2")

        GY = GEO[:, 8 * S:10 * S]
        l2 = sl(GEO, 4, 2); w2 = sl(GEO, 6, 2)

        # yaw first (Sin dep), then l/w (UV-product dep), then the rest
        nc.sync.dma_start(out=GY, in_=d_geoy[:, :])
        nc.sync.dma_start(out=GEO[:, 4 * S:8 * S], in_=d_geo[:, 4 * S:8 * S])
        nc.sync.dma_start(out=W, in_=d_w[:, :])
        nc.sync.dma_start(out=GEO[:, 0:4 * S], in_=d_geo[:, 0:4 * S])
        nc.sync.dma_start(out=CTF, in_=d_ctf[:, :])
        nc.sync.dma_start(out=IOUP, in_=d_ioup[:, :])
        nc.sync.dma_start(out=IOUT, in_=d_iout[:, :])

        # ============ corners + uv smalls (both quads at 2S width) ============
        with tc.tile_pool(name="corn", bufs=1) as pc:
            co2 = pc.tile([P, 2 * S], BF, name="co2")
            si2 = pc.tile([P, 2 * S], BF, name="si2")
            A.activation(co2, GY, Act.Sin, bias=PI2)
            A.activation(si2, GY, Act.Sin)
            UX2 = sl(UVT, 0, 2); VX2 = sl(UVT, 2, 2)
            UY2 = sl(UVT, 4, 2); VY2 = sl(UVT, 6, 2)
            V.tensor_tensor(UX2, l2, co2, Alu.mult)
            V.tensor_tensor(VX2, w2, si2, Alu.mult)
            V.tensor_tensor(UY2, l2, si2, Alu.mult)
            V.tensor_tensor(VY2, w2, co2, Alu.mult)
            V.tensor_reduce(ACCS[:, 6:7], W, AX_X, Alu.add)
            As2 = pc.tile([P, 2 * S], BF, name="As2")
            Ad2 = pc.tile([P, 2 * S], BF, name="Ad2")
            Ps2 = pc.tile([P, 2 * S], BF, name="Ps2")
            Pd2 = pc.tile([P, 2 * S], BF, name="Pd2")
            V.tensor_tensor(As2, UX2, VX2, Alu.add)
            V.tensor_tensor(Ad2, UX2, VX2, Alu.subtract)
            V.tensor_tensor(Ps2, UY2, VY2, Alu.add)
            V.tensor_tensor(Pd2, UY2, VY2, Alu.subtract)
            cx2 = sl(GEO, 0, 2); cy2 = sl(GEO, 2, 2)
            V.scalar_tensor_tensor(sl(CORX, 0, 2), As2, 0.5, cx2, Alu.mult, Alu.add)
            V.scalar_tensor_tensor(sl(CORX, 2, 2), Ad2, -0.5, cx2, Alu.mult, Alu.add)
            V.scalar_tensor_tensor(sl(CORX, 4, 2), As2, -0.5, cx2, Alu.mult, Alu.add)
            V.scalar_tensor_tensor(sl(CORX, 6, 2), Ad2, 0.5, cx2, Alu.mult, Alu.add)
            V.scalar_tensor_tensor(sl(CORY, 0, 2), Pd2, 0.5, cy2, Alu.mult, Alu.add)
            V.scalar_tensor_tensor(sl(CORY, 2, 2), Ps2, -0.5, cy2, Alu.mult, Alu.add)
            V.scalar_tensor_tensor(sl(CORY, 4, 2), Pd2, -0.5, cy2, Alu.mult, Alu.add)
            V.scalar_tensor_tensor(sl(CORY, 6, 2), Ps2, 0.5, cy2, Alu.mult, Alu.add)
            dump("CORX", CORX); dump("CORY", CORY)

            # crosses CR_{q,k} = cross(S_k, S_{k+1}), both quads jointly
            T1 = pc.tile([P, 8 * S], BF, name="crT1")
            T2 = pc.tile([P, 8 * S], BF, name="crT2")
            V.tensor_tensor(T1[:, 0:6 * S], CORX[:, 0:6 * S], CORY[:, 2 * S:8 * S], Alu.mult)
            V.tensor_tensor(T1[:, 6 * S:8 * S], CORX[:, 6 * S:8 * S], CORY[:, 0:2 * S], Alu.mult)
            V.tensor_tensor(T2[:, 0:6 * S], CORY[:, 0:6 * S], CORX[:, 2 * S:8 * S], Alu.mult)
            V.tensor_tensor(T2[:, 6 * S:8 * S], CORY[:, 6 * S:8 * S], CORX[:, 0:2 * S], Alu.mult)
            V.tensor_tensor(CAB, T1, T2, Alu.subtract)
            V.tensor_tensor(CABn, T2, T1, Alu.subtract)
            dump("CAB", CAB)

        # ============ clip passes: G + C1 for both passes first ============
        # pass 0: segments A (even corner slots, crosses CA), constraints B
        # pass 1: segments B (odd slots), constraints A
        # uv slice index of (ux, vx, uy, vy) for quad q: (0+q, 2+q, 4+q, 6+q)
        # pass tuples: (corner slot parity, CS slot base, CQ parity)
        PASSES = ((0, 1), (1, 0))
        with tc.tile_pool(name="clip", bufs=1) as pcl:
            CLS = pcl.tile([P, 10 * S], BF, name="CLS")
            nc.sync.dma_start(out=CLS, in_=d_cls[:, :])
            G5s = {}
            C1s = {}
            RS2s = {}
            for pi, (sq, qq) in enumerate(PASSES):
                SX = strided(CORX, sq, 4)
                SY = strided(CORY, sq, 4)
                for pair in range(2):
                    g5 = pcl.tile([P, 5 * S], BF, name=f"G5_{pi}_{pair}")
                    Gt = g5[:, 0:4 * S]
                    T1 = pcl.tile([P, 4 * S], BF, name=f"gT1_{pi}_{pair}", tag="gT1")
                    T2 = pcl.tile([P, 4 * S], BF, name=f"gT2_{pi}_{pair}", tag="gT2")
                    if pair == 0:
                        # E0 = (-ux, -uy): G = uy*SX - ux*SY
                        V.tensor_tensor(T1, bc(UVT, 4 + qq, 4), SX, Alu.mult)
                        V.tensor_tensor(T2, bc(UVT, 0 + qq, 4), SY, Alu.mult)
                        V.tensor_tensor(Gt, T1, T2, Alu.subtract)
                    else:
                        # E1 = (-vx, +vy): G = -(vx*SY + vy*SX); store Gneg
                        V.tensor_tensor(T1, bc(UVT, 2 + qq, 4), SY, Alu.mult)
                        V.tensor_tensor(T2, bc(UVT, 6 + qq, 4), SX, Alu.mult)
                        V.tensor_tensor(Gt, T1, T2, Alu.add)
                    A.copy(g5[:, 4 * S:5 * S], g5[:, 0:S])
                    C1 = pcl.tile([P, 4 * S], BF, name=f"C1_{pi}_{pair}")
                    if pair == 0:
                        V.tensor_tensor(C1, g5[:, S:5 * S], g5[:, 0:4 * S], Alu.subtract)
                    else:
                        # G stored negated: C1_true = Gneg_k - Gneg_{k+1}
                        V.tensor_tensor(C1, g5[:, 0:4 * S], g5[:, S:5 * S], Alu.subtract)
                    G5s[(pi, pair)] = g5
                    C1s[(pi, pair)] = C1
                    # start the Act chain for this pair immediately
                    RS2 = pcl.tile([P, 4 * S], BF, name=f"RS2_{pi}_{pair}")
                    A.activation(RS2, C1, Act.Square)
                    RS2s[(pi, pair)] = RS2
                # per-pass Ln/Exp so pass pi's RECs are ready while the
                # vector engine builds pass pi+1's G/C1
                for pair in range(2):
                    A.activation(RS2s[(pi, pair)], RS2s[(pi, pair)], Act.Ln, bias=1e-30)
                for pair in range(2):
                    A.activation(RS2s[(pi, pair)], RS2s[(pi, pair)], Act.Exp, scale=-1.0)
            # focal exp rides the already-loaded exp table
            E = pp.tile([P, 10 * S], BF, name="E")
            A.activation(E, CLS, Act.Exp)

            # ---- U, slab intervals, contributions ----
            CONTRS = []
            for pi, (sq, qq) in enumerate(PASSES):
                LOHI = []
                for pair in range(2):
                    Gt = G5s[(pi, pair)][:, 0:4 * S]
                    C1 = C1s[(pi, pair)]
                    RS2 = RS2s[(pi, pair)]
                    REC = pcl.tile([P, 4 * S], BF, name=f"REC_{pi}_{pair}", tag=f"REC_{pair}")
                    V.tensor_tensor(REC, C1, RS2, Alu.mult)
                    j0, j2 = (0, 2) if pair == 0 else (1, 3)
                    T0g = pcl.tile([P, 4 * S], BF, name=f"T0g_{pi}_{pair}", tag="gT1")
                    U0 = pcl.tile([P, 4 * S], BF, name=f"U0_{pi}_{pair}", tag=f"U0_{pair}")
                    T2g = pcl.tile([P, 4 * S], BF, name=f"T2g_{pi}_{pair}", tag="gT2")
                    U2 = pcl.tile([P, 4 * S], BF, name=f"U2_{pi}_{pair}", tag=f"U2_{pair}")
                    if pair == 0:
                        # U0 = (-CQ_j0 - G)*REC ; U2 = (CQ_j2 - G)*REC
                        V.tensor_tensor(T0g, bc(CABn, 2 * j0 + qq, 4), Gt, Alu.subtract)
                        V.tensor_tensor(T2g, bc(CAB, 2 * j2 + qq, 4), Gt, Alu.subtract)
                    else:
                        # G stored negated: U0 = (Gneg - CQ_j0)*REC
                        #                   U2 = (Gneg + CQ_j2)*REC
                        V.tensor_tensor(T0g, Gt, bc(CAB, 2 * j0 + qq, 4), Alu.subtract)
                        V.tensor_tensor(T2g, Gt, bc(CAB, 2 * j2 + qq, 4), Alu.add)
                    V.tensor_tensor(U0, T0g, REC, Alu.mult)
                    V.tensor_tensor(U2, T2g, REC, Alu.mult)
                    lo = pcl.tile([P, 4 * S], BF, name=f"lo_{pi}_{pair}", tag=f"lo_{pair}")
                    hi = pcl.tile([P, 4 * S], BF, name=f"hi_{pi}_{pair}", tag=f"hi_{pair}")
                    V.tensor_tensor(lo, U0, U2, Alu.min)
                    V.tensor_tensor(hi, U0, U2, Alu.max)
                    LOHI.append((lo, hi))
                LO = LOHI[0][0]; HI = LOHI[0][1]
                V.tensor_tensor(LO, LO, LOHI[1][0], Alu.max)
                V.tensor_tensor(HI, HI, LOHI[1][1], Alu.min)
                T0 = pcl.tile([P, 4 * S], BF, name=f"T0_{pi}", tag="gT1")
                T1v = pcl.tile([P, 4 * S], BF, name=f"T1v_{pi}", tag="gT2")
                V.tensor_scalar(T0, LO, 0.0, 1.0, Alu.max, Alu.min)
                V.tensor_scalar(T1v, HI, 1.0, 0.0, Alu.min, Alu.max)
                DT = pcl.tile([P, 4 * S], BF, name=f"DT_{pi}", tag="U0_0")
                V.tensor_tensor(DT, T1v, T0, Alu.subtract)
                V.tensor_scalar(DT, DT, 0.0, None, Alu.max)
                CONTR = pcl.tile([P, 4 * S], BF, name=f"CONTR_{pi}", tag=f"CONTR_{pi}")
                V.tensor_tensor(CONTR, DT, strided(CAB, sq, 4), Alu.mult)
                CONTRS.append(CONTR)
                dump(f"CONTR_{pi}", CONTR)
            # joint fold of both passes' contributions
            FF = pcl.tile([P, 2 * S], BF, name="FF", tag="gT1")
            GGt = pcl.tile([P, 2 * S], BF, name="GGt", tag="gT2")
            V.tensor_tensor(FF, CONTRS[0][:, 0:2 * S], CONTRS[0][:, 2 * S:4 * S], Alu.add)
            V.tensor_tensor(GGt, CONTRS[1][:, 0:2 * S], CONTRS[1][:, 2 * S:4 * S], Alu.add)
            V.tensor_tensor(FF, FF, GGt, Alu.add)
            V.tensor_tensor(SAB, sl(FF, 0), sl(FF, 1), Alu.add)
            dump("SAB", SAB)
            # ---- smooth-L1 + BCE ----
            with tc.tile_pool(name="sl1", bufs=1) as ps:
                ZBP = ps.tile([P, 4 * S], BF, name="ZBP", tag="ZBP")
                ZBT = ps.tile([P, 4 * S], BF, name="ZBT", tag="ZBT")
                nc.sync.dma_start(out=ZBP, in_=d_zbp[:, :])
                nc.sync.dma_start(out=ZBT, in_=d_zbt[:, :])
                D = ps.tile([P, 4 * S], BF, name="D", tag="D")
                AD = ps.tile([P, 4 * S], BF, name="AD", tag="AD")
                V.tensor_tensor(D, ZBP, ZBT, Alu.subtract)
                A.activation(AD, D, Act.Abs)
                M = ps.tile([P, 4 * S], BF, name="M", tag="ZBP")
                MD = ps.tile([P, 4 * S], BF, name="MD", tag="ZBT")
                V.tensor_scalar(M, AD, 1.0, None, Alu.min)
                V.tensor_tensor(MD, M, AD, Alu.mult)
                M2H = ps.tile([P, 4 * S], BF, name="M2H", tag="D")
                SL1 = ps.tile([P, 4 * S], BF, name="SL1", tag="AD")
                A.activation(M2H, M, Act.Square, scale=float(np.sqrt(0.5)))
                V.tensor_tensor(SL1, MD, M2H, Alu.subtract)
                V.tensor_tensor(SL1, SL1, bc(W, 0, 4), Alu.mult)
                V.tensor_reduce(ZACC, SL1.rearrange("p (c f) -> p c f", c=4),
                                AX_X, Alu.add)
                V.tensor_copy(ACCS[:, 2:3], ZACC[:, 0:1])
                V.tensor_copy(ACCS[:, 3:4], ZACC[:, 1:2])
                V.tensor_tensor(ACCS[:, 4:5], ZACC[:, 2:3], ZACC[:, 3:4], Alu.add)
                dump("SL1", SL1)

            with tc.tile_pool(name="bce", bufs=1) as pb:
                AXb = pb.tile([P, S], BF, name="AXb")
                SP = pb.tile([P, S], BF, name="SP")
                RL = pb.tile([P, S], BF, name="RL")
                XY = pb.tile([P, S], BF, name="XY")
                A.activation(AXb, IOUP, Act.Abs)
                EB = pb.tile([P, S], BF, name="EB")
                A.activation(EB, AXb, Act.Exp, scale=-1.0)
                A.activation(SP, EB, Act.Ln, bias=1.0)
                A.activation(RL, IOUP, Act.Relu)
                V.tensor_tensor(XY, IOUP, IOUT, Alu.mult)
                V.tensor_tensor(RL, RL, XY, Alu.subtract)
                V.tensor_tensor(RL, RL, SP, Alu.add)
                V.tensor_tensor(RL, RL, W, Alu.mult)
                V.tensor_reduce(ACCS[:, 5:6], RL, AX_X, Alu.add)
                dump("BCE", RL)


        # ============ focal part 1: folds, mask-select, pt ============
        with tc.tile_pool(name="focal", bufs=1) as pf:
            IDX10 = pf.tile([P, 10 * S], BF, name="IDX10")
            for c in range(10):
                G.memset(sl(IDX10, c), float(c))
            MK10 = pf.tile([P, 10 * S], BF, name="MK10")
            V.tensor_tensor(MK10, IDX10, bc(CTF, 0, 10), Alu.is_equal)
            EM = pf.tile([P, 10 * S], BF, name="EM")
            V.tensor_tensor(EM, E, MK10, Alu.mult)
            F1 = pf.tile([P, 5 * S], BF, name="F1")
            V.tensor_tensor(F1, E[:, 0:5 * S], E[:, 5 * S:10 * S], Alu.add)
            V.tensor_tensor(F1[:, 0:2 * S], F1[:, 0:2 * S], F1[:, 2 * S:4 * S], Alu.add)
            Ssum = pf.tile([P, S], BF, name="Ssum")
            V.tensor_tensor(Ssum, sl(F1, 0), sl(F1, 1), Alu.add)
            V.tensor_tensor(Ssum, Ssum, sl(F1, 4), Alu.add)
            F2 = pf.tile([P, 5 * S], BF, name="F2", tag="IDXr")
            V.tensor_tensor(F2, EM[:, 0:5 * S], EM[:, 5 * S:10 * S], Alu.add)
            V.tensor_tensor(F2[:, 0:2 * S], F2[:, 0:2 * S], F2[:, 2 * S:4 * S], Alu.add)
            ET = pf.tile([P, S], BF, name="ET")
            V.tensor_tensor(ET, sl(F2, 0), sl(F2, 1), Alu.add)
            V.tensor_tensor(ET, ET, sl(F2, 4), Alu.add)
            Ssf = pf.tile([P, S], F32, name="Ssf")
            V.tensor_copy(Ssf, Ssum)
            RSf = pf.tile([P, S], F32, name="RSf")
            V.reciprocal_approx_fast(RSf, Ssf)
            RS = pf.tile([P, S], BF, name="RS")
            A.copy(RS, RSf)
            PT = pf.tile([P, S], BF, name="PT")
            V.tensor_tensor(PT, ET, RS, Alu.mult)
            V.tensor_scalar(PT, PT, EPS, 1.0 - EPS, Alu.max, Alu.min)
            LG = pf.tile([P, S], BF, name="LG")
            A.activation(LG, PT, Act.Ln)

            # ============ v term part 1 (both quads at 2S width) ============
            with tc.tile_pool(name="vterm", bufs=1) as pv:
                rlf = pv.tile([P, 2 * S], F32, name="rlf")
                rwf = pv.tile([P, 2 * S], F32, name="rwf")
                V.tensor_scalar(rlf, l2, EPS, None, Alu.add)
                V.reciprocal_approx_fast(rlf, rlf)
                V.tensor_scalar(rwf, w2, 1e-30, None, Alu.add)
                V.reciprocal_approx_fast(rwf, rwf)
                rl = pv.tile([P, 2 * S], BF, name="rl")
                rw = pv.tile([P, 2 * S], BF, name="rw")
                A.copy(rl, rlf)
                A.copy(rw, rwf)
                x1 = pv.tile([P, 2 * S], BF, name="x1")
                x2 = pv.tile([P, 2 * S], BF, name="x2")
                V.tensor_tensor(x1, w2, rl, Alu.mult)
                V.tensor_tensor(x2, l2, rw, Alu.mult)
                mn = pv.tile([P, 2 * S], BF, name="mn")
                V.tensor_tensor(mn, x1, x2, Alu.min)
                aa = pv.tile([P, 2 * S], BF, name="aa")
                A.activation(aa, mn, Act.Arctan)

                # ============ extents -> c2, d2 (covers the arctan) ============
                with tc.tile_pool(name="d2c2", bufs=1) as pd:
                    exts = []
                    for (CT, op, nm) in ((CORX, Alu.max, "xmax"), (CORX, Alu.min, "xmin"),
                                         (CORY, Alu.max, "ymax"), (CORY, Alu.min, "ymin")):
                        f1 = pd.tile([P, 4 * S], BF, name=f"f1_{nm}", tag="f1")
                        V.tensor_tensor(f1, CT[:, 0:4 * S], CT[:, 4 * S:8 * S], op)
                        V.tensor_tensor(f1[:, 0:2 * S], f1[:, 0:2 * S], f1[:, 2 * S:4 * S], op)
                        ex = pd.tile([P, S], BF, name=f"ext_{nm}", tag=f"ext_{nm}")
                        V.tensor_tensor(ex, sl(f1, 0), sl(f1, 1), op)
                        exts.append(ex)
                    BW = pd.tile([P, S], BF, name="BW")
                    BH = pd.tile([P, S], BF, name="BH")
                    V.tensor_tensor(BW, exts[0], exts[1], Alu.subtract)
                    V.tensor_tensor(BH, exts[2], exts[3], Alu.subtract)
                    SQW = pd.tile([P, S], BF, name="SQW", tag="f1")
                    SQH = pd.tile([P, S], BF, name="SQH", tag="sqh")
                    A.activation(SQW, BW, Act.Square)
                    A.activation(SQH, BH, Act.Square)
                    C2 = pd.tile([P, S], F32, name="C2")
                    V.scalar_tensor_tensor(C2, SQW, EPS, SQH, Alu.add, Alu.add)
                    RC2f = pd.tile([P, S], F32, name="RC2f")
                    V.reciprocal_approx_fast(RC2f, C2)
                    RC2 = pd.tile([P, S], BF, name="RC2")
                    A.copy(RC2, RC2f)
                    # d2: (cxP-cxT)^2 + (cyP-cyT)^2 via one 2S-wide pass
                    DXY = pd.tile([P, 2 * S], BF, name="DXY")
                    V.tensor_tensor(DXY, strided(GEO, 0, 2), strided(GEO, 1, 2),
                                    Alu.subtract)
                    SQ2 = pd.tile([P, 2 * S], BF, name="SQ2")
                    A.activation(SQ2, DXY, Act.Square)
                    D2 = pd.tile([P, S], BF, name="D2t")
                    V.tensor_tensor(D2, sl(SQ2, 0), sl(SQ2, 1), Alu.add)
                    V.tensor_tensor(D2C2, D2, RC2, Alu.mult)
                    dump("D2C2", D2C2)

                # ============ v term part 2 ============
                m8 = pv.tile([P, 2 * S], U8, name="m8")
                V.tensor_scalar(m8, x1, 1.0, None, Alu.is_gt)
                tt2 = pv.tile([P, 2 * S], BF, name="tt2")
                V.tensor_scalar(tt2, aa, -1.0, PI2, Alu.mult, Alu.add)
                AT = pv.tile([P, 2 * S], BF, name="AT")
                A.copy(AT, aa)
                V.copy_predicated(AT, m8, tt2)
                DV = pv.tile([P, S], BF, name="DV")
                V.tensor_tensor(DV, sl(AT, 0), sl(AT, 1), Alu.subtract)
                A.activation(Vv, DV, Act.Square, scale=float(2.0 / np.pi))
                dump("Vv", Vv)

            # ============ iou + bev assembly, focal tail interleaved ============
            with tc.tile_pool(name="asm", bufs=1) as pa:
                INTER = pa.tile([P, S], BF, name="INTER")
                A.activation(INTER, SAB, Act.Abs, scale=0.5)
                AR2 = pa.tile([P, 2 * S], BF, name="AR2")
                V.tensor_tensor(AR2, l2, w2, Alu.mult)
                UN = pa.tile([P, S], BF, name="UN")
                V.tensor_tensor(UN, sl(AR2, 0), sl(AR2, 1), Alu.add)
                V.tensor_tensor(UN, UN, INTER, Alu.subtract)
                UNf = pa.tile([P, S], F32, name="UNf")
                V.tensor_scalar(UNf, UN, EPS, None, Alu.max)
                URCf = pa.tile([P, S], F32, name="URCf")
                V.reciprocal_approx_fast(URCf, UNf)
                URC = pa.tile([P, S], BF, name="URC")
                A.copy(URC, URCf)
                # focal tail filler while URC/OMI round-trip the Act engine
                OMP = pf.tile([P, S], BF, name="OMP")
                V.tensor_scalar(OMP, PT, -1.0, 1.0, Alu.mult, Alu.add)
                MPOSF = pf.tile([P, S], BF, name="MPOSF")
                V.tensor_scalar(MPOSF, CTF, 0.0, None, Alu.is_gt)
                ALPHn = pf.tile([P, S], BF, name="ALPHn")
                # negated alpha_t: 0.5*mpos - 0.75  (cls sum negated; host flips)
                V.tensor_scalar(ALPHn, MPOSF, 0.5, -0.75, Alu.mult, Alu.add)
                IOU = pa.tile([P, S], BF, name="IOU")
                V.tensor_tensor(IOU, INTER, URC, Alu.mult)
                OMI = pa.tile([P, S], BF, name="OMI")
                A.activation(OMI, IOU, Act.Relu, scale=-1.0, bias=1.0)
                FL = pf.tile([P, S], BF, name="FL")
                V.tensor_tensor(FL, OMP, OMP, Alu.mult)
                V.tensor_tensor(FL, FL, LG, Alu.mult)
                DEN = pa.tile([P, S], F32, name="DEN")
                V.scalar_tensor_tensor(DEN, OMI, EPS, Vv, Alu.add, Alu.add)
                DRCf = pa.tile([P, S], F32, name="DRCf")
                V.reciprocal_approx_fast(DRCf, DEN)
                DRC = pa.tile([P, S], BF, name="DRC")
                A.copy(DRC, DRCf)
                V.tensor_tensor(FL, FL, ALPHn, Alu.mult)
                V.tensor_reduce(ACCS[:, 0:1], FL, AX_X, Alu.add)
                ALC = pa.tile([P, S], BF, name="ALC")
                V.tensor_tensor(ALC, Vv, DRC, Alu.mult)
                V.tensor_tensor(ALC, ALC, Vv, Alu.mult)
                LB = pa.tile([P, S], BF, name="LB")
                V.tensor_tensor(LB, OMI, D2C2, Alu.add)
                V.tensor_tensor(LB, LB, ALC, Alu.add)
                V.tensor_tensor(LB, LB, W, Alu.mult)
                V.tensor_reduce(ACCS[:, 1:2], LB, AX_X, Alu.add)
                dump("IOU", IOU)
                dump("LBW", LB)
            dump("PT", PT)

        A.memzero(ACCS[:, 7:8])
        nc.sync.dma_start(out=d_out[:, :], in_=ACCS)

    nc.compile()
    nc._dbg_outs = dbg_outs
    return nc


def _get_nc():
    if "nc" not in _CACHE:
        _ensure_ntff_hook()
        _CACHE["nc"] = _build()
    return _CACHE["nc"]


def _prep_core(cls_b, regp_b, regt_b, ioup_b, iout_b, ct_b, w_b):
    """Build one core's input map (bf16, [P, C*S] layouts) from f32 [C,H,W]."""
    import ml_dtypes
    BF = ml_dtypes.bfloat16

    def chans(x, idxs):
        # x [C,H,W] -> [P, len(idxs)*S] slot-interleaved
        sel = x[idxs].reshape(len(idxs), P, S)
        return np.ascontiguousarray(sel.transpose(1, 0, 2).reshape(P, len(idxs) * S)).astype(BF)

    def geo_interleave(xp, xt, idxs):
        # slots [cP, cT] per channel: [P, 2*len(idxs)*S]
        selp = xp[idxs].reshape(len(idxs), P, S)
        selt = xt[idxs].reshape(len(idxs), P, S)
        inter = np.stack([selp, selt], axis=1)  # [C, 2, P, S]
        return np.ascontiguousarray(
            inter.transpose(2, 0, 1, 3).reshape(P, 2 * len(idxs) * S)).astype(BF)

    return {
        "cls": chans(cls_b, list(range(10))),
        "geo": geo_interleave(regp_b, regt_b, [0, 1, 3, 4]),
        "geoy": geo_interleave(regp_b, regt_b, [6]),
        "zbp": chans(regp_b, [2, 5, 7, 8]),
        "zbt": chans(regt_b, [2, 5, 7, 8]),
        "ioup": ioup_b.reshape(P, S).astype(BF),
        "iout": iout_b.reshape(P, S).astype(BF),
        "ctf": ct_b.reshape(P, S).astype(np.float32).astype(BF),
        "w": w_b.reshape(P, S).astype(BF),
    }


def kernel(**inputs):
    from concourse.bass_utils import run_bass_kernel_spmd

    nc = _get_nc()
    cls_pred = np.asarray(inputs["cls_pred"], dtype=np.float32)
    reg_pred = np.asarray(inputs["reg_pred"], dtype=np.float32)
    iou_pred = np.asarray(inputs["iou_pred"], dtype=np.float32)
    cls_targets = np.asarray(inputs["cls_targets"], dtype=np.int32)
    reg_targets = np.asarray(inputs["reg_targets"], dtype=np.float32)
    reg_weights = np.asarray(inputs["reg_weights"], dtype=np.float32)
    iou_targets = np.asarray(inputs["iou_targets"], dtype=np.float32)

    B = cls_pred.shape[0]
    in_maps = []
    for b in range(B):
        in_maps.append(_prep_core(cls_pred[b], reg_pred[b], reg_targets[b],
                                  iou_pred[b], iou_targets[b],
                                  cls_targets[b], reg_weights[b]))
    res = run_bass_kernel_spmd(nc, in_maps, core_ids=list(range(8)))
    _CACHE["last_result"] = res
    sums = np.zeros(8, np.float64)
    for r in res.results:
        sums += r["out"].astype(np.float64).sum(axis=0)
    num_pos = max(sums[6], 1.0)
    out = np.array([sums[0], sums[1], sums[2], sums[3], sums[4], sums[5]],
                   np.float64) / num_pos
    return out.astype(np.float32)
